# revision 3
# baseline (speedup 1.0000x reference)
"""Trainium2 Bass kernel for a dense transformer block, distributed over 8
NeuronCores.

Sharding:
  phase 1 (attention): tensor-parallel over heads — each core computes 2 of
    the 16 heads end-to-end (QKV projections + causal softmax(QK^T)V), and
    returns the unnormalized per-head output O^T together with the softmax
    denominators (obtained via a ones-column appended to V).
  phase 2 (Wo + norms + FFN): data-parallel over tokens — each core handles
    512 of the 4096 token rows with replicated weights.

The host glues the phases: transposes x, normalizes/concats heads, and
re-shards tokens.  All matmuls run as float32r (full-rate fp32 PE mode).
"""

import math
from contextlib import ExitStack

import ml_dtypes
import numpy as np

BF_NP = ml_dtypes.bfloat16

import concourse.bass as bass
import concourse.mybir as mybir
import concourse.tile as tile
from concourse import bacc
from concourse.bass_utils import run_bass_kernel_spmd
from concourse.masks import make_identity, make_upper_triangular

FP = mybir.dt.float32
FPR = mybir.dt.float32r
BF = mybir.dt.bfloat16
AF = mybir.ActivationFunctionType

N_CORES = 8
P = 128
EPS = 1e-6

# exec times (ns) of the most recent kernel() call, one entry per phase, when
# tracing was enabled via BASS_TRACE=1; None entries otherwise.
LAST_EXEC_NS = []


def _install_ntff_hook_shim():
    """Provide antenv.axon_hooks when the image lacks it, so trace=True can
    drive NTFF profiling through libaxon_pjrt's C ABI (same contract as
    trn_boot's step-6 hook). No-op if the real module exists or the .so is
    missing/old."""
    try:
        import antenv.axon_hooks  # noqa: F401
        return
    except ImportError:
        pass
    import contextlib
    import ctypes
    import sys
    import types

    try:
        lib = ctypes.CDLL("/opt/axon/libaxon_pjrt.so")
    except OSError:
        return
    if not hasattr(lib, "axon_start_nrt_profile"):
        return
    lib.axon_start_nrt_profile.argtypes = [
        ctypes.POINTER(ctypes.c_int64), ctypes.c_size_t]
    lib.axon_start_nrt_profile.restype = ctypes.c_int64
    lib.axon_stop_nrt_profile.argtypes = [ctypes.c_char_p]
    lib.axon_stop_nrt_profile.restype = ctypes.c_int64

    @contextlib.contextmanager
    def _hook(output_dir, device_ids):
        import jax
        jax.devices()
        if device_ids:
            ids = (ctypes.c_int64 * len(device_ids))(*device_ids)
            rc = lib.axon_start_nrt_profile(ids, len(device_ids))
        else:
            rc = lib.axon_start_nrt_profile(None, 0)
        if rc != 0:
            raise RuntimeError(f"axon_start_nrt_profile rc={rc}")
        try:
            yield
        finally:
            n = lib.axon_stop_nrt_profile(str(output_dir).encode())
            if n < 0:
                raise RuntimeError(f"axon_stop_nrt_profile rc={n}")

    mod = types.ModuleType("antenv.axon_hooks")
    mod.get_axon_ntff_profile_hook = lambda: _hook

    def set_axon_ntff_profile_hook(h):
        mod.get_axon_ntff_profile_hook = lambda: h

    mod.set_axon_ntff_profile_hook = set_axon_ntff_profile_hook
    import antenv
    antenv.axon_hooks = mod
    sys.modules["antenv.axon_hooks"] = mod


def _fpr(ap):
    return ap.bitcast(FPR)


# --------------------------------------------------------------------------
# phase 1: per-core attention over a pair of heads
# --------------------------------------------------------------------------

def build_phase1(B, T, C, DH):
    HP = 2                      # heads per core
    DA = DH + 1                 # head dim + ones row (softmax denominator)
    NCC = C // P                # contraction chunks
    NT = T // P                 # key/value blocks of 128
    NQ = T // 512               # query chunks of 512
    NK = T // 1024              # query tiles of 1024
    scale = float(C) ** -0.5    # NOTE: reference scales by C**-0.5, not DH

    nc = bacc.Bacc("TRN2", debug=False)
    xT_d = nc.dram_tensor("xT", [B, C, T], BF, kind="ExternalInput").ap()
    wq_d = nc.dram_tensor("wq", [C, HP * DH], BF, kind="ExternalInput").ap()
    wk_d = nc.dram_tensor("wk", [C, HP * DH], BF, kind="ExternalInput").ap()
    wv_d = nc.dram_tensor("wv", [C, HP * DH], BF, kind="ExternalInput").ap()
    ot_d = nc.dram_tensor("ot", [B, HP, DA, T], FP, kind="ExternalOutput").ap()

    with tile.TileContext(nc) as tc, ExitStack() as ctx:
        const = ctx.enter_context(tc.tile_pool(name="const", bufs=1))
        xpool = ctx.enter_context(tc.tile_pool(name="xp", bufs=1))
        wpool = ctx.enter_context(tc.tile_pool(name="wp", bufs=1))
        qk_pool = ctx.enter_context(tc.tile_pool(name="qk", bufs=2))
        vt_pool = ctx.enter_context(tc.tile_pool(name="vtp", bufs=2))
        vaug_pool = ctx.enter_context(tc.tile_pool(name="vaug", bufs=2))
        pt_pool = ctx.enter_context(tc.tile_pool(name="pt", bufs=4))
        ot_pool = ctx.enter_context(tc.tile_pool(name="otp", bufs=2))

        # additive mask for the diagonal 128x128 block of S^T [s', q']:
        # 0 where q' >= s' (causal-valid), -1e30 where q' < s'
        negmask = const.tile([P, P], FP)
        nc.gpsimd.memset(negmask[:], 0.0)
        nc.gpsimd.affine_select(
            out=negmask[:], in_=negmask[:],
            compare_op=mybir.AluOpType.is_ge, fill=-1e30,
            base=0, pattern=[[1, P]], channel_multiplier=-1)
        ident = const.tile([P, P], BF)
        make_identity(nc, ident[:])
        ones_col = const.tile([P, NT * HP, 1], FP)
        nc.vector.memset(ones_col[:], 1.0)

        # weight chunks, loaded once
        wts = {}
        for name, src in (("q", wq_d), ("k", wk_d), ("v", wv_d)):
            wts[name] = []
            for c in range(NCC):
                t = wpool.tile([P, HP * DH], BF, tag=f"w{name}{c}")
                nc.sync.dma_start(out=t[:], in_=src[c * P:(c + 1) * P, :])
                wts[name].append(t)

        for b in range(B):
            xts = []
            for c in range(NCC):
                xt = xpool.tile([P, T], BF, tag=f"x{c}")
                nc.sync.dma_start(out=xt[:], in_=xT_d[b, c * P:(c + 1) * P, :])
                xts.append(xt)

            qt = qk_pool.tile([P, T], BF, tag="qt")
            kt = qk_pool.tile([P, T], BF, tag="kt")
            vaug = vaug_pool.tile([P, NT * HP, DA], BF, tag="vaug")
            # ones column per head-block (softmax denominator row of O^T)
            nc.vector.tensor_copy(vaug[:, :, DA - 1:DA], ones_col[:])

            with tc.tile_pool(name="proj_ps", bufs=3, space="PSUM") as proj_ps, \
                 tc.tile_pool(name="vt_ps", bufs=2, space="PSUM") as vt_ps:
                for wt, dst in ((wts["q"], qt), (wts["k"], kt)):
                    for n in range(NQ):
                        ps = proj_ps.tile([P, 512], FP, tag="proj")
                        for c in range(NCC):
                            nc.tensor.matmul(
                                ps[:], wt[c][:], xts[c][:, n * 512:(n + 1) * 512],
                                start=(c == 0), stop=(c == NCC - 1))
                        nc.vector.tensor_copy(dst[:, n * 512:(n + 1) * 512], ps[:])
                # V, then transpose into [s, d] layout with ones columns
                for n in range(NQ):
                    ps = proj_ps.tile([P, 512], FP, tag="proj")
                    for c in range(NCC):
                        nc.tensor.matmul(
                            ps[:], wts["v"][c][:], xts[c][:, n * 512:(n + 1) * 512],
                            start=(c == 0), stop=(c == NCC - 1))
                    vt = vt_pool.tile([P, 512], BF, tag="vt")
                    nc.vector.tensor_copy(vt[:], ps[:])
                    for u in range(4):
                        j = 4 * n + u
                        tp = vt_ps.tile([P, P], BF, tag="vtp")
                        nc.tensor.transpose(tp[:], vt[:, u * P:(u + 1) * P], ident[:])
                        nc.vector.tensor_copy(
                            vaug[:, j * HP, 0:DH], tp[:, 0:DH])
                        nc.vector.tensor_copy(
                            vaug[:, j * HP + 1, 0:DH], tp[:, DH:2 * DH])

            with tc.tile_pool(name="s_ps", bufs=2, space="PSUM") as s_ps, \
                 tc.tile_pool(name="o_ps", bufs=1, space="PSUM") as o_ps:
                ot_sbs = [ot_pool.tile([DA, T], FP, tag=f"ot{h}", name=f"ot{h}")
                          for h in range(HP)]
                for k in range(NK):
                    q_lo = 1024 * k
                    q_hi = 1024 * (k + 1)
                    o_tiles = [o_ps.tile([DA, 1024], FP, tag=f"o{h}", name=f"o{h}")
                               for h in range(HP)]
                    for j in range(8 * (k + 1)):
                        s0 = j * P
                        a0 = max(s0, q_lo)
                        # 512-grid chunks of the valid q range in this stripe
                        chunks = []
                        m0 = a0 // 512
                        for m in range(m0, q_hi // 512):
                            a = max(a0, m * 512)
                            e = (m + 1) * 512
                            chunks.append((a, e))
                        stl = [s_ps.tile([P, 1024], FP, tag="s", name="s")
                               for _ in range(HP)]
                        # emit head pairs adjacently: rows 0-63 (head A) and
                        # 64-127 (head B) run concurrently in the PE array
                        for (a, e) in chunks:
                            for h in range(HP):
                                hs = slice(h * DH, (h + 1) * DH)
                                nc.tensor.matmul(
                                    stl[h][:, a - q_lo:e - q_lo],
                                    kt[hs, s0:s0 + P], qt[hs, a:e],
                                    start=True, stop=True,
                                    tile_position=(h * DH, 0))
                        if q_lo <= s0:
                            for h in range(HP):
                                # diagonal block: additive causal mask
                                nc.vector.tensor_add(
                                    stl[h][:, s0 - q_lo:s0 - q_lo + P],
                                    stl[h][:, s0 - q_lo:s0 - q_lo + P],
                                    negmask[:])
                        for h in range(HP):
                            ptk = pt_pool.tile([P, 1024], BF, tag="pt")
                            nc.scalar.activation(
                                ptk[:, a0 - q_lo:1024], stl[h][:, a0 - q_lo:1024],
                                AF.Exp, scale=scale)
                            va = vaug[:, j * HP + h, :]
                            for (a, e) in chunks:
                                last_j = e // P - 1
                                nc.tensor.matmul(
                                    o_tiles[h][:, a - q_lo:e - q_lo],
                                    va, ptk[:, a - q_lo:e - q_lo],
                                    start=(j == 0), stop=(j == last_j))
                    for h in range(HP):
                        nc.vector.tensor_copy(
                            ot_sbs[h][:, q_lo:q_hi], o_tiles[h][:])
                for h in range(HP):
                    nc.sync.dma_start(out=ot_d[b, h], in_=ot_sbs[h][:])
    nc.compile()
    return nc


# --------------------------------------------------------------------------
# phase 2: per-core Wo projection + residual + rmsnorm + FFN + rmsnorm
# --------------------------------------------------------------------------

def build_phase2(NTOK, C, DFF):
    NTB = NTOK // P
    NCH = C // P
    NDF = DFF // P
    NG = DFF // 512
    halves = [(st, min(512, C - st)) for st in range(0, C, 512)]
    NH = len(halves)            # <=512-wide chunks of the channel dim

    nc = bacc.Bacc("TRN2", debug=False)
    xc_d = nc.dram_tensor("xc", [NTOK, C], FP, kind="ExternalInput").ap()
    at_d = nc.dram_tensor("attnT", [C, NTOK], BF, kind="ExternalInput").ap()
    wo_d = nc.dram_tensor("wo", [C, C], BF, kind="ExternalInput").ap()
    w1_d = nc.dram_tensor("w1", [C, DFF], BF, kind="ExternalInput").ap()
    w2_d = nc.dram_tensor("w2", [DFF, C], BF, kind="ExternalInput").ap()
    g1_d = nc.dram_tensor("g1", [C], FP, kind="ExternalInput").ap()
    g2_d = nc.dram_tensor("g2", [C], FP, kind="ExternalInput").ap()
    b1_d = nc.dram_tensor("b1", [DFF], FP, kind="ExternalInput").ap()
    b2_d = nc.dram_tensor("b2", [C], FP, kind="ExternalInput").ap()
    out_d = nc.dram_tensor("out", [NTOK, C], FP, kind="ExternalOutput").ap()

    def bcast_rows(src_ap, cols):
        # DRAM vector [cols] -> [P, cols] (same row in every partition)
        return bass.AP(tensor=src_ap.tensor, offset=src_ap.offset,
                       ap=[[0, P], [1, cols]])

    def col_ap(src_ap, start):
        # DRAM vector slice [start:start+P] -> [P, 1] (one value per partition)
        return bass.AP(tensor=src_ap.tensor, offset=src_ap.offset + start,
                       ap=[[1, P], [0, 1]])

    with tile.TileContext(nc) as tc, ExitStack() as ctx:
        const = ctx.enter_context(tc.tile_pool(name="const", bufs=1))
        work = ctx.enter_context(tc.tile_pool(name="work", bufs=2))
        stats = ctx.enter_context(tc.tile_pool(name="stats", bufs=4))
        h_pool = ctx.enter_context(tc.tile_pool(name="hp", bufs=1))
        ht_pool = ctx.enter_context(tc.tile_pool(name="htp", bufs=1))
        at_pool = ctx.enter_context(tc.tile_pool(name="atp", bufs=1))

        ident = const.tile([P, P], FP)
        make_identity(nc, ident[:])
        eps_t = const.tile([P, 1], FP)
        nc.vector.memset(eps_t[:], EPS)
        g1b = const.tile([P, C], FP)
        nc.sync.dma_start(out=g1b[:], in_=bcast_rows(g1_d, C))
        g2b = const.tile([P, C], FP)
        nc.sync.dma_start(out=g2b[:], in_=bcast_rows(g2_d, C))
        b2b = const.tile([P, C], FP)
        nc.sync.dma_start(out=b2b[:], in_=bcast_rows(b2_d, C))
        b1s = []
        for d in range(NDF):
            t = const.tile([P, 1], FP, tag=f"b1_{d}")
            nc.sync.dma_start(out=t[:], in_=col_ap(b1_d, d * P))
            b1s.append(t)

        def rmsnorm(src, gb, out_tag):
            sq = work.tile([P, C], FP, tag="sq")
            ssum = stats.tile([P, 1], FP, tag="ssum")
            nc.scalar.activation(sq[:], src[:], AF.Square, accum_out=ssum[:])
            rstd = stats.tile([P, 1], FP, tag="rstd")
            nc.scalar.activation(rstd[:], ssum[:], AF.Sqrt,
                                 scale=1.0 / C, bias=eps_t[:])
            rinv = stats.tile([P, 1], FP, tag="rinv")
            nc.vector.reciprocal(rinv[:], rstd[:])
            out = work.tile([P, C], FP, tag=out_tag)
            nc.vector.tensor_scalar_mul(out[:], src[:], rinv[:])
            nc.vector.tensor_mul(out[:], out[:], gb[:])
            return out

        # ---- stage 0: o = attnT^T @ Wo; r1 = x + o; h = rmsnorm(r1)*g1
        hs = []
        with tc.tile_pool(name="o_ps", bufs=1, space="PSUM") as o_ps, \
             tc.tile_pool(name="wop", bufs=NCH) as wop, \
             tc.tile_pool(name="atsp", bufs=NCH) as atsp, \
             tc.tile_pool(name="xcp", bufs=1) as xcp:
            atts, wots = [], []
            for c in range(NCH):
                att = atsp.tile([P, NTOK], BF, tag="at", name="at")
                nc.sync.dma_start(out=att[:], in_=at_d[c * P:(c + 1) * P, :])
                wot = wop.tile([P, C], BF, tag="wo", name="wo")
                nc.sync.dma_start(out=wot[:], in_=wo_d[c * P:(c + 1) * P, :])
                atts.append(att)
                wots.append(wot)
            xcs = []
            for tb in range(NTB):
                t = xcp.tile([P, C], FP, tag=f"xc{tb}")
                nc.sync.dma_start(out=t[:], in_=xc_d[tb * P:(tb + 1) * P, :])
                xcs.append(t)
            o_tiles = [o_ps.tile([P, 512], FP, tag=f"ops{i}", name=f"ops{i}")
                       for i in range(NTB * NH)]
            for c in range(NCH):
                att = atts[c]
                wot = wots[c]
                for tb in range(NTB):
                    for half, (hst, hw) in enumerate(halves):
                        nc.tensor.matmul(
                            o_tiles[tb * NH + half][:, :hw],
                            att[:, tb * P:(tb + 1) * P],
                            wot[:, hst:hst + hw],
                            start=(c == 0), stop=(c == NCH - 1))
            for tb in range(NTB):
                r1 = work.tile([P, C], FP, tag="r1")
                for half, (hst, hw) in enumerate(halves):
                    nc.vector.tensor_add(
                        r1[:, hst:hst + hw],
                        o_tiles[tb * NH + half][:, :hw],
                        xcs[tb][:, hst:hst + hw])
                hn = rmsnorm(r1, g1b, "hn")
                h = h_pool.tile([P, C], FP, tag=f"h{tb}")
                nc.vector.tensor_copy(h[:], hn[:])
                hs.append(h)

        # ---- stage 1: hT
        hts = [ht_pool.tile([P, NTOK], BF, tag=f"ht{c}", name=f"ht{c}")
               for c in range(NCH)]
        with tc.tile_pool(name="t_ps", bufs=4, space="PSUM") as t_ps:
            for tb in range(NTB):
                for c in range(NCH):
                    tp = t_ps.tile([P, P], FP, tag="tp")
                    nc.tensor.transpose(
                        tp[:], hs[tb][:, c * P:(c + 1) * P], ident[:])
                    nc.vector.tensor_copy(hts[c][:, tb * P:(tb + 1) * P], tp[:])

        # ---- stage 2: aT = silu(W1^T @ h^T + b1)
        ats = []
        w2p = ctx.enter_context(tc.tile_pool(name="w2p", bufs=5))
        with tc.tile_pool(name="a_ps", bufs=8, space="PSUM") as a_ps, \
             tc.tile_pool(name="w1p", bufs=5) as w1p, \
             tc.tile_pool(name="sgp", bufs=3) as sgp:
            for g in range(NG):
                aps = [a_ps.tile([P, NTOK], FP, tag="a", name="a") for _ in range(4)]
                for c in range(NCH):
                    w1t = w1p.tile([P, 512], BF, tag="w1")
                    nc.sync.dma_start(
                        out=w1t[:],
                        in_=w1_d[c * P:(c + 1) * P, g * 512:(g + 1) * 512])
                    for u in range(4):
                        nc.tensor.matmul(
                            aps[u][:], w1t[:, u * P:(u + 1) * P],
                            hts[c][:],
                            start=(c == 0), stop=(c == NCH - 1))
                for u in range(4):
                    d = 4 * g + u
                    sg = sgp.tile([P, NTOK], FP, tag="sg")
                    nc.scalar.activation(sg[:], aps[u][:], AF.Sigmoid,
                                         bias=b1s[d][:], scale=1.0)
                    at_t = at_pool.tile([P, NTOK], BF, tag=f"at{d}")
                    # silu(z) for z = a + b1: (a + b1) * sigmoid(a + b1)
                    nc.vector.scalar_tensor_tensor(
                        at_t[:], aps[u][:], b1s[d][:], sg[:],
                        op0=mybir.AluOpType.add, op1=mybir.AluOpType.mult)
                    ats.append(at_t)

        # ---- stage 3: f = aT^T @ W2; r2 = h + b2 + f; out = rmsnorm(r2)*g2
        with tc.tile_pool(name="f_ps", bufs=1, space="PSUM") as f_ps:
            fts = [f_ps.tile([P, 512], FP, tag=f"f{i}", name=f"f{i}")
                   for i in range(NTB * NH)]
            for d in range(NDF):
                w2t = w2p.tile([P, C], BF, tag="w2")
                nc.sync.dma_start(out=w2t[:], in_=w2_d[d * P:(d + 1) * P, :])
                for tb in range(NTB):
                    for half, (hst, hw) in enumerate(halves):
                        nc.tensor.matmul(
                            fts[tb * NH + half][:, :hw],
                            ats[d][:, tb * P:(tb + 1) * P],
                            w2t[:, hst:hst + hw],
                            start=(d == 0), stop=(d == NDF - 1))
            for tb in range(NTB):
                hb = work.tile([P, C], FP, tag="hb")
                nc.vector.tensor_add(hb[:], hs[tb][:], b2b[:])
                r2 = work.tile([P, C], FP, tag="r2")
                for half, (hst, hw) in enumerate(halves):
                    nc.vector.tensor_add(
                        r2[:, hst:hst + hw],
                        fts[tb * NH + half][:, :hw],
                        hb[:, hst:hst + hw])
                o = rmsnorm(r2, g2b, "outt")
                nc.sync.dma_start(out=out_d[tb * P:(tb + 1) * P, :], in_=o[:])
    nc.compile()
    return nc


# --------------------------------------------------------------------------
# host orchestration
# --------------------------------------------------------------------------

_CACHE = {}


def _phase1(B, T, C, DH):
    key = ("p1", B, T, C, DH)
    if key not in _CACHE:
        _CACHE[key] = build_phase1(B, T, C, DH)
    return _CACHE[key]


def _phase2(NTOK, C, DFF):
    key = ("p2", NTOK, C, DFF)
    if key not in _CACHE:
        _CACHE[key] = build_phase2(NTOK, C, DFF)
    return _CACHE[key]


def _run(nc, in_maps):
    import os
    trace = bool(os.environ.get("KERNEL_TRACE"))
    kwargs = {}
    if trace:
        _install_ntff_hook_shim()
        tdir = os.environ.get("KERNEL_TRACE_DIR")
        if tdir:
            phase_dir = os.path.join(tdir, f"phase{len(LAST_EXEC_NS)}")
            os.makedirs(phase_dir, exist_ok=True)
            kwargs["tmpdir"] = phase_dir
    res = run_bass_kernel_spmd(nc, in_maps, core_ids=list(range(N_CORES)),
                               trace=trace, **kwargs)
    LAST_EXEC_NS.append(res.exec_time_ns)
    return res.results


def kernel(x, Wq, Wk, Wv, Wo, bo, W1, b1, W2, b2, g1, g2):
    f32 = lambda a: np.ascontiguousarray(np.asarray(a), dtype=np.float32)
    x = f32(x)
    Wq, Wk, Wv, Wo, bo = f32(Wq), f32(Wk), f32(Wv), f32(Wo), f32(bo)
    W1, b1, W2, b2, g1, g2 = f32(W1), f32(b1), f32(W2), f32(b2), f32(g1), f32(g2)

    B, T, C = x.shape
    H, _, DH = Wq.shape
    HP = H // N_CORES           # heads per core (2)
    DA = DH + 1
    LAST_EXEC_NS.clear()

    # ---- phase 1
    nc1 = _phase1(B, T, C, DH)
    xT = np.ascontiguousarray(x.transpose(0, 2, 1)).astype(BF_NP)
    in1 = []
    for i in range(N_CORES):
        pq = Wq[HP * i:HP * (i + 1)].transpose(1, 0, 2).reshape(C, HP * DH)
        pk = Wk[HP * i:HP * (i + 1)].transpose(1, 0, 2).reshape(C, HP * DH)
        pv = Wv[HP * i:HP * (i + 1)].transpose(1, 0, 2).reshape(C, HP * DH)
        in1.append({"xT": xT,
                    "wq": np.ascontiguousarray(pq).astype(BF_NP),
                    "wk": np.ascontiguousarray(pk).astype(BF_NP),
                    "wv": np.ascontiguousarray(pv).astype(BF_NP)})
    res1 = _run(nc1, in1)

    attn = np.empty((B, T, C), np.float32)
    for i in range(N_CORES):
        ot = res1[i]["ot"]                    # [B, HP, DA, T]
        o = ot[:, :, :DH, :]
        den = ot[:, :, DH, :]
        on = o / den[:, :, None, :]
        for hh in range(HP):
            hcol = (HP * i + hh) * DH
            attn[:, :, hcol:hcol + DH] = on[:, hh].transpose(0, 2, 1)

    # ---- phase 2
    NTOK = B * T // N_CORES
    nc2 = _phase2(NTOK, C, W1.shape[1])
    xf = x.reshape(B * T, C) + bo             # fold bo into the residual
    af = attn.reshape(B * T, C)
    wo_bf = Wo.astype(BF_NP)
    w1_bf = W1.astype(BF_NP)
    w2_bf = W2.astype(BF_NP)
    in2 = []
    for k in range(N_CORES):
        sl = slice(k * NTOK, (k + 1) * NTOK)
        in2.append({
            "xc": np.ascontiguousarray(xf[sl]),
            "attnT": np.ascontiguousarray(af[sl].T).astype(BF_NP),
            "wo": wo_bf, "w1": w1_bf, "w2": w2_bf,
            "g1": g1, "g2": g2, "b1": b1, "b2": b2,
        })
    res2 = _run(nc2, in2)
    out = np.concatenate([res2[k]["out"] for k in range(N_CORES)], axis=0)
    return out.reshape(B, T, C)



# revision 17
# speedup vs baseline: 1.2838x; 1.2838x over previous
"""Trainium2 Bass kernel for a dense transformer block, distributed over 8
NeuronCores.

Sharding:
  phase 1 (attention): tensor-parallel over heads — each core computes 2 of
    the 16 heads end-to-end (QKV projections + causal softmax(QK^T)V), and
    returns the unnormalized per-head output O^T together with the softmax
    denominators (obtained via a ones-column appended to V).
  phase 2 (Wo + norms + FFN): data-parallel over tokens — each core handles
    512 of the 4096 token rows with replicated weights.

The host glues the phases: transposes x, normalizes/concats heads, and
re-shards tokens.  All matmuls run as float32r (full-rate fp32 PE mode).
"""

import math
from contextlib import ExitStack

import ml_dtypes
import numpy as np

BF_NP = ml_dtypes.bfloat16

import concourse.bass as bass
import concourse.mybir as mybir
import concourse.tile as tile
from concourse import bacc
from concourse.bass_utils import run_bass_kernel_spmd
from concourse.masks import make_identity, make_upper_triangular

FP = mybir.dt.float32
FPR = mybir.dt.float32r
BF = mybir.dt.bfloat16
AF = mybir.ActivationFunctionType

N_CORES = 8
P = 128
EPS = 1e-6

# exec times (ns) of the most recent kernel() call, one entry per phase, when
# tracing was enabled via BASS_TRACE=1; None entries otherwise.
LAST_EXEC_NS = []


def _install_ntff_hook_shim():
    """Provide antenv.axon_hooks when the image lacks it, so trace=True can
    drive NTFF profiling through libaxon_pjrt's C ABI (same contract as
    trn_boot's step-6 hook). No-op if the real module exists or the .so is
    missing/old."""
    try:
        import antenv.axon_hooks  # noqa: F401
        return
    except ImportError:
        pass
    import contextlib
    import ctypes
    import sys
    import types

    try:
        lib = ctypes.CDLL("/opt/axon/libaxon_pjrt.so")
    except OSError:
        return
    if not hasattr(lib, "axon_start_nrt_profile"):
        return
    lib.axon_start_nrt_profile.argtypes = [
        ctypes.POINTER(ctypes.c_int64), ctypes.c_size_t]
    lib.axon_start_nrt_profile.restype = ctypes.c_int64
    lib.axon_stop_nrt_profile.argtypes = [ctypes.c_char_p]
    lib.axon_stop_nrt_profile.restype = ctypes.c_int64

    @contextlib.contextmanager
    def _hook(output_dir, device_ids):
        import jax
        jax.devices()
        if device_ids:
            ids = (ctypes.c_int64 * len(device_ids))(*device_ids)
            rc = lib.axon_start_nrt_profile(ids, len(device_ids))
        else:
            rc = lib.axon_start_nrt_profile(None, 0)
        if rc != 0:
            raise RuntimeError(f"axon_start_nrt_profile rc={rc}")
        try:
            yield
        finally:
            n = lib.axon_stop_nrt_profile(str(output_dir).encode())
            if n < 0:
                raise RuntimeError(f"axon_stop_nrt_profile rc={n}")

    mod = types.ModuleType("antenv.axon_hooks")
    mod.get_axon_ntff_profile_hook = lambda: _hook

    def set_axon_ntff_profile_hook(h):
        mod.get_axon_ntff_profile_hook = lambda: h

    mod.set_axon_ntff_profile_hook = set_axon_ntff_profile_hook
    import antenv
    antenv.axon_hooks = mod
    sys.modules["antenv.axon_hooks"] = mod


def _fpr(ap):
    return ap.bitcast(FPR)


# --------------------------------------------------------------------------
# phase 1: per-core attention over a pair of heads (fp8 + DoubleRow)
# --------------------------------------------------------------------------

PF8 = mybir.dt.float8e4
F8_NP = ml_dtypes.float8_e4m3
WS = 32.0          # q/k/v weight upscale; folded out via exp scale + host div


class _Fillers:
    """FIFO of deferred PE-work emitters, drained one unit at a time into
    the exp-gated attention loop so the tensor engine never idles."""

    def __init__(self):
        self.q = []

    def add(self, fn):
        self.q.append(fn)

    def take(self, n):
        out, self.q = self.q[:n], self.q[n:]
        return out

    def drain(self):
        out, self.q = self.q, []
        return out


def build_phase1(B, T, C, DH):
    HP = 2                      # heads per core
    DA = DH + 1                 # head dim + ones row (softmax denominator)
    DAP = 80                    # DA padded so the DoubleRow pair stride is 16B-aligned
    NKK = C // 256              # DoubleRow contraction pairs
    NST = T // P                # key stripes of 128
    NM = T // 512               # query blocks of 512
    scale = float(C) ** -0.5 / (WS * WS)
    DR = mybir.MatmulPerfMode.DoubleRow

    nc = bacc.Bacc("TRN2", debug=False)
    xT_d = nc.dram_tensor("xT8", [B, NKK, P, 2, T], PF8,
                          kind="ExternalInput").ap()
    wq_d = nc.dram_tensor("wq8", [NKK, P, 2, HP * DH], PF8,
                          kind="ExternalInput").ap()
    wk_d = nc.dram_tensor("wk8", [NKK, P, 2, HP * DH], PF8,
                          kind="ExternalInput").ap()
    wv_d = nc.dram_tensor("wv8", [2 * NKK, P, HP * DH], PF8,
                          kind="ExternalInput").ap()
    ot_d = nc.dram_tensor("ot", [B, HP, DA, T], FP, kind="ExternalOutput").ap()

    with tile.TileContext(nc) as tc, ExitStack() as ctx:
        const = ctx.enter_context(tc.tile_pool(name="const", bufs=1))
        xp = ctx.enter_context(tc.tile_pool(name="xp", bufs=1))
        wp = ctx.enter_context(tc.tile_pool(name="wp", bufs=1))
        qkp = ctx.enter_context(tc.tile_pool(name="qkp", bufs=1))
        vap = ctx.enter_context(tc.tile_pool(name="vap", bufs=1))
        ptkp = ctx.enter_context(tc.tile_pool(name="ptkp", bufs=6))
        otp = ctx.enter_context(tc.tile_pool(name="otp", bufs=1))
        pp = ctx.enter_context(tc.tile_pool(name="pp", bufs=2, space="PSUM"))
        sp = ctx.enter_context(tc.tile_pool(name="sp", bufs=2, space="PSUM"))
        op = ctx.enter_context(tc.tile_pool(name="op", bufs=1, space="PSUM"))

        # additive causal mask for a diagonal 128x128 block of S^T [s', q']:
        # 0 where q' >= s', -1e30 where q' < s'
        negmask = const.tile([P, P], FP)
        nc.gpsimd.memset(negmask[:], 0.0)
        nc.gpsimd.affine_select(
            out=negmask[:], in_=negmask[:],
            compare_op=mybir.AluOpType.is_ge, fill=-1e30,
            base=0, pattern=[[1, P]], channel_multiplier=-1)
        # preload the Exp table set before the attention loop needs it
        warm = const.tile([P, 1], FP)
        nc.vector.memset(warm[:], 0.0)
        nc.scalar.activation(warm[:], warm[:], AF.Exp, scale=1.0)

        wqs, wks, wvs = [], [], []
        for kk in range(NKK):
            wq = wp.tile([P, 2, HP * DH], PF8, tag=f"wq{kk}")
            nc.sync.dma_start(out=wq[:], in_=wq_d[kk])
            wk = wp.tile([P, 2, HP * DH], PF8, tag=f"wk{kk}")
            nc.sync.dma_start(out=wk[:], in_=wk_d[kk])
            wqs.append(wq)
            wks.append(wk)
        for c in range(2 * NKK):
            wv = wp.tile([P, HP * DH], PF8, tag=f"wv{c}")
            nc.sync.dma_start(out=wv[:], in_=wv_d[c])
            wvs.append(wv)

        xts, qts, kts, vaugs, otsbs = {}, {}, {}, {}, {}

        def load_x(b):
            for kk in range(NKK):
                t = xp.tile([P, 2, T], PF8, tag=f"x{b}_{kk}")
                nc.sync.dma_start(out=t[:], in_=xT_d[b, kk])
                xts[(b, kk)] = t

        for b in range(B):
            qts[b] = qkp.tile([P, T], PF8, tag=f"qt{b}", name=f"qt{b}")
            kts[b] = qkp.tile([P, T], PF8, tag=f"kt{b}", name=f"kt{b}")
            va = vap.tile([P, NST // 2, 2, HP, DAP], PF8, tag=f"va{b}",
                          name=f"va{b}")
            nc.vector.memset(va[:], 1.0)    # ones column survives in col DH
            vaugs[b] = va
            for h in range(HP):
                otsbs[(b, h)] = otp.tile([DA, T], FP, tag=f"ot{b}{h}", name=f"ot{b}{h}")

        def emit_qk_chunk(b, proj, n):
            # one 512-token chunk of the Q or K projection (NKK DR matmuls)
            wt = wqs if proj == 0 else wks
            dst = qts[b] if proj == 0 else kts[b]
            ps = pp.tile([P, 512], FP, tag="pp")
            for kk in range(NKK):
                nc.tensor.matmul(
                    ps[:], wt[kk][:],
                    xts[(b, kk)][:, :, n * 512:(n + 1) * 512],
                    start=(kk == 0), stop=(kk == NKK - 1), perf_mode=DR)
            nc.vector.tensor_copy(dst[:, n * 512:(n + 1) * 512], ps[:])

        def emit_v_stripe(b, s):
            # V^T stripe s directly in [token, head-dim] layout
            ps = pp.tile([P, 512], FP, tag="pp")
            out = ps[:, 0:HP * DH]
            for c in range(2 * NKK):
                nc.tensor.matmul(
                    out, xts[(b, c // 2)][:, c % 2, s * P:(s + 1) * P],
                    wvs[c][:],
                    start=(c == 0), stop=(c == 2 * NKK - 1))
            for h in range(HP):
                nc.vector.tensor_copy(
                    vaugs[b][:, s // 2, s % 2, h, 0:DH],
                    ps[:, h * DH:(h + 1) * DH])

        def emit_spv_block(b, m, fillers):
            o_tiles = [op.tile([DA, 512], FP, tag=f"o{h}", name=f"o{h}")
                       for h in range(HP)]
            npairs = 2 * (m + 1)
            for jj in range(npairs):
                ptk = ptkp.tile([P, 2, HP, 512], PF8, tag="ptk")
                for ss in range(2):
                    j = 2 * jj + ss
                    s0 = j * P
                    diag = s0 >= m * 512
                    off = s0 - m * 512 if diag else 0
                    st = sp.tile([P, HP, 512], FP, tag="st")
                    for h in range(HP):
                        hs = slice(h * DH, (h + 1) * DH)
                        nc.tensor.matmul(
                            st[:, h, off:512],
                            kts[b][hs, s0:s0 + P],
                            qts[b][hs, m * 512 + off:(m + 1) * 512],
                            start=True, stop=True,
                            tile_position=(h * DH, 0))
                    if diag:
                        for h in range(HP):
                            nc.vector.tensor_add(
                                st[:, h, off:off + P],
                                st[:, h, off:off + P], negmask[:])
                        if off > 0:
                            nc.vector.memset(ptk[:, ss, :, 0:off], 0.0)
                    nc.scalar.activation(
                        ptk[:, ss, :, off:512], st[:, :, off:512],
                        AF.Exp, scale=scale)
                    for f in fillers.take(1):
                        f()
                for h in range(HP):
                    nc.tensor.matmul(
                        o_tiles[h][:], vaugs[b][:, jj, :, h, 0:DA],
                        ptk[:, :, h, :],
                        start=(jj == 0), stop=(jj == npairs - 1),
                        perf_mode=DR)
            for h in range(HP):
                nc.vector.tensor_copy(
                    otsbs[(b, h)][:, m * 512:(m + 1) * 512], o_tiles[h][:])

        # ---- batch 0 prelude: Q/K projections + first V stripes
        load_x(0)
        for n in range(NM):
            emit_qk_chunk(0, 0, n)
            emit_qk_chunk(0, 1, n)
        for s in range(4):
            emit_v_stripe(0, s)
        load_x(1)

        # fillers drained into b0's attention loop: rest of V(b0), all of
        # Q/K(b1), first V stripes of b1
        fill0 = _Fillers()
        for s in range(4, NST):
            fill0.add(lambda s=s: emit_v_stripe(0, s))
        for n in range(NM):
            fill0.add(lambda n=n: emit_qk_chunk(1, 0, n))
            fill0.add(lambda n=n: emit_qk_chunk(1, 1, n))
        for s in range(4):
            fill0.add(lambda s=s: emit_v_stripe(1, s))
        for m in range(NM):
            emit_spv_block(0, m, fill0)
        for f in fill0.drain():
            f()
        for h in range(HP):
            nc.scalar.dma_start(out=ot_d[0, h], in_=otsbs[(0, h)][:])

        fill1 = _Fillers()
        for s in range(4, NST):
            fill1.add(lambda s=s: emit_v_stripe(1, s))
        for m in range(NM):
            emit_spv_block(1, m, fill1)
        for f in fill1.drain():
            f()
        for h in range(HP):
            nc.scalar.dma_start(out=ot_d[1, h], in_=otsbs[(1, h)][:])
    nc.compile()
    return nc


def build_phase1_v1(B, T, C, DH):
    HP = 2                      # heads per core
    DA = DH + 1                 # head dim + ones row (softmax denominator)
    NCC = C // P                # contraction chunks
    NT = T // P                 # key/value blocks of 128
    NQ = T // 512               # query chunks of 512
    NK = T // 1024              # query tiles of 1024
    scale = float(C) ** -0.5    # NOTE: reference scales by C**-0.5, not DH

    nc = bacc.Bacc("TRN2", debug=False)
    xT_d = nc.dram_tensor("xT", [B, C, T], BF, kind="ExternalInput").ap()
    wq_d = nc.dram_tensor("wq", [C, HP * DH], BF, kind="ExternalInput").ap()
    wk_d = nc.dram_tensor("wk", [C, HP * DH], BF, kind="ExternalInput").ap()
    wv_d = nc.dram_tensor("wv", [C, HP * DH], BF, kind="ExternalInput").ap()
    ot_d = nc.dram_tensor("ot", [B, HP, DA, T], FP, kind="ExternalOutput").ap()

    with tile.TileContext(nc) as tc, ExitStack() as ctx:
        const = ctx.enter_context(tc.tile_pool(name="const", bufs=1))
        xpool = ctx.enter_context(tc.tile_pool(name="xp", bufs=1))
        wpool = ctx.enter_context(tc.tile_pool(name="wp", bufs=1))
        qk_pool = ctx.enter_context(tc.tile_pool(name="qk", bufs=2))
        vt_pool = ctx.enter_context(tc.tile_pool(name="vtp", bufs=2))
        vaug_pool = ctx.enter_context(tc.tile_pool(name="vaug", bufs=2))
        pt_pool = ctx.enter_context(tc.tile_pool(name="pt", bufs=4))
        ot_pool = ctx.enter_context(tc.tile_pool(name="otp", bufs=2))

        # additive mask for the diagonal 128x128 block of S^T [s', q']:
        # 0 where q' >= s' (causal-valid), -1e30 where q' < s'
        negmask = const.tile([P, P], FP)
        nc.gpsimd.memset(negmask[:], 0.0)
        nc.gpsimd.affine_select(
            out=negmask[:], in_=negmask[:],
            compare_op=mybir.AluOpType.is_ge, fill=-1e30,
            base=0, pattern=[[1, P]], channel_multiplier=-1)
        ident = const.tile([P, P], BF)
        make_identity(nc, ident[:])
        ones_col = const.tile([P, NT * HP, 1], FP)
        nc.vector.memset(ones_col[:], 1.0)

        # weight chunks, loaded once
        wts = {}
        for name, src in (("q", wq_d), ("k", wk_d), ("v", wv_d)):
            wts[name] = []
            for c in range(NCC):
                t = wpool.tile([P, HP * DH], BF, tag=f"w{name}{c}")
                nc.sync.dma_start(out=t[:], in_=src[c * P:(c + 1) * P, :])
                wts[name].append(t)

        for b in range(B):
            xts = []
            for c in range(NCC):
                xt = xpool.tile([P, T], BF, tag=f"x{c}")
                nc.sync.dma_start(out=xt[:], in_=xT_d[b, c * P:(c + 1) * P, :])
                xts.append(xt)

            qt = qk_pool.tile([P, T], BF, tag="qt")
            kt = qk_pool.tile([P, T], BF, tag="kt")
            vaug = vaug_pool.tile([P, NT * HP, DA], BF, tag="vaug")
            # ones column per head-block (softmax denominator row of O^T)
            nc.vector.tensor_copy(vaug[:, :, DA - 1:DA], ones_col[:])

            with tc.tile_pool(name="proj_ps", bufs=3, space="PSUM") as proj_ps, \
                 tc.tile_pool(name="vt_ps", bufs=2, space="PSUM") as vt_ps:
                for wt, dst in ((wts["q"], qt), (wts["k"], kt)):
                    for n in range(NQ):
                        ps = proj_ps.tile([P, 512], FP, tag="proj")
                        for c in range(NCC):
                            nc.tensor.matmul(
                                ps[:], wt[c][:], xts[c][:, n * 512:(n + 1) * 512],
                                start=(c == 0), stop=(c == NCC - 1))
                        nc.vector.tensor_copy(dst[:, n * 512:(n + 1) * 512], ps[:])
                # V, then transpose into [s, d] layout with ones columns
                for n in range(NQ):
                    ps = proj_ps.tile([P, 512], FP, tag="proj")
                    for c in range(NCC):
                        nc.tensor.matmul(
                            ps[:], wts["v"][c][:], xts[c][:, n * 512:(n + 1) * 512],
                            start=(c == 0), stop=(c == NCC - 1))
                    vt = vt_pool.tile([P, 512], BF, tag="vt")
                    nc.vector.tensor_copy(vt[:], ps[:])
                    for u in range(4):
                        j = 4 * n + u
                        tp = vt_ps.tile([P, P], BF, tag="vtp")
                        nc.tensor.transpose(tp[:], vt[:, u * P:(u + 1) * P], ident[:])
                        nc.vector.tensor_copy(
                            vaug[:, j * HP, 0:DH], tp[:, 0:DH])
                        nc.vector.tensor_copy(
                            vaug[:, j * HP + 1, 0:DH], tp[:, DH:2 * DH])

            with tc.tile_pool(name="s_ps", bufs=2, space="PSUM") as s_ps, \
                 tc.tile_pool(name="o_ps", bufs=1, space="PSUM") as o_ps:
                ot_sbs = [ot_pool.tile([DA, T], FP, tag=f"ot{h}", name=f"ot{h}")
                          for h in range(HP)]
                for k in range(NK):
                    q_lo = 1024 * k
                    q_hi = 1024 * (k + 1)
                    o_tiles = [o_ps.tile([DA, 1024], FP, tag=f"o{h}", name=f"o{h}")
                               for h in range(HP)]
                    for j in range(8 * (k + 1)):
                        s0 = j * P
                        a0 = max(s0, q_lo)
                        # 512-grid chunks of the valid q range in this stripe
                        chunks = []
                        m0 = a0 // 512
                        for m in range(m0, q_hi // 512):
                            a = max(a0, m * 512)
                            e = (m + 1) * 512
                            chunks.append((a, e))
                        stl = [s_ps.tile([P, 1024], FP, tag="s", name="s")
                               for _ in range(HP)]
                        # emit head pairs adjacently: rows 0-63 (head A) and
                        # 64-127 (head B) run concurrently in the PE array
                        for (a, e) in chunks:
                            for h in range(HP):
                                hs = slice(h * DH, (h + 1) * DH)
                                nc.tensor.matmul(
                                    stl[h][:, a - q_lo:e - q_lo],
                                    kt[hs, s0:s0 + P], qt[hs, a:e],
                                    start=True, stop=True,
                                    tile_position=(h * DH, 0))
                        if q_lo <= s0:
                            for h in range(HP):
                                # diagonal block: additive causal mask
                                nc.vector.tensor_add(
                                    stl[h][:, s0 - q_lo:s0 - q_lo + P],
                                    stl[h][:, s0 - q_lo:s0 - q_lo + P],
                                    negmask[:])
                        for h in range(HP):
                            ptk = pt_pool.tile([P, 1024], BF, tag="pt")
                            nc.scalar.activation(
                                ptk[:, a0 - q_lo:1024], stl[h][:, a0 - q_lo:1024],
                                AF.Exp, scale=scale)
                            va = vaug[:, j * HP + h, :]
                            for (a, e) in chunks:
                                last_j = e // P - 1
                                nc.tensor.matmul(
                                    o_tiles[h][:, a - q_lo:e - q_lo],
                                    va, ptk[:, a - q_lo:e - q_lo],
                                    start=(j == 0), stop=(j == last_j))
                    for h in range(HP):
                        nc.vector.tensor_copy(
                            ot_sbs[h][:, q_lo:q_hi], o_tiles[h][:])
                for h in range(HP):
                    nc.sync.dma_start(out=ot_d[b, h], in_=ot_sbs[h][:])
    nc.compile()
    return nc


# --------------------------------------------------------------------------
# phase 2: per-core Wo projection + residual + rmsnorm + FFN + rmsnorm
# --------------------------------------------------------------------------

def build_phase2(NTOK, C, DFF):
    NTB = NTOK // P             # 4 token tiles of 128
    NCH = C // P                # 8 channel chunks
    NDF = DFF // P              # 32 dff chunks
    NG = DFF // 512             # 8 dff groups of 512

    nc = bacc.Bacc("TRN2", debug=False)
    xc_d = nc.dram_tensor("xc", [NTOK, C], FP, kind="ExternalInput").ap()
    at_d = nc.dram_tensor("attnT", [C, NTOK], BF, kind="ExternalInput").ap()
    wo_d = nc.dram_tensor("wo", [C, C], BF, kind="ExternalInput").ap()
    # host-permuted W1: [g, c, 128, 512] so each (g, c) chunk is contiguous
    w1_d = nc.dram_tensor("w1p", [NG, NCH, P, 512], BF, kind="ExternalInput").ap()
    w2_d = nc.dram_tensor("w2", [DFF, C], BF, kind="ExternalInput").ap()
    g1_d = nc.dram_tensor("g1", [C], FP, kind="ExternalInput").ap()
    g2_d = nc.dram_tensor("g2", [C], FP, kind="ExternalInput").ap()
    b1_d = nc.dram_tensor("b1", [DFF], FP, kind="ExternalInput").ap()
    b2_d = nc.dram_tensor("b2", [C], FP, kind="ExternalInput").ap()
    out_d = nc.dram_tensor("out", [NTOK, C], FP, kind="ExternalOutput").ap()

    def bcast_rows(src_ap, cols):
        # DRAM vector [cols] -> [P, cols] (same row in every partition)
        return bass.AP(tensor=src_ap.tensor, offset=src_ap.offset,
                       ap=[[0, P], [1, cols]])

    with tile.TileContext(nc) as tc, ExitStack() as ctx:
        const = ctx.enter_context(tc.tile_pool(name="const", bufs=1))
        work = ctx.enter_context(tc.tile_pool(name="work", bufs=2))
        stats = ctx.enter_context(tc.tile_pool(name="stats", bufs=4))
        h_pool = ctx.enter_context(tc.tile_pool(name="hp", bufs=1))
        hb2p = ctx.enter_context(tc.tile_pool(name="hb2p", bufs=1))
        ht_pool = ctx.enter_context(tc.tile_pool(name="htp", bufs=1))
        at_pool = ctx.enter_context(tc.tile_pool(name="atp", bufs=1))
        w2sb = ctx.enter_context(tc.tile_pool(name="w2sb", bufs=10))

        ident = const.tile([P, P], BF)
        make_identity(nc, ident[:])
        eps_t = const.tile([P, 1], FP)
        nc.vector.memset(eps_t[:], EPS)
        # preload the Sqrt/Square table set before stage-0 norms need it
        warm = stats.tile([P, 1], FP, tag="warm")
        nc.scalar.activation(warm[:], eps_t[:], AF.Sqrt, scale=1.0)
        g1b = const.tile([P, C], FP)
        nc.sync.dma_start(out=g1b[:], in_=bcast_rows(g1_d, C))
        g2b = const.tile([P, C], FP)
        nc.sync.dma_start(out=g2b[:], in_=bcast_rows(g2_d, C))
        b2b = const.tile([P, C], FP)
        nc.sync.dma_start(out=b2b[:], in_=bcast_rows(b2_d, C))
        b1b = const.tile([P, DFF], FP)
        nc.sync.dma_start(out=b1b[:], in_=bcast_rows(b1_d, DFF))

        def rmsnorm_to(dst, src, gb):
            # dst = src * rsqrt(mean(src^2) + eps) * gb
            sq = work.tile([P, C], FP, tag="sq")
            ssum = stats.tile([P, 1], FP, tag="ssum")
            nc.scalar.activation(sq[:], src[:], AF.Square, accum_out=ssum[:])
            rstd = stats.tile([P, 1], FP, tag="rstd")
            nc.scalar.activation(rstd[:], ssum[:], AF.Sqrt,
                                 scale=1.0 / C, bias=eps_t[:])
            rinv = stats.tile([P, 1], FP, tag="rinv")
            nc.vector.reciprocal(rinv[:], rstd[:])
            nc.vector.scalar_tensor_tensor(
                dst[:], src[:], rinv[:], gb[:],
                op0=mybir.AluOpType.mult, op1=mybir.AluOpType.mult)

        # ---- stage 0: o = attnT^T @ Wo; r1 = x + o; h = rmsnorm(r1)*g1; hT
        hs = []
        hts = [ht_pool.tile([P, NTOK], BF, tag=f"ht{c}", name=f"ht{c}")
               for c in range(NCH)]
        with tc.tile_pool(name="o_ps", bufs=2, space="PSUM") as o_ps, \
             tc.tile_pool(name="t_ps", bufs=2, space="PSUM") as t_ps, \
             tc.tile_pool(name="wop", bufs=NCH) as wop, \
             tc.tile_pool(name="atsp", bufs=NCH) as atsp, \
             tc.tile_pool(name="xcp", bufs=1) as xcp, \
             tc.tile_pool(name="hbf", bufs=2) as hbf:
            atts, wots = [], []
            for c in range(NCH):
                att = atsp.tile([P, NTOK], BF, tag="at", name="at")
                nc.sync.dma_start(out=att[:], in_=at_d[c * P:(c + 1) * P, :])
                wot = wop.tile([P, C], BF, tag="wo", name="wo")
                nc.sync.dma_start(out=wot[:], in_=wo_d[c * P:(c + 1) * P, :])
                atts.append(att)
                wots.append(wot)
            xcs = []
            for tb in range(NTB):
                t = xcp.tile([P, C], FP, tag=f"xc{tb}")
                nc.sync.dma_start(out=t[:], in_=xc_d[tb * P:(tb + 1) * P, :])
                xcs.append(t)

            def wo_mms(tb, o):
                for c in range(NCH):
                    for half in range(2):
                        nc.tensor.matmul(
                            o[:, half * 512:(half + 1) * 512],
                            atts[c][:, tb * P:(tb + 1) * P],
                            wots[c][:, half * 512:(half + 1) * 512],
                            start=(c == 0), stop=(c == NCH - 1))

            def norm_h(tb, o):
                r1 = work.tile([P, C], FP, tag="r1")
                nc.vector.tensor_add(r1[:], o[:], xcs[tb][:])
                h = h_pool.tile([P, C], FP, tag=f"h{tb}")
                rmsnorm_to(h, r1, g1b)
                hs.append(h)
                hb = hbf.tile([P, C], BF, tag="hbf")
                nc.vector.tensor_copy(hb[:], h[:])
                return hb

            def trans_h(tb, hb):
                for c in range(NCH):
                    tp = t_ps.tile([P, P], BF, tag="tp")
                    nc.tensor.transpose(
                        tp[:], hb[:, c * P:(c + 1) * P], ident[:])
                    nc.vector.tensor_copy(hts[c][:, tb * P:(tb + 1) * P], tp[:])

            # pipeline: Wo(tb) | Wo(tb+1) + trans(tb) | ... so the PE never
            # waits on the norm chain
            o_tiles, hbs = [], []
            for tb in range(NTB):
                o = o_ps.tile([P, C], FP, tag="o")
                wo_mms(tb, o)
                o_tiles.append(o)
                hbs.append(norm_h(tb, o))
                if tb >= 1:
                    trans_h(tb - 1, hbs[tb - 1])
            trans_h(NTB - 1, hbs[NTB - 1])

        # ---- stage 2: aT = silu(W1^T @ h^T + b1)  [dff, tok] bf16
        ats = []
        with tc.tile_pool(name="a_ps", bufs=8, space="PSUM") as a_ps, \
             tc.tile_pool(name="w1p", bufs=24) as w1p, \
             tc.tile_pool(name="sgp", bufs=3) as sgp:
            for g in range(NG):
                w1ts = []
                for c in range(NCH):
                    w1t = w1p.tile([P, 512], BF, tag="w1")
                    nc.gpsimd.dma_start(out=w1t[:], in_=w1_d[g, c])
                    w1ts.append(w1t)
                aps = [a_ps.tile([P, NTOK], FP, tag="a", name="a")
                       for _ in range(4)]
                for c in range(NCH):
                    for u in range(4):
                        nc.tensor.matmul(
                            aps[u][:], w1ts[c][:, u * P:(u + 1) * P],
                            hts[c][:],
                            start=(c == 0), stop=(c == NCH - 1))
                for u in range(4):
                    d = 4 * g + u
                    sg = sgp.tile([P, NTOK], FP, tag="sg")
                    nc.scalar.activation(sg[:], aps[u][:], AF.Sigmoid,
                                         bias=b1b[:, d:d + 1], scale=1.0)
                    at_t = at_pool.tile([P, NTOK], BF, tag=f"at{d}")
                    # silu(z) for z = a + b1: (a + b1) * sigmoid(a + b1)
                    nc.vector.scalar_tensor_tensor(
                        at_t[:], aps[u][:], b1b[:, d:d + 1], sg[:],
                        op0=mybir.AluOpType.add, op1=mybir.AluOpType.mult)
                    ats.append(at_t)

        # hb[tb] = h + b2, precomputed on DVE while the PE runs W2
        hb2s = []
        for tb in range(NTB):
            hb2 = hb2p.tile([P, C], FP, tag=f"hb2_{tb}")
            nc.vector.tensor_add(hb2[:], hs[tb][:], b2b[:])
            hb2s.append(hb2)

        # preload the Sqrt table set during W2 so the tail norm doesn't
        # pay the ~2.7us table switch
        dummy = stats.tile([P, 1], FP, tag="dummy")
        nc.scalar.activation(dummy[:], eps_t[:], AF.Sqrt, scale=1.0)

        # ---- stage 3: f = aT^T @ W2; r2 = h + b2 + f; out = rmsnorm(r2)*g2
        with tc.tile_pool(name="f_ps", bufs=1, space="PSUM") as f_ps:
            fts = [f_ps.tile([P, C], FP, tag=f"f{tb}", name=f"f{tb}")
                   for tb in range(NTB)]
            for d in range(NDF):
                w2t = w2sb.tile([P, C], BF, tag="w2")
                nc.gpsimd.dma_start(out=w2t[:], in_=w2_d[d * P:(d + 1) * P, :])
                for tb in range(NTB):
                    for half in range(2):
                        nc.tensor.matmul(
                            fts[tb][:, half * 512:(half + 1) * 512],
                            ats[d][:, tb * P:(tb + 1) * P],
                            w2t[:, half * 512:(half + 1) * 512],
                            start=(d == 0), stop=(d == NDF - 1))
            for tb in range(NTB):
                r2 = work.tile([P, C], FP, tag="r2")
                nc.vector.tensor_add(r2[:], fts[tb][:], hb2s[tb][:])
                o = work.tile([P, C], FP, tag="outt")
                rmsnorm_to(o, r2, g2b)
                nc.scalar.dma_start(out=out_d[tb * P:(tb + 1) * P, :], in_=o[:])
    nc.compile()
    return nc


# --------------------------------------------------------------------------
# host orchestration
# --------------------------------------------------------------------------

_CACHE = {}


def _phase1(B, T, C, DH):
    key = ("p1", B, T, C, DH)
    if key not in _CACHE:
        _CACHE[key] = build_phase1(B, T, C, DH)
    return _CACHE[key]


def _phase2(NTOK, C, DFF):
    key = ("p2", NTOK, C, DFF)
    if key not in _CACHE:
        _CACHE[key] = build_phase2(NTOK, C, DFF)
    return _CACHE[key]


def _run(nc, in_maps):
    import os
    trace = bool(os.environ.get("KERNEL_TRACE"))
    kwargs = {}
    if trace:
        _install_ntff_hook_shim()
        tdir = os.environ.get("KERNEL_TRACE_DIR")
        if tdir:
            phase_dir = os.path.join(tdir, f"phase{len(LAST_EXEC_NS)}")
            os.makedirs(phase_dir, exist_ok=True)
            kwargs["tmpdir"] = phase_dir
    res = run_bass_kernel_spmd(nc, in_maps, core_ids=list(range(N_CORES)),
                               trace=trace, **kwargs)
    LAST_EXEC_NS.append(res.exec_time_ns)
    return res.results


def kernel(x, Wq, Wk, Wv, Wo, bo, W1, b1, W2, b2, g1, g2):
    f32 = lambda a: np.ascontiguousarray(np.asarray(a), dtype=np.float32)
    x = f32(x)
    Wq, Wk, Wv, Wo, bo = f32(Wq), f32(Wk), f32(Wv), f32(Wo), f32(bo)
    W1, b1, W2, b2, g1, g2 = f32(W1), f32(b1), f32(W2), f32(b2), f32(g1), f32(g2)

    B, T, C = x.shape
    H, _, DH = Wq.shape
    HP = H // N_CORES           # heads per core (2)
    DA = DH + 1
    LAST_EXEC_NS.clear()

    # ---- phase 1
    nc1 = _phase1(B, T, C, DH)
    NKK = C // 256
    # [B,T,C] -> [B, kk, 128, 2, T] with channel c = (2*kk+s)*128 + p
    xT8 = np.ascontiguousarray(
        x.transpose(0, 2, 1).reshape(B, NKK, 2, 128, T)
        .transpose(0, 1, 3, 2, 4)).astype(F8_NP)
    in1 = []
    for i in range(N_CORES):
        pq = Wq[HP * i:HP * (i + 1)].transpose(1, 0, 2).reshape(C, HP * DH)
        pk = Wk[HP * i:HP * (i + 1)].transpose(1, 0, 2).reshape(C, HP * DH)
        pv = Wv[HP * i:HP * (i + 1)].transpose(1, 0, 2).reshape(C, HP * DH)
        pair = lambda w: np.ascontiguousarray(
            (w * WS).reshape(NKK, 2, 128, HP * DH)
            .transpose(0, 2, 1, 3)).astype(F8_NP)
        in1.append({"xT8": xT8,
                    "wq8": pair(pq),
                    "wk8": pair(pk),
                    "wv8": np.ascontiguousarray(
                        (pv * WS).reshape(2 * NKK, 128, HP * DH)
                    ).astype(F8_NP)})
    res1 = _run(nc1, in1)

    attn = np.empty((B, T, C), np.float32)
    for i in range(N_CORES):
        ot = res1[i]["ot"]                    # [B, HP, DA, T]
        o = ot[:, :, :DH, :]
        den = ot[:, :, DH, :] * WS            # V carries a x32 scale
        on = o / den[:, :, None, :]
        for hh in range(HP):
            hcol = (HP * i + hh) * DH
            attn[:, :, hcol:hcol + DH] = on[:, hh].transpose(0, 2, 1)

    # ---- phase 2
    NTOK = B * T // N_CORES
    nc2 = _phase2(NTOK, C, W1.shape[1])
    xf = x.reshape(B * T, C) + bo             # fold bo into the residual
    af = attn.reshape(B * T, C)
    DFF = W1.shape[1]
    wo_bf = Wo.astype(BF_NP)
    # [C, DFF] -> [g, c, 128, 512] so each (g, c) chunk is DMA-contiguous
    w1_bf = np.ascontiguousarray(
        W1.reshape(C // 128, 128, DFF // 512, 512)
        .transpose(2, 0, 1, 3)).astype(BF_NP)
    w2_bf = W2.astype(BF_NP)
    in2 = []
    for k in range(N_CORES):
        sl = slice(k * NTOK, (k + 1) * NTOK)
        in2.append({
            "xc": np.ascontiguousarray(xf[sl]),
            "attnT": np.ascontiguousarray(af[sl].T).astype(BF_NP),
            "wo": wo_bf, "w1p": w1_bf, "w2": w2_bf,
            "g1": g1, "g2": g2, "b1": b1, "b2": b2,
        })
    res2 = _run(nc2, in2)
    out = np.concatenate([res2[k]["out"] for k in range(N_CORES)], axis=0)
    return out.reshape(B, T, C)



# revision 27
# speedup vs baseline: 1.3265x; 1.0333x over previous
"""Trainium2 Bass kernel for a dense transformer block, distributed over 8
NeuronCores.

Sharding:
  phase 1 (attention): tensor-parallel over heads — each core computes 2 of
    the 16 heads end-to-end (QKV projections + causal softmax(QK^T)V), and
    returns the unnormalized per-head output O^T together with the softmax
    denominators (obtained via a ones-column appended to V).
  phase 2 (Wo + norms + FFN): data-parallel over tokens — each core handles
    512 of the 4096 token rows with replicated weights.

The host glues the phases: transposes x, normalizes/concats heads, and
re-shards tokens.  All matmuls run as float32r (full-rate fp32 PE mode).
"""

import math
from contextlib import ExitStack

import ml_dtypes
import numpy as np

BF_NP = ml_dtypes.bfloat16

import concourse.bass as bass
import concourse.mybir as mybir
import concourse.tile as tile
from concourse import bacc
from concourse.bass_utils import run_bass_kernel_spmd
from concourse.masks import make_identity, make_upper_triangular

FP = mybir.dt.float32
FPR = mybir.dt.float32r
BF = mybir.dt.bfloat16
AF = mybir.ActivationFunctionType

N_CORES = 8
P = 128
EPS = 1e-6

# exec times (ns) of the most recent kernel() call, one entry per phase, when
# tracing was enabled via BASS_TRACE=1; None entries otherwise.
LAST_EXEC_NS = []


def _install_ntff_hook_shim():
    """Provide antenv.axon_hooks when the image lacks it, so trace=True can
    drive NTFF profiling through libaxon_pjrt's C ABI (same contract as
    trn_boot's step-6 hook). No-op if the real module exists or the .so is
    missing/old."""
    try:
        import antenv.axon_hooks  # noqa: F401
        return
    except ImportError:
        pass
    import contextlib
    import ctypes
    import sys
    import types

    try:
        lib = ctypes.CDLL("/opt/axon/libaxon_pjrt.so")
    except OSError:
        return
    if not hasattr(lib, "axon_start_nrt_profile"):
        return
    lib.axon_start_nrt_profile.argtypes = [
        ctypes.POINTER(ctypes.c_int64), ctypes.c_size_t]
    lib.axon_start_nrt_profile.restype = ctypes.c_int64
    lib.axon_stop_nrt_profile.argtypes = [ctypes.c_char_p]
    lib.axon_stop_nrt_profile.restype = ctypes.c_int64

    @contextlib.contextmanager
    def _hook(output_dir, device_ids):
        import jax
        jax.devices()
        if device_ids:
            ids = (ctypes.c_int64 * len(device_ids))(*device_ids)
            rc = lib.axon_start_nrt_profile(ids, len(device_ids))
        else:
            rc = lib.axon_start_nrt_profile(None, 0)
        if rc != 0:
            raise RuntimeError(f"axon_start_nrt_profile rc={rc}")
        try:
            yield
        finally:
            n = lib.axon_stop_nrt_profile(str(output_dir).encode())
            if n < 0:
                raise RuntimeError(f"axon_stop_nrt_profile rc={n}")

    mod = types.ModuleType("antenv.axon_hooks")
    mod.get_axon_ntff_profile_hook = lambda: _hook

    def set_axon_ntff_profile_hook(h):
        mod.get_axon_ntff_profile_hook = lambda: h

    mod.set_axon_ntff_profile_hook = set_axon_ntff_profile_hook
    import antenv
    antenv.axon_hooks = mod
    sys.modules["antenv.axon_hooks"] = mod


def _fpr(ap):
    return ap.bitcast(FPR)


def _enable_ldw_opt():
    """Flip walrus's --enable-ldw-opt to true so LDWEIGHTS overlaps with
    in-flight matmuls (background weight buffer). Verified by the rel-err
    check; idempotent."""
    import concourse.bass_utils as _bu
    if getattr(_bu, "_ldw_opt_patched", False):
        return
    orig = _bu.run_command

    def patched(cmd, *a, **kw):
        if isinstance(cmd, list) and getattr(_bu, "_ldw_opt_enable", False):
            cmd = ["--enable-ldw-opt=true" if c == "--enable-ldw-opt=false"
                   else c for c in cmd]
        return orig(cmd, *a, **kw)

    _bu.run_command = patched
    _bu._ldw_opt_patched = True


_enable_ldw_opt()


# --------------------------------------------------------------------------
# phase 1: per-core attention over a pair of heads (fp8 + DoubleRow)
# --------------------------------------------------------------------------

PF8 = mybir.dt.float8e4
F8_NP = ml_dtypes.float8_e4m3
WS = 32.0          # q/k/v weight upscale; folded out via exp scale + host div


class _Fillers:
    """FIFO of deferred PE-work emitters, drained one unit at a time into
    the exp-gated attention loop so the tensor engine never idles."""

    def __init__(self):
        self.q = []

    def add(self, fn):
        self.q.append(fn)

    def take(self, n):
        out, self.q = self.q[:n], self.q[n:]
        return out

    def drain(self):
        out, self.q = self.q, []
        return out


def build_phase1(B, T, C, DH):
    HP = 2                      # heads per core
    DA = DH + 1                 # head dim + ones row (softmax denominator)
    DAP = 80                    # DA padded so the DoubleRow pair stride is 16B-aligned
    NKK = C // 256              # DoubleRow contraction pairs
    NST = T // P                # key stripes of 128
    NM = T // 512               # query blocks of 512
    scale = float(C) ** -0.5 / (WS * WS)
    DR = mybir.MatmulPerfMode.DoubleRow

    nc = bacc.Bacc("TRN2", debug=False)
    # token-sliced so each (b, n, kk) chunk is contiguous and the first
    # projection can start after one 128KB DMA
    xT_d = nc.dram_tensor("xT8", [B, NM, NKK, P, 2, 512], PF8,
                          kind="ExternalInput").ap()
    wq_d = nc.dram_tensor("wq8", [NKK, P, 2, HP * DH], PF8,
                          kind="ExternalInput").ap()
    wk_d = nc.dram_tensor("wk8", [NKK, P, 2, HP * DH], PF8,
                          kind="ExternalInput").ap()
    wv_d = nc.dram_tensor("wv8", [2 * NKK, P, HP * DH], PF8,
                          kind="ExternalInput").ap()
    ot_d = nc.dram_tensor("ot", [B, HP, DA, T], FP, kind="ExternalOutput").ap()

    with tile.TileContext(nc) as tc, ExitStack() as ctx:
        const = ctx.enter_context(tc.tile_pool(name="const", bufs=1))
        xp = ctx.enter_context(tc.tile_pool(name="xp", bufs=1))
        wp = ctx.enter_context(tc.tile_pool(name="wp", bufs=1))
        qkp = ctx.enter_context(tc.tile_pool(name="qkp", bufs=1))
        vap = ctx.enter_context(tc.tile_pool(name="vap", bufs=1))
        ptkp = ctx.enter_context(tc.tile_pool(name="ptkp", bufs=6))
        otp = ctx.enter_context(tc.tile_pool(name="otp", bufs=1))
        pp = ctx.enter_context(tc.tile_pool(name="pp", bufs=2, space="PSUM"))
        sp = ctx.enter_context(tc.tile_pool(name="sp", bufs=2, space="PSUM"))
        op = ctx.enter_context(tc.tile_pool(name="op", bufs=1, space="PSUM"))

        # additive causal mask for a diagonal 128x128 block of S^T [s', q']:
        # 0 where q' >= s', -1e30 where q' < s'
        negmask = const.tile([P, P], FP)
        nc.gpsimd.memset(negmask[:], 0.0)
        nc.gpsimd.affine_select(
            out=negmask[:], in_=negmask[:],
            compare_op=mybir.AluOpType.is_ge, fill=-1e30,
            base=0, pattern=[[1, P]], channel_multiplier=-1)
        # preload the Exp table set before the attention loop needs it
        warm = const.tile([P, 1], FP)
        nc.vector.memset(warm[:], 0.0)
        nc.scalar.activation(warm[:], warm[:], AF.Exp, scale=1.0)

        wqs, wks, wvs = [], [], []
        for kk in range(NKK):
            wq = wp.tile([P, 2, HP * DH], PF8, tag=f"wq{kk}")
            nc.sync.dma_start(out=wq[:], in_=wq_d[kk])
            wk = wp.tile([P, 2, HP * DH], PF8, tag=f"wk{kk}")
            nc.sync.dma_start(out=wk[:], in_=wk_d[kk])
            wqs.append(wq)
            wks.append(wk)
        for c in range(2 * NKK):
            wv = wp.tile([P, HP * DH], PF8, tag=f"wv{c}")
            nc.sync.dma_start(out=wv[:], in_=wv_d[c])
            wvs.append(wv)

        xts, qts, kts, vaugs, otsbs = {}, {}, {}, {}, {}

        def load_x(b):
            for n in range(NM):
                for kk in range(NKK):
                    t = xp.tile([P, 2, 512], PF8, tag=f"x{b}_{n}_{kk}")
                    nc.sync.dma_start(out=t[:], in_=xT_d[b, n, kk])
                    xts[(b, n, kk)] = t

        for b in range(B):
            qts[b] = qkp.tile([P, T], PF8, tag=f"qt{b}", name=f"qt{b}")
            kts[b] = qkp.tile([P, T], PF8, tag=f"kt{b}", name=f"kt{b}")
            va = vap.tile([P, NST // 2, 2, HP, DAP], PF8, tag=f"va{b}",
                          name=f"va{b}")
            nc.vector.memset(va[:], 1.0)    # ones column survives in col DH
            vaugs[b] = va
            for h in range(HP):
                otsbs[(b, h)] = otp.tile([DA, T], FP, tag=f"ot{b}{h}", name=f"ot{b}{h}")

        def emit_qk_chunk(b, proj, n):
            # one 512-token chunk of the Q or K projection (NKK DR matmuls)
            wt = wqs if proj == 0 else wks
            dst = qts[b] if proj == 0 else kts[b]
            ps = pp.tile([P, 512], FP, tag="pp")
            for kk in range(NKK):
                nc.tensor.matmul(
                    ps[:], wt[kk][:],
                    xts[(b, n, kk)][:, :, :],
                    start=(kk == 0), stop=(kk == NKK - 1), perf_mode=DR)
            nc.vector.tensor_copy(dst[:, n * 512:(n + 1) * 512], ps[:])

        def emit_v_stripe(b, s):
            # V^T stripe s directly in [token, head-dim] layout
            n, so = s // 4, (s % 4) * P
            ps = pp.tile([P, 512], FP, tag="pp")
            out = ps[:, 0:HP * DH]
            for c in range(2 * NKK):
                nc.tensor.matmul(
                    out, xts[(b, n, c // 2)][:, c % 2, so:so + P],
                    wvs[c][:],
                    start=(c == 0), stop=(c == 2 * NKK - 1))
            for h in range(HP):
                nc.vector.tensor_copy(
                    vaugs[b][:, s // 2, s % 2, h, 0:DH],
                    ps[:, h * DH:(h + 1) * DH])

        def emit_spv_block(b, m, fillers):
            o_tiles = [op.tile([DA, 512], FP, tag=f"o{h}", name=f"o{h}")
                       for h in range(HP)]
            npairs = 2 * (m + 1)
            for jj in range(npairs):
                ptk = ptkp.tile([P, 2, HP, 512], PF8, tag="ptk")
                for ss in range(2):
                    j = 2 * jj + ss
                    s0 = j * P
                    diag = s0 >= m * 512
                    off = s0 - m * 512 if diag else 0
                    st = sp.tile([P, HP, 512], FP, tag="st")
                    for h in range(HP):
                        hs = slice(h * DH, (h + 1) * DH)
                        nc.tensor.matmul(
                            st[:, h, off:512],
                            kts[b][hs, s0:s0 + P],
                            qts[b][hs, m * 512 + off:(m + 1) * 512],
                            start=True, stop=True,
                            tile_position=(h * DH, 0))
                    if diag:
                        for h in range(HP):
                            nc.vector.tensor_add(
                                st[:, h, off:off + P],
                                st[:, h, off:off + P], negmask[:])
                        if off > 0:
                            nc.vector.memset(ptk[:, ss, :, 0:off], 0.0)
                    nc.scalar.activation(
                        ptk[:, ss, :, off:512], st[:, :, off:512],
                        AF.Exp, scale=scale)
                    for f in fillers.take(1):
                        f()
                for h in range(HP):
                    nc.tensor.matmul(
                        o_tiles[h][:], vaugs[b][:, jj, :, h, 0:DA],
                        ptk[:, :, h, :],
                        start=(jj == 0), stop=(jj == npairs - 1),
                        perf_mode=DR)
            for h in range(HP):
                nc.vector.tensor_copy(
                    otsbs[(b, h)][:, m * 512:(m + 1) * 512], o_tiles[h][:])

        # ---- batch 0 prelude: Q/K projections + first V stripes
        load_x(0)
        for n in range(NM):
            emit_qk_chunk(0, 0, n)
            emit_qk_chunk(0, 1, n)
        for s in range(4):
            emit_v_stripe(0, s)
        load_x(1)

        # b0/b1 blocks interleaved so the exp (ACT) stream never drains and
        # the PE always has S/PV + filler (V / Q/K projection) work. The
        # filler FIFO order matches each unit's first use in block order.
        fill = _Fillers()
        for s in range(4, 12):
            fill.add(lambda s=s: emit_v_stripe(0, s))
        for n in range(NM):
            fill.add(lambda n=n: emit_qk_chunk(1, 0, n))
            fill.add(lambda n=n: emit_qk_chunk(1, 1, n))
        for s in range(4):
            fill.add(lambda s=s: emit_v_stripe(1, s))
        for s in range(12, NST):
            fill.add(lambda s=s: emit_v_stripe(0, s))
        for s in range(4, NST):
            fill.add(lambda s=s: emit_v_stripe(1, s))

        for b, m in ((0, 0), (0, 1), (0, 2), (1, 0),
                     (0, 3), (1, 1), (1, 2), (1, 3)):
            emit_spv_block(b, m, fill)
            if (b, m) == (0, 3):
                for h in range(HP):
                    nc.scalar.dma_start(out=ot_d[0, h], in_=otsbs[(0, h)][:])
        for f in fill.drain():
            f()
        for h in range(HP):
            nc.scalar.dma_start(out=ot_d[1, h], in_=otsbs[(1, h)][:])
    nc.compile()
    return nc


def build_phase1_v1(B, T, C, DH):
    HP = 2                      # heads per core
    DA = DH + 1                 # head dim + ones row (softmax denominator)
    NCC = C // P                # contraction chunks
    NT = T // P                 # key/value blocks of 128
    NQ = T // 512               # query chunks of 512
    NK = T // 1024              # query tiles of 1024
    scale = float(C) ** -0.5    # NOTE: reference scales by C**-0.5, not DH

    nc = bacc.Bacc("TRN2", debug=False)
    xT_d = nc.dram_tensor("xT", [B, C, T], BF, kind="ExternalInput").ap()
    wq_d = nc.dram_tensor("wq", [C, HP * DH], BF, kind="ExternalInput").ap()
    wk_d = nc.dram_tensor("wk", [C, HP * DH], BF, kind="ExternalInput").ap()
    wv_d = nc.dram_tensor("wv", [C, HP * DH], BF, kind="ExternalInput").ap()
    ot_d = nc.dram_tensor("ot", [B, HP, DA, T], FP, kind="ExternalOutput").ap()

    with tile.TileContext(nc) as tc, ExitStack() as ctx:
        const = ctx.enter_context(tc.tile_pool(name="const", bufs=1))
        xpool = ctx.enter_context(tc.tile_pool(name="xp", bufs=1))
        wpool = ctx.enter_context(tc.tile_pool(name="wp", bufs=1))
        qk_pool = ctx.enter_context(tc.tile_pool(name="qk", bufs=2))
        vt_pool = ctx.enter_context(tc.tile_pool(name="vtp", bufs=2))
        vaug_pool = ctx.enter_context(tc.tile_pool(name="vaug", bufs=2))
        pt_pool = ctx.enter_context(tc.tile_pool(name="pt", bufs=4))
        ot_pool = ctx.enter_context(tc.tile_pool(name="otp", bufs=2))

        # additive mask for the diagonal 128x128 block of S^T [s', q']:
        # 0 where q' >= s' (causal-valid), -1e30 where q' < s'
        negmask = const.tile([P, P], FP)
        nc.gpsimd.memset(negmask[:], 0.0)
        nc.gpsimd.affine_select(
            out=negmask[:], in_=negmask[:],
            compare_op=mybir.AluOpType.is_ge, fill=-1e30,
            base=0, pattern=[[1, P]], channel_multiplier=-1)
        ident = const.tile([P, P], BF)
        make_identity(nc, ident[:])
        ones_col = const.tile([P, NT * HP, 1], FP)
        nc.vector.memset(ones_col[:], 1.0)

        # weight chunks, loaded once
        wts = {}
        for name, src in (("q", wq_d), ("k", wk_d), ("v", wv_d)):
            wts[name] = []
            for c in range(NCC):
                t = wpool.tile([P, HP * DH], BF, tag=f"w{name}{c}")
                nc.sync.dma_start(out=t[:], in_=src[c * P:(c + 1) * P, :])
                wts[name].append(t)

        for b in range(B):
            xts = []
            for c in range(NCC):
                xt = xpool.tile([P, T], BF, tag=f"x{c}")
                nc.sync.dma_start(out=xt[:], in_=xT_d[b, c * P:(c + 1) * P, :])
                xts.append(xt)

            qt = qk_pool.tile([P, T], BF, tag="qt")
            kt = qk_pool.tile([P, T], BF, tag="kt")
            vaug = vaug_pool.tile([P, NT * HP, DA], BF, tag="vaug")
            # ones column per head-block (softmax denominator row of O^T)
            nc.vector.tensor_copy(vaug[:, :, DA - 1:DA], ones_col[:])

            with tc.tile_pool(name="proj_ps", bufs=3, space="PSUM") as proj_ps, \
                 tc.tile_pool(name="vt_ps", bufs=2, space="PSUM") as vt_ps:
                for wt, dst in ((wts["q"], qt), (wts["k"], kt)):
                    for n in range(NQ):
                        ps = proj_ps.tile([P, 512], FP, tag="proj")
                        for c in range(NCC):
                            nc.tensor.matmul(
                                ps[:], wt[c][:], xts[c][:, n * 512:(n + 1) * 512],
                                start=(c == 0), stop=(c == NCC - 1))
                        nc.vector.tensor_copy(dst[:, n * 512:(n + 1) * 512], ps[:])
                # V, then transpose into [s, d] layout with ones columns
                for n in range(NQ):
                    ps = proj_ps.tile([P, 512], FP, tag="proj")
                    for c in range(NCC):
                        nc.tensor.matmul(
                            ps[:], wts["v"][c][:], xts[c][:, n * 512:(n + 1) * 512],
                            start=(c == 0), stop=(c == NCC - 1))
                    vt = vt_pool.tile([P, 512], BF, tag="vt")
                    nc.vector.tensor_copy(vt[:], ps[:])
                    for u in range(4):
                        j = 4 * n + u
                        tp = vt_ps.tile([P, P], BF, tag="vtp")
                        nc.tensor.transpose(tp[:], vt[:, u * P:(u + 1) * P], ident[:])
                        nc.vector.tensor_copy(
                            vaug[:, j * HP, 0:DH], tp[:, 0:DH])
                        nc.vector.tensor_copy(
                            vaug[:, j * HP + 1, 0:DH], tp[:, DH:2 * DH])

            with tc.tile_pool(name="s_ps", bufs=2, space="PSUM") as s_ps, \
                 tc.tile_pool(name="o_ps", bufs=1, space="PSUM") as o_ps:
                ot_sbs = [ot_pool.tile([DA, T], FP, tag=f"ot{h}", name=f"ot{h}")
                          for h in range(HP)]
                for k in range(NK):
                    q_lo = 1024 * k
                    q_hi = 1024 * (k + 1)
                    o_tiles = [o_ps.tile([DA, 1024], FP, tag=f"o{h}", name=f"o{h}")
                               for h in range(HP)]
                    for j in range(8 * (k + 1)):
                        s0 = j * P
                        a0 = max(s0, q_lo)
                        # 512-grid chunks of the valid q range in this stripe
                        chunks = []
                        m0 = a0 // 512
                        for m in range(m0, q_hi // 512):
                            a = max(a0, m * 512)
                            e = (m + 1) * 512
                            chunks.append((a, e))
                        stl = [s_ps.tile([P, 1024], FP, tag="s", name="s")
                               for _ in range(HP)]
                        # emit head pairs adjacently: rows 0-63 (head A) and
                        # 64-127 (head B) run concurrently in the PE array
                        for (a, e) in chunks:
                            for h in range(HP):
                                hs = slice(h * DH, (h + 1) * DH)
                                nc.tensor.matmul(
                                    stl[h][:, a - q_lo:e - q_lo],
                                    kt[hs, s0:s0 + P], qt[hs, a:e],
                                    start=True, stop=True,
                                    tile_position=(h * DH, 0))
                        if q_lo <= s0:
                            for h in range(HP):
                                # diagonal block: additive causal mask
                                nc.vector.tensor_add(
                                    stl[h][:, s0 - q_lo:s0 - q_lo + P],
                                    stl[h][:, s0 - q_lo:s0 - q_lo + P],
                                    negmask[:])
                        for h in range(HP):
                            ptk = pt_pool.tile([P, 1024], BF, tag="pt")
                            nc.scalar.activation(
                                ptk[:, a0 - q_lo:1024], stl[h][:, a0 - q_lo:1024],
                                AF.Exp, scale=scale)
                            va = vaug[:, j * HP + h, :]
                            for (a, e) in chunks:
                                last_j = e // P - 1
                                nc.tensor.matmul(
                                    o_tiles[h][:, a - q_lo:e - q_lo],
                                    va, ptk[:, a - q_lo:e - q_lo],
                                    start=(j == 0), stop=(j == last_j))
                    for h in range(HP):
                        nc.vector.tensor_copy(
                            ot_sbs[h][:, q_lo:q_hi], o_tiles[h][:])
                for h in range(HP):
                    nc.sync.dma_start(out=ot_d[b, h], in_=ot_sbs[h][:])
    nc.compile()
    return nc


# --------------------------------------------------------------------------
# phase 2: per-core Wo projection + residual + rmsnorm + FFN + rmsnorm
# --------------------------------------------------------------------------

def build_phase2(NTOK, C, DFF):
    NTB = NTOK // P             # 4 token tiles of 128
    NCH = C // P                # 8 channel chunks
    NDF = DFF // P              # 32 dff chunks
    NG = DFF // 512             # 8 dff groups of 512

    nc = bacc.Bacc("TRN2", debug=False)
    xc_d = nc.dram_tensor("xc", [NTOK, C], FP, kind="ExternalInput").ap()
    at_d = nc.dram_tensor("attnT", [C, NTOK], BF, kind="ExternalInput").ap()
    wo_d = nc.dram_tensor("wo", [C, C], BF, kind="ExternalInput").ap()
    # host-permuted W1: [g, c, 128, 512] so each (g, c) chunk is contiguous
    w1_d = nc.dram_tensor("w1p", [NG, NCH, P, 512], BF, kind="ExternalInput").ap()
    w2_d = nc.dram_tensor("w2", [DFF, C], BF, kind="ExternalInput").ap()
    g1_d = nc.dram_tensor("g1", [C], FP, kind="ExternalInput").ap()
    g2_d = nc.dram_tensor("g2", [C], FP, kind="ExternalInput").ap()
    b1_d = nc.dram_tensor("b1", [DFF], FP, kind="ExternalInput").ap()
    b2_d = nc.dram_tensor("b2", [C], FP, kind="ExternalInput").ap()
    out_d = nc.dram_tensor("out", [NTOK, C], FP, kind="ExternalOutput").ap()

    def bcast_rows(src_ap, cols):
        # DRAM vector [cols] -> [P, cols] (same row in every partition)
        return bass.AP(tensor=src_ap.tensor, offset=src_ap.offset,
                       ap=[[0, P], [1, cols]])

    with tile.TileContext(nc) as tc, ExitStack() as ctx:
        const = ctx.enter_context(tc.tile_pool(name="const", bufs=1))
        work = ctx.enter_context(tc.tile_pool(name="work", bufs=2))
        stats = ctx.enter_context(tc.tile_pool(name="stats", bufs=4))
        h_pool = ctx.enter_context(tc.tile_pool(name="hp", bufs=1))
        hb2p = ctx.enter_context(tc.tile_pool(name="hb2p", bufs=1))
        ht_pool = ctx.enter_context(tc.tile_pool(name="htp", bufs=1))
        at_pool = ctx.enter_context(tc.tile_pool(name="atp", bufs=1))
        w2sb = ctx.enter_context(tc.tile_pool(name="w2sb", bufs=10))

        ident = const.tile([P, P], BF)
        make_identity(nc, ident[:])
        eps_t = const.tile([P, 1], FP)
        nc.vector.memset(eps_t[:], EPS)
        # preload the Sqrt/Square table set before stage-0 norms need it
        warm = stats.tile([P, 1], FP, tag="warm")
        nc.scalar.activation(warm[:], eps_t[:], AF.Sqrt, scale=1.0)
        # broadcasts go on the vector queue so the sync queue serves the
        # stage-0 operands (attnT/Wo/xc) first
        g1b = const.tile([P, C], FP)
        nc.scalar.dma_start(out=g1b[:], in_=bcast_rows(g1_d, C))
        g2b = const.tile([P, C], FP)
        nc.scalar.dma_start(out=g2b[:], in_=bcast_rows(g2_d, C))
        b2b = const.tile([P, C], FP)
        nc.scalar.dma_start(out=b2b[:], in_=bcast_rows(b2_d, C))
        b1b = const.tile([P, DFF], FP)
        nc.scalar.dma_start(out=b1b[:], in_=bcast_rows(b1_d, DFF))

        def rmsnorm_to(dst, src, gb):
            # dst = src * rsqrt(mean(src^2) + eps) * gb
            sq = work.tile([P, C], FP, tag="sq")
            ssum = stats.tile([P, 1], FP, tag="ssum")
            nc.scalar.activation(sq[:], src[:], AF.Square, accum_out=ssum[:])
            rstd = stats.tile([P, 1], FP, tag="rstd")
            nc.scalar.activation(rstd[:], ssum[:], AF.Sqrt,
                                 scale=1.0 / C, bias=eps_t[:])
            rinv = stats.tile([P, 1], FP, tag="rinv")
            nc.vector.reciprocal(rinv[:], rstd[:])
            nc.vector.scalar_tensor_tensor(
                dst[:], src[:], rinv[:], gb[:],
                op0=mybir.AluOpType.mult, op1=mybir.AluOpType.mult)

        # ---- stage 0: o = attnT^T @ Wo; r1 = x + o; h = rmsnorm(r1)*g1; hT
        hs = []
        hts = [ht_pool.tile([P, NTOK], BF, tag=f"ht{c}", name=f"ht{c}")
               for c in range(NCH)]
        with tc.tile_pool(name="o_ps", bufs=2, space="PSUM") as o_ps, \
             tc.tile_pool(name="t_ps", bufs=2, space="PSUM") as t_ps, \
             tc.tile_pool(name="wop", bufs=NCH) as wop, \
             tc.tile_pool(name="atsp", bufs=NCH) as atsp, \
             tc.tile_pool(name="xcp", bufs=1) as xcp, \
             tc.tile_pool(name="hbf", bufs=2) as hbf:
            atts, wots = [], []
            for c in range(NCH):
                att = atsp.tile([P, NTOK], BF, tag="at", name="at")
                nc.sync.dma_start(out=att[:], in_=at_d[c * P:(c + 1) * P, :])
                wot = wop.tile([P, C], BF, tag="wo", name="wo")
                nc.sync.dma_start(out=wot[:], in_=wo_d[c * P:(c + 1) * P, :])
                atts.append(att)
                wots.append(wot)
            xcs = []
            for tb in range(NTB):
                t = xcp.tile([P, C], FP, tag=f"xc{tb}")
                nc.sync.dma_start(out=t[:], in_=xc_d[tb * P:(tb + 1) * P, :])
                xcs.append(t)

            def wo_mms(tb, o):
                for c in range(NCH):
                    for half in range(2):
                        nc.tensor.matmul(
                            o[:, half * 512:(half + 1) * 512],
                            atts[c][:, tb * P:(tb + 1) * P],
                            wots[c][:, half * 512:(half + 1) * 512],
                            start=(c == 0), stop=(c == NCH - 1))

            def norm_h(tb, o):
                r1 = work.tile([P, C], FP, tag="r1")
                nc.vector.tensor_add(r1[:], o[:], xcs[tb][:])
                h = h_pool.tile([P, C], FP, tag=f"h{tb}")
                rmsnorm_to(h, r1, g1b)
                hs.append(h)
                hb = hbf.tile([P, C], BF, tag="hbf")
                nc.vector.tensor_copy(hb[:], h[:])
                return hb

            def trans_h(tb, hb):
                for c in range(NCH):
                    tp = t_ps.tile([P, P], BF, tag="tp")
                    nc.tensor.transpose(
                        tp[:], hb[:, c * P:(c + 1) * P], ident[:])
                    nc.vector.tensor_copy(hts[c][:, tb * P:(tb + 1) * P], tp[:])

            # pipeline: Wo(tb) | Wo(tb+1) + trans(tb) | ... so the PE never
            # waits on the norm chain
            o_tiles, hbs = [], []
            for tb in range(NTB):
                o = o_ps.tile([P, C], FP, tag="o")
                wo_mms(tb, o)
                o_tiles.append(o)
                hbs.append(norm_h(tb, o))
                if tb >= 1:
                    trans_h(tb - 1, hbs[tb - 1])
            trans_h(NTB - 1, hbs[NTB - 1])

        # ---- stage 2: aT = silu(W1^T @ h^T + b1)  [dff, tok] bf16
        ats = []
        with tc.tile_pool(name="a_ps", bufs=8, space="PSUM") as a_ps, \
             tc.tile_pool(name="w1p", bufs=24) as w1p, \
             tc.tile_pool(name="sgp", bufs=3) as sgp:
            for g in range(NG):
                w1ts = []
                for c in range(NCH):
                    w1t = w1p.tile([P, 512], BF, tag="w1")
                    nc.gpsimd.dma_start(out=w1t[:], in_=w1_d[g, c])
                    w1ts.append(w1t)
                aps = [a_ps.tile([P, NTOK], FP, tag="a", name="a")
                       for _ in range(4)]
                for c in range(NCH):
                    for u in range(4):
                        nc.tensor.matmul(
                            aps[u][:], w1ts[c][:, u * P:(u + 1) * P],
                            hts[c][:],
                            start=(c == 0), stop=(c == NCH - 1))
                for u in range(4):
                    d = 4 * g + u
                    sg = sgp.tile([P, NTOK], FP, tag="sg")
                    nc.scalar.activation(sg[:], aps[u][:], AF.Sigmoid,
                                         bias=b1b[:, d:d + 1], scale=1.0)
                    at_t = at_pool.tile([P, NTOK], BF, tag=f"at{d}")
                    # silu(z) for z = a + b1: (a + b1) * sigmoid(a + b1)
                    nc.vector.scalar_tensor_tensor(
                        at_t[:], aps[u][:], b1b[:, d:d + 1], sg[:],
                        op0=mybir.AluOpType.add, op1=mybir.AluOpType.mult)
                    ats.append(at_t)

        # hb[tb] = h + b2, precomputed on DVE while the PE runs W2
        hb2s = []
        for tb in range(NTB):
            hb2 = hb2p.tile([P, C], FP, tag=f"hb2_{tb}")
            nc.vector.tensor_add(hb2[:], hs[tb][:], b2b[:])
            hb2s.append(hb2)

        # preload the Sqrt table set during W2 so the tail norm doesn't
        # pay the ~2.7us table switch
        dummy = stats.tile([P, 1], FP, tag="dummy")
        nc.scalar.activation(dummy[:], eps_t[:], AF.Sqrt, scale=1.0)

        # ---- stage 3: f = aT^T @ W2; r2 = h + b2 + f; out = rmsnorm(r2)*g2
        with tc.tile_pool(name="f_ps", bufs=1, space="PSUM") as f_ps:
            fts = [f_ps.tile([P, C], FP, tag=f"f{tb}", name=f"f{tb}")
                   for tb in range(NTB)]
            for d in range(NDF):
                w2t = w2sb.tile([P, C], BF, tag="w2")
                nc.gpsimd.dma_start(out=w2t[:], in_=w2_d[d * P:(d + 1) * P, :])
                for tb in range(NTB):
                    for half in range(2):
                        nc.tensor.matmul(
                            fts[tb][:, half * 512:(half + 1) * 512],
                            ats[d][:, tb * P:(tb + 1) * P],
                            w2t[:, half * 512:(half + 1) * 512],
                            start=(d == 0), stop=(d == NDF - 1))
            for tb in range(NTB):
                r2 = work.tile([P, C], FP, tag="r2")
                nc.vector.tensor_add(r2[:], fts[tb][:], hb2s[tb][:])
                o = work.tile([P, C], FP, tag="outt")
                rmsnorm_to(o, r2, g2b)
                nc.scalar.dma_start(out=out_d[tb * P:(tb + 1) * P, :], in_=o[:])
    nc.compile()
    return nc


# --------------------------------------------------------------------------
# host orchestration
# --------------------------------------------------------------------------

_CACHE = {}


def _phase1(B, T, C, DH):
    key = ("p1", B, T, C, DH)
    if key not in _CACHE:
        _CACHE[key] = build_phase1(B, T, C, DH)
    return _CACHE[key]


def _phase2(NTOK, C, DFF):
    key = ("p2", NTOK, C, DFF)
    if key not in _CACHE:
        _CACHE[key] = build_phase2(NTOK, C, DFF)
    return _CACHE[key]


def _run(nc, in_maps, ldw_opt=False):
    import os
    import concourse.bass_utils as _bu
    _bu._ldw_opt_enable = ldw_opt    # DR ldweights are incompatible with it
    trace = bool(os.environ.get("KERNEL_TRACE"))
    kwargs = {}
    if trace:
        _install_ntff_hook_shim()
        tdir = os.environ.get("KERNEL_TRACE_DIR")
        if tdir:
            phase_dir = os.path.join(tdir, f"phase{len(LAST_EXEC_NS)}")
            os.makedirs(phase_dir, exist_ok=True)
            kwargs["tmpdir"] = phase_dir
    res = run_bass_kernel_spmd(nc, in_maps, core_ids=list(range(N_CORES)),
                               trace=trace, **kwargs)
    LAST_EXEC_NS.append(res.exec_time_ns)
    return res.results


def kernel(x, Wq, Wk, Wv, Wo, bo, W1, b1, W2, b2, g1, g2):
    f32 = lambda a: np.ascontiguousarray(np.asarray(a), dtype=np.float32)
    x = f32(x)
    Wq, Wk, Wv, Wo, bo = f32(Wq), f32(Wk), f32(Wv), f32(Wo), f32(bo)
    W1, b1, W2, b2, g1, g2 = f32(W1), f32(b1), f32(W2), f32(b2), f32(g1), f32(g2)

    B, T, C = x.shape
    H, _, DH = Wq.shape
    HP = H // N_CORES           # heads per core (2)
    DA = DH + 1
    LAST_EXEC_NS.clear()

    # ---- phase 1
    nc1 = _phase1(B, T, C, DH)
    NKK = C // 256
    # [B,T,C] -> [B, n, kk, 128, 2, 512] with channel c = (2*kk+s)*128 + p
    # and token t = n*512 + t'
    xT8 = np.ascontiguousarray(
        x.transpose(0, 2, 1).reshape(B, NKK, 2, 128, T // 512, 512)
        .transpose(0, 4, 1, 3, 2, 5)).astype(F8_NP)
    in1 = []
    for i in range(N_CORES):
        pq = Wq[HP * i:HP * (i + 1)].transpose(1, 0, 2).reshape(C, HP * DH)
        pk = Wk[HP * i:HP * (i + 1)].transpose(1, 0, 2).reshape(C, HP * DH)
        pv = Wv[HP * i:HP * (i + 1)].transpose(1, 0, 2).reshape(C, HP * DH)
        pair = lambda w: np.ascontiguousarray(
            (w * WS).reshape(NKK, 2, 128, HP * DH)
            .transpose(0, 2, 1, 3)).astype(F8_NP)
        in1.append({"xT8": xT8,
                    "wq8": pair(pq),
                    "wk8": pair(pk),
                    "wv8": np.ascontiguousarray(
                        (pv * WS).reshape(2 * NKK, 128, HP * DH)
                    ).astype(F8_NP)})
    res1 = _run(nc1, in1)

    attn = np.empty((B, T, C), np.float32)
    for i in range(N_CORES):
        ot = res1[i]["ot"]                    # [B, HP, DA, T]
        o = ot[:, :, :DH, :]
        den = ot[:, :, DH, :] * WS            # V carries a x32 scale
        on = o / den[:, :, None, :]
        for hh in range(HP):
            hcol = (HP * i + hh) * DH
            attn[:, :, hcol:hcol + DH] = on[:, hh].transpose(0, 2, 1)

    # ---- phase 2
    NTOK = B * T // N_CORES
    nc2 = _phase2(NTOK, C, W1.shape[1])
    xf = x.reshape(B * T, C) + bo             # fold bo into the residual
    af = attn.reshape(B * T, C)
    DFF = W1.shape[1]
    wo_bf = Wo.astype(BF_NP)
    # [C, DFF] -> [g, c, 128, 512] so each (g, c) chunk is DMA-contiguous
    w1_bf = np.ascontiguousarray(
        W1.reshape(C // 128, 128, DFF // 512, 512)
        .transpose(2, 0, 1, 3)).astype(BF_NP)
    w2_bf = W2.astype(BF_NP)
    in2 = []
    for k in range(N_CORES):
        sl = slice(k * NTOK, (k + 1) * NTOK)
        in2.append({
            "xc": np.ascontiguousarray(xf[sl]),
            "attnT": np.ascontiguousarray(af[sl].T).astype(BF_NP),
            "wo": wo_bf, "w1p": w1_bf, "w2": w2_bf,
            "g1": g1, "g2": g2, "b1": b1, "b2": b2,
        })
    res2 = _run(nc2, in2)
    out = np.concatenate([res2[k]["out"] for k in range(N_CORES)], axis=0)
    return out.reshape(B, T, C)



# revision 29
# speedup vs baseline: 1.4166x; 1.0679x over previous
"""Trainium2 Bass kernel for a dense transformer block, distributed over 8
NeuronCores.

Sharding:
  phase 1 (attention): tensor-parallel over heads — each core computes 2 of
    the 16 heads end-to-end (QKV projections + causal softmax(QK^T)V), and
    returns the unnormalized per-head output O^T together with the softmax
    denominators (obtained via a ones-column appended to V).
  phase 2 (Wo + norms + FFN): data-parallel over tokens — each core handles
    512 of the 4096 token rows with replicated weights.

The host glues the phases: transposes x, normalizes/concats heads, and
re-shards tokens.  All matmuls run as float32r (full-rate fp32 PE mode).
"""

import math
from contextlib import ExitStack

import ml_dtypes
import numpy as np

BF_NP = ml_dtypes.bfloat16

import concourse.bass as bass
import concourse.mybir as mybir
import concourse.tile as tile
from concourse import bacc
from concourse.bass_utils import run_bass_kernel_spmd
from concourse.masks import make_identity, make_upper_triangular

FP = mybir.dt.float32
FPR = mybir.dt.float32r
BF = mybir.dt.bfloat16
AF = mybir.ActivationFunctionType

N_CORES = 8
P = 128
EPS = 1e-6

# exec times (ns) of the most recent kernel() call, one entry per phase, when
# tracing was enabled via BASS_TRACE=1; None entries otherwise.
LAST_EXEC_NS = []


def _install_ntff_hook_shim():
    """Provide antenv.axon_hooks when the image lacks it, so trace=True can
    drive NTFF profiling through libaxon_pjrt's C ABI (same contract as
    trn_boot's step-6 hook). No-op if the real module exists or the .so is
    missing/old."""
    try:
        import antenv.axon_hooks  # noqa: F401
        return
    except ImportError:
        pass
    import contextlib
    import ctypes
    import sys
    import types

    try:
        lib = ctypes.CDLL("/opt/axon/libaxon_pjrt.so")
    except OSError:
        return
    if not hasattr(lib, "axon_start_nrt_profile"):
        return
    lib.axon_start_nrt_profile.argtypes = [
        ctypes.POINTER(ctypes.c_int64), ctypes.c_size_t]
    lib.axon_start_nrt_profile.restype = ctypes.c_int64
    lib.axon_stop_nrt_profile.argtypes = [ctypes.c_char_p]
    lib.axon_stop_nrt_profile.restype = ctypes.c_int64

    @contextlib.contextmanager
    def _hook(output_dir, device_ids):
        import jax
        jax.devices()
        if device_ids:
            ids = (ctypes.c_int64 * len(device_ids))(*device_ids)
            rc = lib.axon_start_nrt_profile(ids, len(device_ids))
        else:
            rc = lib.axon_start_nrt_profile(None, 0)
        if rc != 0:
            raise RuntimeError(f"axon_start_nrt_profile rc={rc}")
        try:
            yield
        finally:
            n = lib.axon_stop_nrt_profile(str(output_dir).encode())
            if n < 0:
                raise RuntimeError(f"axon_stop_nrt_profile rc={n}")

    mod = types.ModuleType("antenv.axon_hooks")
    mod.get_axon_ntff_profile_hook = lambda: _hook

    def set_axon_ntff_profile_hook(h):
        mod.get_axon_ntff_profile_hook = lambda: h

    mod.set_axon_ntff_profile_hook = set_axon_ntff_profile_hook
    import antenv
    antenv.axon_hooks = mod
    sys.modules["antenv.axon_hooks"] = mod


def _fpr(ap):
    return ap.bitcast(FPR)


def _enable_ldw_opt():
    """Flip walrus's --enable-ldw-opt to true so LDWEIGHTS overlaps with
    in-flight matmuls (background weight buffer). Verified by the rel-err
    check; idempotent."""
    import concourse.bass_utils as _bu
    if getattr(_bu, "_ldw_opt_patched", False):
        return
    orig = _bu.run_command

    def patched(cmd, *a, **kw):
        if isinstance(cmd, list) and getattr(_bu, "_ldw_opt_enable", False):
            cmd = ["--enable-ldw-opt=true" if c == "--enable-ldw-opt=false"
                   else c for c in cmd]
        return orig(cmd, *a, **kw)

    _bu.run_command = patched
    _bu._ldw_opt_patched = True


_enable_ldw_opt()


# --------------------------------------------------------------------------
# phase 1: per-core attention over a pair of heads (fp8 + DoubleRow)
# --------------------------------------------------------------------------

PF8 = mybir.dt.float8e4
F8_NP = ml_dtypes.float8_e4m3
WS = 32.0          # q/k/v weight upscale; folded out via exp scale + host div


class _Fillers:
    """FIFO of deferred PE-work emitters, drained one unit at a time into
    the exp-gated attention loop so the tensor engine never idles."""

    def __init__(self):
        self.q = []

    def add(self, fn):
        self.q.append(fn)

    def take(self, n):
        out, self.q = self.q[:n], self.q[n:]
        return out

    def drain(self):
        out, self.q = self.q, []
        return out


def build_phase1(B, T, C, DH):
    HP = 2                      # heads per core
    DA = DH + 1                 # head dim + ones row (softmax denominator)
    DAP = 80                    # DA padded so the DoubleRow pair stride is 16B-aligned
    NKK = C // 256              # DoubleRow contraction pairs
    NST = T // P                # key stripes of 128
    NM = T // 512               # query blocks of 512
    scale = float(C) ** -0.5 / (WS * WS)
    DR = mybir.MatmulPerfMode.DoubleRow

    nc = bacc.Bacc("TRN2", debug=False)
    # token-sliced so each (b, n, kk) chunk is contiguous and the first
    # projection can start after one 128KB DMA
    xT_d = nc.dram_tensor("xT8", [B, NM, NKK, P, 2, 512], PF8,
                          kind="ExternalInput").ap()
    wq_d = nc.dram_tensor("wq8", [NKK, P, 2, HP * DH], PF8,
                          kind="ExternalInput").ap()
    wk_d = nc.dram_tensor("wk8", [NKK, P, 2, HP * DH], PF8,
                          kind="ExternalInput").ap()
    wv_d = nc.dram_tensor("wv8", [2 * NKK, P, HP * DH], PF8,
                          kind="ExternalInput").ap()
    ot_d = nc.dram_tensor("ot", [B, HP, DA, T], FP, kind="ExternalOutput").ap()

    with tile.TileContext(nc) as tc, ExitStack() as ctx:
        const = ctx.enter_context(tc.tile_pool(name="const", bufs=1))
        xp = ctx.enter_context(tc.tile_pool(name="xp", bufs=1))
        wp = ctx.enter_context(tc.tile_pool(name="wp", bufs=1))
        qkp = ctx.enter_context(tc.tile_pool(name="qkp", bufs=1))
        vap = ctx.enter_context(tc.tile_pool(name="vap", bufs=1))
        ptkp = ctx.enter_context(tc.tile_pool(name="ptkp", bufs=6))
        otp = ctx.enter_context(tc.tile_pool(name="otp", bufs=1))
        pp = ctx.enter_context(tc.tile_pool(name="pp", bufs=2, space="PSUM"))
        sp = ctx.enter_context(tc.tile_pool(name="sp", bufs=2, space="PSUM"))
        op = ctx.enter_context(tc.tile_pool(name="op", bufs=1, space="PSUM"))

        # preload the Exp table set before the attention loop needs it
        warm = const.tile([P, 1], FP)
        nc.vector.memset(warm[:], 0.0)
        nc.scalar.activation(warm[:], warm[:], AF.Exp, scale=1.0)

        wqs, wks, wvs = [], [], []
        for kk in range(NKK):
            wq = wp.tile([P, 2, HP * DH], PF8, tag=f"wq{kk}")
            nc.sync.dma_start(out=wq[:], in_=wq_d[kk])
            wk = wp.tile([P, 2, HP * DH], PF8, tag=f"wk{kk}")
            nc.sync.dma_start(out=wk[:], in_=wk_d[kk])
            wqs.append(wq)
            wks.append(wk)
        for c in range(2 * NKK):
            wv = wp.tile([P, HP * DH], PF8, tag=f"wv{c}")
            nc.sync.dma_start(out=wv[:], in_=wv_d[c])
            wvs.append(wv)

        xts, qts, kts, vaugs, otsbs = {}, {}, {}, {}, {}

        def load_x(b):
            for n in range(NM):
                for kk in range(NKK):
                    t = xp.tile([P, 2, 512], PF8, tag=f"x{b}_{n}_{kk}")
                    nc.sync.dma_start(out=t[:], in_=xT_d[b, n, kk])
                    xts[(b, n, kk)] = t

        for b in range(B):
            qts[b] = qkp.tile([P, T], PF8, tag=f"qt{b}", name=f"qt{b}")
            kts[b] = qkp.tile([P, T], PF8, tag=f"kt{b}", name=f"kt{b}")
            va = vap.tile([P, NST // 2, 2, HP, DAP], PF8, tag=f"va{b}",
                          name=f"va{b}")
            nc.vector.memset(va[:], 1.0)    # ones column survives in col DH
            vaugs[b] = va
            for h in range(HP):
                otsbs[(b, h)] = otp.tile([DA, T], FP, tag=f"ot{b}{h}", name=f"ot{b}{h}")

        def emit_qk_chunk(b, proj, n):
            # one 512-token chunk of the Q or K projection (NKK DR matmuls)
            wt = wqs if proj == 0 else wks
            dst = qts[b] if proj == 0 else kts[b]
            ps = pp.tile([P, 512], FP, tag="pp")
            for kk in range(NKK):
                nc.tensor.matmul(
                    ps[:], wt[kk][:],
                    xts[(b, n, kk)][:, :, :],
                    start=(kk == 0), stop=(kk == NKK - 1), perf_mode=DR)
            nc.vector.tensor_copy(dst[:, n * 512:(n + 1) * 512], ps[:])

        def emit_v_stripe(b, s):
            # V^T stripe s directly in [token, head-dim] layout
            n, so = s // 4, (s % 4) * P
            ps = pp.tile([P, 512], FP, tag="pp")
            out = ps[:, 0:HP * DH]
            for c in range(2 * NKK):
                nc.tensor.matmul(
                    out, xts[(b, n, c // 2)][:, c % 2, so:so + P],
                    wvs[c][:],
                    start=(c == 0), stop=(c == 2 * NKK - 1))
            for h in range(HP):
                nc.vector.tensor_copy(
                    vaugs[b][:, s // 2, s % 2, h, 0:DH],
                    ps[:, h * DH:(h + 1) * DH])

        def emit_spv_block(b, m, fillers):
            o_tiles = [op.tile([DA, 512], FP, tag=f"o{h}", name=f"o{h}")
                       for h in range(HP)]
            npairs = 2 * (m + 1)
            for jj in range(npairs):
                ptk = ptkp.tile([P, 2, HP, 512], PF8, tag="ptk")
                for ss in range(2):
                    j = 2 * jj + ss
                    s0 = j * P
                    diag = s0 >= m * 512
                    off = s0 - m * 512 if diag else 0
                    st = sp.tile([P, HP, 512], FP, tag="st")
                    for h in range(HP):
                        hs = slice(h * DH, (h + 1) * DH)
                        nc.tensor.matmul(
                            st[:, h, off:512],
                            kts[b][hs, s0:s0 + P],
                            qts[b][hs, m * 512 + off:(m + 1) * 512],
                            start=True, stop=True,
                            tile_position=(h * DH, 0))
                    if diag and off > 0:
                        nc.gpsimd.memset(ptk[:, ss, :, 0:off], 0.0)
                    nc.scalar.activation(
                        ptk[:, ss, :, off:512], st[:, :, off:512],
                        AF.Exp, scale=scale)
                    if diag:
                        # zero the non-causal triangle of the diagonal 128
                        # cols after the exp, off the ACT critical path
                        for h in range(HP):
                            nc.gpsimd.affine_select(
                                out=ptk[:, ss, h, off:off + P],
                                in_=ptk[:, ss, h, off:off + P],
                                compare_op=mybir.AluOpType.is_ge, fill=0.0,
                                base=0, pattern=[[1, P]],
                                channel_multiplier=-1)
                    for f in fillers.take(1):
                        f()
                for h in range(HP):
                    nc.tensor.matmul(
                        o_tiles[h][:], vaugs[b][:, jj, :, h, 0:DA],
                        ptk[:, :, h, :],
                        start=(jj == 0), stop=(jj == npairs - 1),
                        perf_mode=DR)
            for h in range(HP):
                nc.vector.tensor_copy(
                    otsbs[(b, h)][:, m * 512:(m + 1) * 512], o_tiles[h][:])

        # ---- minimal prelude: only what block (0,0) needs up front
        load_x(0)
        emit_qk_chunk(0, 0, 0)
        emit_qk_chunk(0, 1, 0)
        emit_v_stripe(0, 0)
        emit_v_stripe(0, 1)
        load_x(1)

        # b0/b1 blocks interleaved so the exp (ACT) stream never drains and
        # the PE always has S/PV + filler (V / Q/K projection) work. The
        # filler FIFO order matches each unit's first use in block order:
        # blocks b0m0(4 slots) b0m1(8) b0m2(12) b1m0(4) b0m3(16) b1m1(8)
        # b1m2(12) b1m3(16).
        fill = _Fillers()
        fill.add(lambda: emit_v_stripe(0, 2))
        fill.add(lambda: emit_v_stripe(0, 3))
        fill.add(lambda: emit_qk_chunk(0, 0, 1))
        fill.add(lambda: emit_qk_chunk(0, 1, 1))
        for s in range(4, 8):
            fill.add(lambda s=s: emit_v_stripe(0, s))
        for n in (2, 3):
            fill.add(lambda n=n: emit_qk_chunk(0, 0, n))
            fill.add(lambda n=n: emit_qk_chunk(0, 1, n))
        for s in range(8, 12):
            fill.add(lambda s=s: emit_v_stripe(0, s))
        fill.add(lambda: emit_qk_chunk(1, 0, 0))
        fill.add(lambda: emit_qk_chunk(1, 1, 0))
        for s in range(4):
            fill.add(lambda s=s: emit_v_stripe(1, s))
        fill.add(lambda: emit_qk_chunk(1, 0, 1))
        fill.add(lambda: emit_qk_chunk(1, 1, 1))
        for s in range(12, NST):
            fill.add(lambda s=s: emit_v_stripe(0, s))
        for n in (2, 3):
            fill.add(lambda n=n: emit_qk_chunk(1, 0, n))
            fill.add(lambda n=n: emit_qk_chunk(1, 1, n))
        for s in range(4, NST):
            fill.add(lambda s=s: emit_v_stripe(1, s))

        for b, m in ((0, 0), (0, 1), (0, 2), (1, 0),
                     (0, 3), (1, 1), (1, 2), (1, 3)):
            emit_spv_block(b, m, fill)
            if (b, m) == (0, 3):
                for h in range(HP):
                    nc.scalar.dma_start(out=ot_d[0, h], in_=otsbs[(0, h)][:])
        for f in fill.drain():
            f()
        for h in range(HP):
            nc.scalar.dma_start(out=ot_d[1, h], in_=otsbs[(1, h)][:])
    nc.compile()
    return nc


def build_phase1_v1(B, T, C, DH):
    HP = 2                      # heads per core
    DA = DH + 1                 # head dim + ones row (softmax denominator)
    NCC = C // P                # contraction chunks
    NT = T // P                 # key/value blocks of 128
    NQ = T // 512               # query chunks of 512
    NK = T // 1024              # query tiles of 1024
    scale = float(C) ** -0.5    # NOTE: reference scales by C**-0.5, not DH

    nc = bacc.Bacc("TRN2", debug=False)
    xT_d = nc.dram_tensor("xT", [B, C, T], BF, kind="ExternalInput").ap()
    wq_d = nc.dram_tensor("wq", [C, HP * DH], BF, kind="ExternalInput").ap()
    wk_d = nc.dram_tensor("wk", [C, HP * DH], BF, kind="ExternalInput").ap()
    wv_d = nc.dram_tensor("wv", [C, HP * DH], BF, kind="ExternalInput").ap()
    ot_d = nc.dram_tensor("ot", [B, HP, DA, T], FP, kind="ExternalOutput").ap()

    with tile.TileContext(nc) as tc, ExitStack() as ctx:
        const = ctx.enter_context(tc.tile_pool(name="const", bufs=1))
        xpool = ctx.enter_context(tc.tile_pool(name="xp", bufs=1))
        wpool = ctx.enter_context(tc.tile_pool(name="wp", bufs=1))
        qk_pool = ctx.enter_context(tc.tile_pool(name="qk", bufs=2))
        vt_pool = ctx.enter_context(tc.tile_pool(name="vtp", bufs=2))
        vaug_pool = ctx.enter_context(tc.tile_pool(name="vaug", bufs=2))
        pt_pool = ctx.enter_context(tc.tile_pool(name="pt", bufs=4))
        ot_pool = ctx.enter_context(tc.tile_pool(name="otp", bufs=2))

        # additive mask for the diagonal 128x128 block of S^T [s', q']:
        # 0 where q' >= s' (causal-valid), -1e30 where q' < s'
        negmask = const.tile([P, P], FP)
        nc.gpsimd.memset(negmask[:], 0.0)
        nc.gpsimd.affine_select(
            out=negmask[:], in_=negmask[:],
            compare_op=mybir.AluOpType.is_ge, fill=-1e30,
            base=0, pattern=[[1, P]], channel_multiplier=-1)
        ident = const.tile([P, P], BF)
        make_identity(nc, ident[:])
        ones_col = const.tile([P, NT * HP, 1], FP)
        nc.vector.memset(ones_col[:], 1.0)

        # weight chunks, loaded once
        wts = {}
        for name, src in (("q", wq_d), ("k", wk_d), ("v", wv_d)):
            wts[name] = []
            for c in range(NCC):
                t = wpool.tile([P, HP * DH], BF, tag=f"w{name}{c}")
                nc.sync.dma_start(out=t[:], in_=src[c * P:(c + 1) * P, :])
                wts[name].append(t)

        for b in range(B):
            xts = []
            for c in range(NCC):
                xt = xpool.tile([P, T], BF, tag=f"x{c}")
                nc.sync.dma_start(out=xt[:], in_=xT_d[b, c * P:(c + 1) * P, :])
                xts.append(xt)

            qt = qk_pool.tile([P, T], BF, tag="qt")
            kt = qk_pool.tile([P, T], BF, tag="kt")
            vaug = vaug_pool.tile([P, NT * HP, DA], BF, tag="vaug")
            # ones column per head-block (softmax denominator row of O^T)
            nc.vector.tensor_copy(vaug[:, :, DA - 1:DA], ones_col[:])

            with tc.tile_pool(name="proj_ps", bufs=3, space="PSUM") as proj_ps, \
                 tc.tile_pool(name="vt_ps", bufs=2, space="PSUM") as vt_ps:
                for wt, dst in ((wts["q"], qt), (wts["k"], kt)):
                    for n in range(NQ):
                        ps = proj_ps.tile([P, 512], FP, tag="proj")
                        for c in range(NCC):
                            nc.tensor.matmul(
                                ps[:], wt[c][:], xts[c][:, n * 512:(n + 1) * 512],
                                start=(c == 0), stop=(c == NCC - 1))
                        nc.vector.tensor_copy(dst[:, n * 512:(n + 1) * 512], ps[:])
                # V, then transpose into [s, d] layout with ones columns
                for n in range(NQ):
                    ps = proj_ps.tile([P, 512], FP, tag="proj")
                    for c in range(NCC):
                        nc.tensor.matmul(
                            ps[:], wts["v"][c][:], xts[c][:, n * 512:(n + 1) * 512],
                            start=(c == 0), stop=(c == NCC - 1))
                    vt = vt_pool.tile([P, 512], BF, tag="vt")
                    nc.vector.tensor_copy(vt[:], ps[:])
                    for u in range(4):
                        j = 4 * n + u
                        tp = vt_ps.tile([P, P], BF, tag="vtp")
                        nc.tensor.transpose(tp[:], vt[:, u * P:(u + 1) * P], ident[:])
                        nc.vector.tensor_copy(
                            vaug[:, j * HP, 0:DH], tp[:, 0:DH])
                        nc.vector.tensor_copy(
                            vaug[:, j * HP + 1, 0:DH], tp[:, DH:2 * DH])

            with tc.tile_pool(name="s_ps", bufs=2, space="PSUM") as s_ps, \
                 tc.tile_pool(name="o_ps", bufs=1, space="PSUM") as o_ps:
                ot_sbs = [ot_pool.tile([DA, T], FP, tag=f"ot{h}", name=f"ot{h}")
                          for h in range(HP)]
                for k in range(NK):
                    q_lo = 1024 * k
                    q_hi = 1024 * (k + 1)
                    o_tiles = [o_ps.tile([DA, 1024], FP, tag=f"o{h}", name=f"o{h}")
                               for h in range(HP)]
                    for j in range(8 * (k + 1)):
                        s0 = j * P
                        a0 = max(s0, q_lo)
                        # 512-grid chunks of the valid q range in this stripe
                        chunks = []
                        m0 = a0 // 512
                        for m in range(m0, q_hi // 512):
                            a = max(a0, m * 512)
                            e = (m + 1) * 512
                            chunks.append((a, e))
                        stl = [s_ps.tile([P, 1024], FP, tag="s", name="s")
                               for _ in range(HP)]
                        # emit head pairs adjacently: rows 0-63 (head A) and
                        # 64-127 (head B) run concurrently in the PE array
                        for (a, e) in chunks:
                            for h in range(HP):
                                hs = slice(h * DH, (h + 1) * DH)
                                nc.tensor.matmul(
                                    stl[h][:, a - q_lo:e - q_lo],
                                    kt[hs, s0:s0 + P], qt[hs, a:e],
                                    start=True, stop=True,
                                    tile_position=(h * DH, 0))
                        if q_lo <= s0:
                            for h in range(HP):
                                # diagonal block: additive causal mask
                                nc.vector.tensor_add(
                                    stl[h][:, s0 - q_lo:s0 - q_lo + P],
                                    stl[h][:, s0 - q_lo:s0 - q_lo + P],
                                    negmask[:])
                        for h in range(HP):
                            ptk = pt_pool.tile([P, 1024], BF, tag="pt")
                            nc.scalar.activation(
                                ptk[:, a0 - q_lo:1024], stl[h][:, a0 - q_lo:1024],
                                AF.Exp, scale=scale)
                            va = vaug[:, j * HP + h, :]
                            for (a, e) in chunks:
                                last_j = e // P - 1
                                nc.tensor.matmul(
                                    o_tiles[h][:, a - q_lo:e - q_lo],
                                    va, ptk[:, a - q_lo:e - q_lo],
                                    start=(j == 0), stop=(j == last_j))
                    for h in range(HP):
                        nc.vector.tensor_copy(
                            ot_sbs[h][:, q_lo:q_hi], o_tiles[h][:])
                for h in range(HP):
                    nc.sync.dma_start(out=ot_d[b, h], in_=ot_sbs[h][:])
    nc.compile()
    return nc


# --------------------------------------------------------------------------
# phase 2: per-core Wo projection + residual + rmsnorm + FFN + rmsnorm
# --------------------------------------------------------------------------

def build_phase2(NTOK, C, DFF):
    NTB = NTOK // P             # 4 token tiles of 128
    NCH = C // P                # 8 channel chunks
    NDF = DFF // P              # 32 dff chunks
    NG = DFF // 512             # 8 dff groups of 512

    NKC = C // 256              # DoubleRow contraction pairs for Wo
    DRM = mybir.MatmulPerfMode.DoubleRow
    nc = bacc.Bacc("TRN2", debug=False)
    xc_d = nc.dram_tensor("xc", [NTOK, C], FP, kind="ExternalInput").ap()
    at_d = nc.dram_tensor("attnT8", [NKC, P, 2, NTOK], PF8,
                          kind="ExternalInput").ap()
    wo_d = nc.dram_tensor("wo8", [NKC, P, 2, C], PF8,
                          kind="ExternalInput").ap()
    # host-permuted W1: [g, c, 128, 512] so each (g, c) chunk is contiguous
    w1_d = nc.dram_tensor("w1p", [NG, NCH, P, 512], BF, kind="ExternalInput").ap()
    w2_d = nc.dram_tensor("w2", [DFF, C], BF, kind="ExternalInput").ap()
    g1_d = nc.dram_tensor("g1", [C], FP, kind="ExternalInput").ap()
    g2_d = nc.dram_tensor("g2", [C], FP, kind="ExternalInput").ap()
    b1_d = nc.dram_tensor("b1", [DFF], FP, kind="ExternalInput").ap()
    b2_d = nc.dram_tensor("b2", [C], FP, kind="ExternalInput").ap()
    out_d = nc.dram_tensor("out", [NTOK, C], FP, kind="ExternalOutput").ap()

    def bcast_rows(src_ap, cols):
        # DRAM vector [cols] -> [P, cols] (same row in every partition)
        return bass.AP(tensor=src_ap.tensor, offset=src_ap.offset,
                       ap=[[0, P], [1, cols]])

    with tile.TileContext(nc) as tc, ExitStack() as ctx:
        const = ctx.enter_context(tc.tile_pool(name="const", bufs=1))
        work = ctx.enter_context(tc.tile_pool(name="work", bufs=2))
        stats = ctx.enter_context(tc.tile_pool(name="stats", bufs=4))
        h_pool = ctx.enter_context(tc.tile_pool(name="hp", bufs=1))
        hb2p = ctx.enter_context(tc.tile_pool(name="hb2p", bufs=1))
        ht_pool = ctx.enter_context(tc.tile_pool(name="htp", bufs=1))
        at_pool = ctx.enter_context(tc.tile_pool(name="atp", bufs=1))
        w2sb = ctx.enter_context(tc.tile_pool(name="w2sb", bufs=10))

        ident = const.tile([P, P], BF)
        make_identity(nc, ident[:])
        eps_t = const.tile([P, 1], FP)
        nc.vector.memset(eps_t[:], EPS)
        sc16 = const.tile([P, 1], FP)
        nc.vector.memset(sc16[:], 1.0 / 16.0)   # undo the x4*x4 Wo fp8 scales
        # preload the Sqrt/Square table set before stage-0 norms need it
        warm = stats.tile([P, 1], FP, tag="warm")
        nc.scalar.activation(warm[:], eps_t[:], AF.Sqrt, scale=1.0)
        # broadcasts go on the vector queue so the sync queue serves the
        # stage-0 operands (attnT/Wo/xc) first
        g1b = const.tile([P, C], FP)
        nc.scalar.dma_start(out=g1b[:], in_=bcast_rows(g1_d, C))
        g2b = const.tile([P, C], FP)
        nc.scalar.dma_start(out=g2b[:], in_=bcast_rows(g2_d, C))
        b2b = const.tile([P, C], FP)
        nc.scalar.dma_start(out=b2b[:], in_=bcast_rows(b2_d, C))
        b1b = const.tile([P, DFF], FP)
        nc.scalar.dma_start(out=b1b[:], in_=bcast_rows(b1_d, DFF))

        def rmsnorm_to(dst, src, gb):
            # dst = src * rsqrt(mean(src^2) + eps) * gb
            sq = work.tile([P, C], FP, tag="sq")
            ssum = stats.tile([P, 1], FP, tag="ssum")
            nc.scalar.activation(sq[:], src[:], AF.Square, accum_out=ssum[:])
            rstd = stats.tile([P, 1], FP, tag="rstd")
            nc.scalar.activation(rstd[:], ssum[:], AF.Sqrt,
                                 scale=1.0 / C, bias=eps_t[:])
            rinv = stats.tile([P, 1], FP, tag="rinv")
            nc.vector.reciprocal(rinv[:], rstd[:])
            nc.vector.scalar_tensor_tensor(
                dst[:], src[:], rinv[:], gb[:],
                op0=mybir.AluOpType.mult, op1=mybir.AluOpType.mult)

        # ---- stage 0: o = attnT^T @ Wo; r1 = x + o; h = rmsnorm(r1)*g1; hT
        hs = []
        hts_all = ht_pool.tile([P, NCH, NTOK], BF, tag="hts", name="hts_all")
        hts = [hts_all[:, c, :] for c in range(NCH)]
        with tc.tile_pool(name="o_ps", bufs=2, space="PSUM") as o_ps, \
             tc.tile_pool(name="t_ps", bufs=2, space="PSUM") as t_ps, \
             tc.tile_pool(name="wop", bufs=NCH) as wop, \
             tc.tile_pool(name="atsp", bufs=NCH) as atsp, \
             tc.tile_pool(name="xcp", bufs=1) as xcp, \
             tc.tile_pool(name="hbf", bufs=2) as hbf:
            atts, wots = [], []
            for c in range(NKC):
                att = atsp.tile([P, 2, NTOK], PF8, tag="at", name="at")
                nc.sync.dma_start(out=att[:], in_=at_d[c])
                wot = wop.tile([P, 2, C], PF8, tag="wo", name="wo")
                nc.sync.dma_start(out=wot[:], in_=wo_d[c])
                atts.append(att)
                wots.append(wot)
            xcs = []
            for tb in range(NTB):
                t = xcp.tile([P, C], FP, tag=f"xc{tb}")
                nc.sync.dma_start(out=t[:], in_=xc_d[tb * P:(tb + 1) * P, :])
                xcs.append(t)

            def wo_mms(tb, o):
                for c in range(NKC):
                    for half in range(2):
                        nc.tensor.matmul(
                            o[:, half * 512:(half + 1) * 512],
                            atts[c][:, :, tb * P:(tb + 1) * P],
                            wots[c][:, :, half * 512:(half + 1) * 512],
                            start=(c == 0), stop=(c == NKC - 1),
                            perf_mode=DRM)

            def norm_h(tb, o):
                r1 = work.tile([P, C], FP, tag="r1")
                nc.vector.scalar_tensor_tensor(
                    r1[:], o[:], sc16[:], xcs[tb][:],
                    op0=mybir.AluOpType.mult, op1=mybir.AluOpType.add)
                h = h_pool.tile([P, C], FP, tag=f"h{tb}")
                rmsnorm_to(h, r1, g1b)
                hs.append(h)
                hb = hbf.tile([P, C], BF, tag="hbf")
                nc.vector.tensor_copy(hb[:], h[:])
                return hb

            def trans_h(tb, hb):
                # 8 PE transposes into one PSUM tile, ONE batched DVE copy
                tp = t_ps.tile([P, NCH, P], BF, tag="tp")
                for c in range(NCH):
                    nc.tensor.transpose(
                        tp[:, c, :], hb[:, c * P:(c + 1) * P], ident[:])
                nc.vector.tensor_copy(
                    hts_all[:, :, tb * P:(tb + 1) * P], tp[:])

            # pipeline: Wo(tb) | Wo(tb+1) + trans(tb) | ... so the PE never
            # waits on the norm chain
            o_tiles, hbs = [], []
            for tb in range(NTB):
                o = o_ps.tile([P, C], FP, tag="o")
                wo_mms(tb, o)
                o_tiles.append(o)
                hbs.append(norm_h(tb, o))
                if tb >= 1:
                    trans_h(tb - 1, hbs[tb - 1])
            trans_h(NTB - 1, hbs[NTB - 1])

        # ---- stage 2: aT = silu(W1^T @ h^T + b1)  [dff, tok] bf16
        ats = []
        with tc.tile_pool(name="a_ps", bufs=8, space="PSUM") as a_ps, \
             tc.tile_pool(name="w1p", bufs=24) as w1p, \
             tc.tile_pool(name="sgp", bufs=3) as sgp:
            for g in range(NG):
                w1ts = []
                for c in range(NCH):
                    w1t = w1p.tile([P, 512], BF, tag="w1")
                    nc.gpsimd.dma_start(out=w1t[:], in_=w1_d[g, c])
                    w1ts.append(w1t)
                aps = [a_ps.tile([P, NTOK], FP, tag="a", name="a")
                       for _ in range(4)]
                for c in range(NCH):
                    for u in range(4):
                        nc.tensor.matmul(
                            aps[u][:], w1ts[c][:, u * P:(u + 1) * P],
                            hts[c],
                            start=(c == 0), stop=(c == NCH - 1))
                for u in range(4):
                    d = 4 * g + u
                    sg = sgp.tile([P, NTOK], FP, tag="sg")
                    nc.scalar.activation(sg[:], aps[u][:], AF.Sigmoid,
                                         bias=b1b[:, d:d + 1], scale=1.0)
                    at_t = at_pool.tile([P, NTOK], BF, tag=f"at{d}")
                    # silu(z) for z = a + b1: (a + b1) * sigmoid(a + b1)
                    nc.vector.scalar_tensor_tensor(
                        at_t[:], aps[u][:], b1b[:, d:d + 1], sg[:],
                        op0=mybir.AluOpType.add, op1=mybir.AluOpType.mult)
                    ats.append(at_t)

        # hb[tb] = h + b2, precomputed on DVE while the PE runs W2
        hb2s = []
        for tb in range(NTB):
            hb2 = hb2p.tile([P, C], FP, tag=f"hb2_{tb}")
            nc.vector.tensor_add(hb2[:], hs[tb][:], b2b[:])
            hb2s.append(hb2)

        # preload the Sqrt table set during W2 so the tail norm doesn't
        # pay the ~2.7us table switch
        dummy = stats.tile([P, 1], FP, tag="dummy")
        nc.scalar.activation(dummy[:], eps_t[:], AF.Sqrt, scale=1.0)

        # ---- stage 3: f = aT^T @ W2; r2 = h + b2 + f; out = rmsnorm(r2)*g2
        with tc.tile_pool(name="f_ps", bufs=1, space="PSUM") as f_ps:
            fts = [f_ps.tile([P, C], FP, tag=f"f{tb}", name=f"f{tb}")
                   for tb in range(NTB)]
            for d in range(NDF):
                w2t = w2sb.tile([P, C], BF, tag="w2")
                nc.gpsimd.dma_start(out=w2t[:], in_=w2_d[d * P:(d + 1) * P, :])
                for tb in range(NTB):
                    for half in range(2):
                        nc.tensor.matmul(
                            fts[tb][:, half * 512:(half + 1) * 512],
                            ats[d][:, tb * P:(tb + 1) * P],
                            w2t[:, half * 512:(half + 1) * 512],
                            start=(d == 0), stop=(d == NDF - 1))
            for tb in range(NTB):
                r2 = work.tile([P, C], FP, tag="r2")
                nc.vector.tensor_add(r2[:], fts[tb][:], hb2s[tb][:])
                o = work.tile([P, C], FP, tag="outt")
                rmsnorm_to(o, r2, g2b)
                eng = nc.scalar if tb % 2 == 0 else nc.sync
                eng.dma_start(out=out_d[tb * P:(tb + 1) * P, :], in_=o[:])
    nc.compile()
    return nc


# --------------------------------------------------------------------------
# host orchestration
# --------------------------------------------------------------------------

_CACHE = {}


def _phase1(B, T, C, DH):
    key = ("p1", B, T, C, DH)
    if key not in _CACHE:
        _CACHE[key] = build_phase1(B, T, C, DH)
    return _CACHE[key]


def _phase2(NTOK, C, DFF):
    key = ("p2", NTOK, C, DFF)
    if key not in _CACHE:
        _CACHE[key] = build_phase2(NTOK, C, DFF)
    return _CACHE[key]


def _run(nc, in_maps, ldw_opt=False):
    import os
    import concourse.bass_utils as _bu
    _bu._ldw_opt_enable = ldw_opt    # DR ldweights are incompatible with it
    trace = bool(os.environ.get("KERNEL_TRACE"))
    kwargs = {}
    if trace:
        _install_ntff_hook_shim()
        tdir = os.environ.get("KERNEL_TRACE_DIR")
        if tdir:
            phase_dir = os.path.join(tdir, f"phase{len(LAST_EXEC_NS)}")
            os.makedirs(phase_dir, exist_ok=True)
            kwargs["tmpdir"] = phase_dir
    res = run_bass_kernel_spmd(nc, in_maps, core_ids=list(range(N_CORES)),
                               trace=trace, **kwargs)
    LAST_EXEC_NS.append(res.exec_time_ns)
    return res.results


def kernel(x, Wq, Wk, Wv, Wo, bo, W1, b1, W2, b2, g1, g2):
    f32 = lambda a: np.ascontiguousarray(np.asarray(a), dtype=np.float32)
    x = f32(x)
    Wq, Wk, Wv, Wo, bo = f32(Wq), f32(Wk), f32(Wv), f32(Wo), f32(bo)
    W1, b1, W2, b2, g1, g2 = f32(W1), f32(b1), f32(W2), f32(b2), f32(g1), f32(g2)

    B, T, C = x.shape
    H, _, DH = Wq.shape
    HP = H // N_CORES           # heads per core (2)
    DA = DH + 1
    LAST_EXEC_NS.clear()

    # ---- phase 1
    nc1 = _phase1(B, T, C, DH)
    NKK = C // 256
    # [B,T,C] -> [B, n, kk, 128, 2, 512] with channel c = (2*kk+s)*128 + p
    # and token t = n*512 + t'
    xT8 = np.ascontiguousarray(
        x.transpose(0, 2, 1).reshape(B, NKK, 2, 128, T // 512, 512)
        .transpose(0, 4, 1, 3, 2, 5)).astype(F8_NP)
    in1 = []
    for i in range(N_CORES):
        pq = Wq[HP * i:HP * (i + 1)].transpose(1, 0, 2).reshape(C, HP * DH)
        pk = Wk[HP * i:HP * (i + 1)].transpose(1, 0, 2).reshape(C, HP * DH)
        pv = Wv[HP * i:HP * (i + 1)].transpose(1, 0, 2).reshape(C, HP * DH)
        pair = lambda w: np.ascontiguousarray(
            (w * WS).reshape(NKK, 2, 128, HP * DH)
            .transpose(0, 2, 1, 3)).astype(F8_NP)
        in1.append({"xT8": xT8,
                    "wq8": pair(pq),
                    "wk8": pair(pk),
                    "wv8": np.ascontiguousarray(
                        (pv * WS).reshape(2 * NKK, 128, HP * DH)
                    ).astype(F8_NP)})
    res1 = _run(nc1, in1)

    attn = np.empty((B, T, C), np.float32)
    for i in range(N_CORES):
        ot = res1[i]["ot"]                    # [B, HP, DA, T]
        o = ot[:, :, :DH, :]
        den = ot[:, :, DH, :] * WS            # V carries a x32 scale
        on = o / den[:, :, None, :]
        for hh in range(HP):
            hcol = (HP * i + hh) * DH
            attn[:, :, hcol:hcol + DH] = on[:, hh].transpose(0, 2, 1)

    # ---- phase 2
    NTOK = B * T // N_CORES
    nc2 = _phase2(NTOK, C, W1.shape[1])
    xf = x.reshape(B * T, C) + bo             # fold bo into the residual
    af = attn.reshape(B * T, C)
    DFF = W1.shape[1]
    NKC = C // 256
    # Wo in fp8 DoubleRow pairs, x4 scale on both operands (undone by 1/16)
    wo8 = np.ascontiguousarray(
        (Wo * 4.0).reshape(NKC, 2, 128, C).transpose(0, 2, 1, 3)).astype(F8_NP)
    # [C, DFF] -> [g, c, 128, 512] so each (g, c) chunk is DMA-contiguous
    w1_bf = np.ascontiguousarray(
        W1.reshape(C // 128, 128, DFF // 512, 512)
        .transpose(2, 0, 1, 3)).astype(BF_NP)
    w2_bf = W2.astype(BF_NP)
    in2 = []
    for k in range(N_CORES):
        sl = slice(k * NTOK, (k + 1) * NTOK)
        at8 = np.ascontiguousarray(
            (af[sl].T * 4.0).reshape(NKC, 2, 128, NTOK)
            .transpose(0, 2, 1, 3)).astype(F8_NP)
        in2.append({
            "xc": np.ascontiguousarray(xf[sl]),
            "attnT8": at8,
            "wo8": wo8, "w1p": w1_bf, "w2": w2_bf,
            "g1": g1, "g2": g2, "b1": b1, "b2": b2,
        })
    res2 = _run(nc2, in2)
    out = np.concatenate([res2[k]["out"] for k in range(N_CORES)], axis=0)
    return out.reshape(B, T, C)



# revision 32
# speedup vs baseline: 1.4492x; 1.0230x over previous
"""Trainium2 Bass kernel for a dense transformer block, distributed over 8
NeuronCores.

Sharding:
  phase 1 (attention): tensor-parallel over heads — each core computes 2 of
    the 16 heads end-to-end (QKV projections + causal softmax(QK^T)V), and
    returns the unnormalized per-head output O^T together with the softmax
    denominators (obtained via a ones-column appended to V).
  phase 2 (Wo + norms + FFN): data-parallel over tokens — each core handles
    512 of the 4096 token rows with replicated weights.

The host glues the phases: transposes x, normalizes/concats heads, and
re-shards tokens.  All matmuls run as float32r (full-rate fp32 PE mode).
"""

import math
from contextlib import ExitStack

import ml_dtypes
import numpy as np

BF_NP = ml_dtypes.bfloat16

import concourse.bass as bass
import concourse.mybir as mybir
import concourse.tile as tile
from concourse import bacc
from concourse.bass_utils import run_bass_kernel_spmd
from concourse.masks import make_identity, make_upper_triangular

FP = mybir.dt.float32
FPR = mybir.dt.float32r
BF = mybir.dt.bfloat16
AF = mybir.ActivationFunctionType

N_CORES = 8
P = 128
EPS = 1e-6

# exec times (ns) of the most recent kernel() call, one entry per phase, when
# tracing was enabled via BASS_TRACE=1; None entries otherwise.
LAST_EXEC_NS = []


def _install_ntff_hook_shim():
    """Provide antenv.axon_hooks when the image lacks it, so trace=True can
    drive NTFF profiling through libaxon_pjrt's C ABI (same contract as
    trn_boot's step-6 hook). No-op if the real module exists or the .so is
    missing/old."""
    try:
        import antenv.axon_hooks  # noqa: F401
        return
    except ImportError:
        pass
    import contextlib
    import ctypes
    import sys
    import types

    try:
        lib = ctypes.CDLL("/opt/axon/libaxon_pjrt.so")
    except OSError:
        return
    if not hasattr(lib, "axon_start_nrt_profile"):
        return
    lib.axon_start_nrt_profile.argtypes = [
        ctypes.POINTER(ctypes.c_int64), ctypes.c_size_t]
    lib.axon_start_nrt_profile.restype = ctypes.c_int64
    lib.axon_stop_nrt_profile.argtypes = [ctypes.c_char_p]
    lib.axon_stop_nrt_profile.restype = ctypes.c_int64

    @contextlib.contextmanager
    def _hook(output_dir, device_ids):
        import jax
        jax.devices()
        if device_ids:
            ids = (ctypes.c_int64 * len(device_ids))(*device_ids)
            rc = lib.axon_start_nrt_profile(ids, len(device_ids))
        else:
            rc = lib.axon_start_nrt_profile(None, 0)
        if rc != 0:
            raise RuntimeError(f"axon_start_nrt_profile rc={rc}")
        try:
            yield
        finally:
            n = lib.axon_stop_nrt_profile(str(output_dir).encode())
            if n < 0:
                raise RuntimeError(f"axon_stop_nrt_profile rc={n}")

    mod = types.ModuleType("antenv.axon_hooks")
    mod.get_axon_ntff_profile_hook = lambda: _hook

    def set_axon_ntff_profile_hook(h):
        mod.get_axon_ntff_profile_hook = lambda: h

    mod.set_axon_ntff_profile_hook = set_axon_ntff_profile_hook
    import antenv
    antenv.axon_hooks = mod
    sys.modules["antenv.axon_hooks"] = mod


def _fpr(ap):
    return ap.bitcast(FPR)


def _enable_ldw_opt():
    """Flip walrus's --enable-ldw-opt to true so LDWEIGHTS overlaps with
    in-flight matmuls (background weight buffer). Verified by the rel-err
    check; idempotent."""
    import concourse.bass_utils as _bu
    if getattr(_bu, "_ldw_opt_patched", False):
        return
    orig = _bu.run_command

    def patched(cmd, *a, **kw):
        if isinstance(cmd, list) and getattr(_bu, "_ldw_opt_enable", False):
            cmd = ["--enable-ldw-opt=true" if c == "--enable-ldw-opt=false"
                   else c for c in cmd]
        return orig(cmd, *a, **kw)

    _bu.run_command = patched
    _bu._ldw_opt_patched = True


_enable_ldw_opt()


# --------------------------------------------------------------------------
# phase 1: per-core attention over a pair of heads (fp8 + DoubleRow)
# --------------------------------------------------------------------------

PF8 = mybir.dt.float8e4
F8_NP = ml_dtypes.float8_e4m3
WS = 32.0          # q/k/v weight upscale; folded out via exp scale + host div


class _Fillers:
    """FIFO of deferred PE-work emitters, drained one unit at a time into
    the exp-gated attention loop so the tensor engine never idles."""

    def __init__(self):
        self.q = []

    def add(self, fn):
        self.q.append(fn)

    def take(self, n):
        out, self.q = self.q[:n], self.q[n:]
        return out

    def drain(self):
        out, self.q = self.q, []
        return out


def build_phase1(B, T, C, DH):
    HP = 2                      # heads per core
    DA = DH + 1                 # head dim + ones row (softmax denominator)
    DAP = 80                    # DA padded so the DoubleRow pair stride is 16B-aligned
    NKK = C // 256              # DoubleRow contraction pairs
    NST = T // P                # key stripes of 128
    NM = T // 512               # query blocks of 512
    scale = float(C) ** -0.5 / (WS * WS)
    DR = mybir.MatmulPerfMode.DoubleRow

    nc = bacc.Bacc("TRN2", debug=False)
    # token-sliced so each (b, n, kk) chunk is contiguous and the first
    # projection can start after one 128KB DMA
    xT_d = nc.dram_tensor("xT8", [B, NM, NKK, P, 2, 512], PF8,
                          kind="ExternalInput").ap()
    wq_d = nc.dram_tensor("wq8", [NKK, P, 2, HP * DH], PF8,
                          kind="ExternalInput").ap()
    wk_d = nc.dram_tensor("wk8", [NKK, P, 2, HP * DH], PF8,
                          kind="ExternalInput").ap()
    wv_d = nc.dram_tensor("wv8", [2 * NKK, P, HP * DH], PF8,
                          kind="ExternalInput").ap()
    ot_d = nc.dram_tensor("ot", [B, HP, DA, T], FP, kind="ExternalOutput").ap()

    with tile.TileContext(nc) as tc, ExitStack() as ctx:
        const = ctx.enter_context(tc.tile_pool(name="const", bufs=1))
        xp = ctx.enter_context(tc.tile_pool(name="xp", bufs=1))
        wp = ctx.enter_context(tc.tile_pool(name="wp", bufs=1))
        qkp = ctx.enter_context(tc.tile_pool(name="qkp", bufs=1))
        vap = ctx.enter_context(tc.tile_pool(name="vap", bufs=1))
        ptkp = ctx.enter_context(tc.tile_pool(name="ptkp", bufs=6))
        otp = ctx.enter_context(tc.tile_pool(name="otp", bufs=1))
        pp = ctx.enter_context(tc.tile_pool(name="pp", bufs=2, space="PSUM"))
        sp = ctx.enter_context(tc.tile_pool(name="sp", bufs=2, space="PSUM"))
        op = ctx.enter_context(tc.tile_pool(name="op", bufs=1, space="PSUM"))

        # preload the Exp table set before the attention loop needs it
        warm = const.tile([P, 1], FP)
        nc.vector.memset(warm[:], 0.0)
        nc.scalar.activation(warm[:], warm[:], AF.Exp, scale=1.0)

        wqs, wks, wvs = [], [], []
        for kk in range(NKK):
            wq = wp.tile([P, 2, HP * DH], PF8, tag=f"wq{kk}")
            nc.sync.dma_start(out=wq[:], in_=wq_d[kk])
            wk = wp.tile([P, 2, HP * DH], PF8, tag=f"wk{kk}")
            nc.sync.dma_start(out=wk[:], in_=wk_d[kk])
            wqs.append(wq)
            wks.append(wk)
        for c in range(2 * NKK):
            wv = wp.tile([P, HP * DH], PF8, tag=f"wv{c}")
            nc.sync.dma_start(out=wv[:], in_=wv_d[c])
            wvs.append(wv)

        xts, qts, kts, vaugs, otsbs = {}, {}, {}, {}, {}

        def load_x(b, ns):
            # one DMA per (b, n) covering all kk chunks; rhs/lhsT slices of
            # the combined tile keep the DoubleRow pair layout
            for n in ns:
                t = xp.tile([P, NKK, 2, 512], PF8, tag=f"x{b}_{n}",
                            name=f"x{b}_{n}")
                src_ap = bass.AP(
                    tensor=xT_d.tensor,
                    offset=xT_d.offset + (b * NM + n) * NKK * P * 1024,
                    ap=[[1024, P], [P * 1024, NKK], [512, 2], [1, 512]])
                nc.sync.dma_start(out=t[:], in_=src_ap)
                for kk in range(NKK):
                    xts[(b, n, kk)] = t[:, kk, :, :]

        for b in range(B):
            qts[b] = qkp.tile([P, T], PF8, tag=f"qt{b}", name=f"qt{b}")
            kts[b] = qkp.tile([P, T], PF8, tag=f"kt{b}", name=f"kt{b}")
            va = vap.tile([P, NST // 2, 2, HP, DAP], PF8, tag=f"va{b}",
                          name=f"va{b}")
            nc.vector.memset(va[:], 1.0)    # ones column survives in col DH
            vaugs[b] = va
            for h in range(HP):
                otsbs[(b, h)] = otp.tile([DA, T], FP, tag=f"ot{b}{h}", name=f"ot{b}{h}")

        def emit_qk_chunk(b, proj, n):
            # one 512-token chunk of the Q or K projection (NKK DR matmuls)
            wt = wqs if proj == 0 else wks
            dst = qts[b] if proj == 0 else kts[b]
            ps = pp.tile([P, 512], FP, tag="pp")
            for kk in range(NKK):
                nc.tensor.matmul(
                    ps[:], wt[kk][:],
                    xts[(b, n, kk)],
                    start=(kk == 0), stop=(kk == NKK - 1), perf_mode=DR)
            nc.vector.tensor_copy(dst[:, n * 512:(n + 1) * 512], ps[:])

        def emit_v_stripe(b, s):
            # V^T stripe s directly in [token, head-dim] layout
            n, so = s // 4, (s % 4) * P
            ps = pp.tile([P, 512], FP, tag="pp")
            out = ps[:, 0:HP * DH]
            for c in range(2 * NKK):
                nc.tensor.matmul(
                    out, xts[(b, n, c // 2)][:, c % 2, so:so + P],
                    wvs[c][:],
                    start=(c == 0), stop=(c == 2 * NKK - 1))

            for h in range(HP):
                nc.vector.tensor_copy(
                    vaugs[b][:, s // 2, s % 2, h, 0:DH],
                    ps[:, h * DH:(h + 1) * DH])

        def emit_spv_block(b, m, fillers):
            o_tiles = [op.tile([DA, 512], FP, tag=f"o{h}", name=f"o{h}")
                       for h in range(HP)]
            npairs = 2 * (m + 1)
            for jj in range(npairs):
                ptk = ptkp.tile([P, 2, HP, 512], PF8, tag="ptk")
                for ss in range(2):
                    j = 2 * jj + ss
                    s0 = j * P
                    diag = s0 >= m * 512
                    off = s0 - m * 512 if diag else 0
                    st = sp.tile([P, HP, 512], FP, tag="st")
                    for h in range(HP):
                        hs = slice(h * DH, (h + 1) * DH)
                        nc.tensor.matmul(
                            st[:, h, off:512],
                            kts[b][hs, s0:s0 + P],
                            qts[b][hs, m * 512 + off:(m + 1) * 512],
                            start=True, stop=True,
                            tile_position=(h * DH, 0))
                    if diag and off > 0:
                        nc.gpsimd.memset(ptk[:, ss, :, 0:off], 0.0)
                    nc.scalar.activation(
                        ptk[:, ss, :, off:512], st[:, :, off:512],
                        AF.Exp, scale=scale)
                    if diag:
                        # zero the non-causal triangle of the diagonal 128
                        # cols after the exp, off the ACT critical path
                        for h in range(HP):
                            nc.gpsimd.affine_select(
                                out=ptk[:, ss, h, off:off + P],
                                in_=ptk[:, ss, h, off:off + P],
                                compare_op=mybir.AluOpType.is_ge, fill=0.0,
                                base=0, pattern=[[1, P]],
                                channel_multiplier=-1)
                    for f in fillers.take(1):
                        f()
                for h in range(HP):
                    nc.tensor.matmul(
                        o_tiles[h][:], vaugs[b][:, jj, :, h, 0:DA],
                        ptk[:, :, h, :],
                        start=(jj == 0), stop=(jj == npairs - 1),
                        perf_mode=DR)
            for h in range(HP):
                nc.vector.tensor_copy(
                    otsbs[(b, h)][:, m * 512:(m + 1) * 512], o_tiles[h][:])

        # ---- minimal prelude: only what block (0,0) needs up front
        load_x(0, [0])
        emit_qk_chunk(0, 0, 0)
        emit_qk_chunk(0, 1, 0)
        emit_v_stripe(0, 0)
        emit_v_stripe(0, 1)
        load_x(0, [1, 2, 3])
        load_x(1, range(NM))

        # b0/b1 blocks interleaved so the exp (ACT) stream never drains and
        # the PE always has S/PV + filler (V / Q/K projection) work. The
        # filler FIFO order matches each unit's first use in block order:
        # blocks b0m0(4 slots) b0m1(8) b0m2(12) b1m0(4) b0m3(16) b1m1(8)
        # b1m2(12) b1m3(16).
        fill = _Fillers()
        fill.add(lambda: emit_v_stripe(0, 2))
        fill.add(lambda: emit_v_stripe(0, 3))
        fill.add(lambda: emit_qk_chunk(0, 0, 1))
        fill.add(lambda: emit_qk_chunk(0, 1, 1))
        for s in range(4, 8):
            fill.add(lambda s=s: emit_v_stripe(0, s))
        for n in (2, 3):
            fill.add(lambda n=n: emit_qk_chunk(0, 0, n))
            fill.add(lambda n=n: emit_qk_chunk(0, 1, n))
        for s in range(8, 12):
            fill.add(lambda s=s: emit_v_stripe(0, s))
        fill.add(lambda: emit_qk_chunk(1, 0, 0))
        fill.add(lambda: emit_qk_chunk(1, 1, 0))
        for s in range(4):
            fill.add(lambda s=s: emit_v_stripe(1, s))
        fill.add(lambda: emit_qk_chunk(1, 0, 1))
        fill.add(lambda: emit_qk_chunk(1, 1, 1))
        for s in range(12, NST):
            fill.add(lambda s=s: emit_v_stripe(0, s))
        for n in (2, 3):
            fill.add(lambda n=n: emit_qk_chunk(1, 0, n))
            fill.add(lambda n=n: emit_qk_chunk(1, 1, n))
        for s in range(4, NST):
            fill.add(lambda s=s: emit_v_stripe(1, s))

        for b, m in ((0, 0), (0, 1), (0, 2), (1, 0),
                     (0, 3), (1, 1), (1, 2), (1, 3)):
            emit_spv_block(b, m, fill)
            for h in range(HP):
                nc.gpsimd.dma_start(
                    out=ot_d[b, h, :, m * 512:(m + 1) * 512],
                    in_=otsbs[(b, h)][:, m * 512:(m + 1) * 512])
        for f in fill.drain():
            f()
    nc.compile()
    return nc


def build_phase1_v1(B, T, C, DH):
    HP = 2                      # heads per core
    DA = DH + 1                 # head dim + ones row (softmax denominator)
    NCC = C // P                # contraction chunks
    NT = T // P                 # key/value blocks of 128
    NQ = T // 512               # query chunks of 512
    NK = T // 1024              # query tiles of 1024
    scale = float(C) ** -0.5    # NOTE: reference scales by C**-0.5, not DH

    nc = bacc.Bacc("TRN2", debug=False)
    xT_d = nc.dram_tensor("xT", [B, C, T], BF, kind="ExternalInput").ap()
    wq_d = nc.dram_tensor("wq", [C, HP * DH], BF, kind="ExternalInput").ap()
    wk_d = nc.dram_tensor("wk", [C, HP * DH], BF, kind="ExternalInput").ap()
    wv_d = nc.dram_tensor("wv", [C, HP * DH], BF, kind="ExternalInput").ap()
    ot_d = nc.dram_tensor("ot", [B, HP, DA, T], FP, kind="ExternalOutput").ap()

    with tile.TileContext(nc) as tc, ExitStack() as ctx:
        const = ctx.enter_context(tc.tile_pool(name="const", bufs=1))
        xpool = ctx.enter_context(tc.tile_pool(name="xp", bufs=1))
        wpool = ctx.enter_context(tc.tile_pool(name="wp", bufs=1))
        qk_pool = ctx.enter_context(tc.tile_pool(name="qk", bufs=2))
        vt_pool = ctx.enter_context(tc.tile_pool(name="vtp", bufs=2))
        vaug_pool = ctx.enter_context(tc.tile_pool(name="vaug", bufs=2))
        pt_pool = ctx.enter_context(tc.tile_pool(name="pt", bufs=4))
        ot_pool = ctx.enter_context(tc.tile_pool(name="otp", bufs=2))

        # additive mask for the diagonal 128x128 block of S^T [s', q']:
        # 0 where q' >= s' (causal-valid), -1e30 where q' < s'
        negmask = const.tile([P, P], FP)
        nc.gpsimd.memset(negmask[:], 0.0)
        nc.gpsimd.affine_select(
            out=negmask[:], in_=negmask[:],
            compare_op=mybir.AluOpType.is_ge, fill=-1e30,
            base=0, pattern=[[1, P]], channel_multiplier=-1)
        ident = const.tile([P, P], BF)
        make_identity(nc, ident[:])
        ones_col = const.tile([P, NT * HP, 1], FP)
        nc.vector.memset(ones_col[:], 1.0)

        # weight chunks, loaded once
        wts = {}
        for name, src in (("q", wq_d), ("k", wk_d), ("v", wv_d)):
            wts[name] = []
            for c in range(NCC):
                t = wpool.tile([P, HP * DH], BF, tag=f"w{name}{c}")
                nc.sync.dma_start(out=t[:], in_=src[c * P:(c + 1) * P, :])
                wts[name].append(t)

        for b in range(B):
            xts = []
            for c in range(NCC):
                xt = xpool.tile([P, T], BF, tag=f"x{c}")
                nc.sync.dma_start(out=xt[:], in_=xT_d[b, c * P:(c + 1) * P, :])
                xts.append(xt)

            qt = qk_pool.tile([P, T], BF, tag="qt")
            kt = qk_pool.tile([P, T], BF, tag="kt")
            vaug = vaug_pool.tile([P, NT * HP, DA], BF, tag="vaug")
            # ones column per head-block (softmax denominator row of O^T)
            nc.vector.tensor_copy(vaug[:, :, DA - 1:DA], ones_col[:])

            with tc.tile_pool(name="proj_ps", bufs=3, space="PSUM") as proj_ps, \
                 tc.tile_pool(name="vt_ps", bufs=2, space="PSUM") as vt_ps:
                for wt, dst in ((wts["q"], qt), (wts["k"], kt)):
                    for n in range(NQ):
                        ps = proj_ps.tile([P, 512], FP, tag="proj")
                        for c in range(NCC):
                            nc.tensor.matmul(
                                ps[:], wt[c][:], xts[c][:, n * 512:(n + 1) * 512],
                                start=(c == 0), stop=(c == NCC - 1))
                        nc.vector.tensor_copy(dst[:, n * 512:(n + 1) * 512], ps[:])
                # V, then transpose into [s, d] layout with ones columns
                for n in range(NQ):
                    ps = proj_ps.tile([P, 512], FP, tag="proj")
                    for c in range(NCC):
                        nc.tensor.matmul(
                            ps[:], wts["v"][c][:], xts[c][:, n * 512:(n + 1) * 512],
                            start=(c == 0), stop=(c == NCC - 1))
                    vt = vt_pool.tile([P, 512], BF, tag="vt")
                    nc.vector.tensor_copy(vt[:], ps[:])
                    for u in range(4):
                        j = 4 * n + u
                        tp = vt_ps.tile([P, P], BF, tag="vtp")
                        nc.tensor.transpose(tp[:], vt[:, u * P:(u + 1) * P], ident[:])
                        nc.vector.tensor_copy(
                            vaug[:, j * HP, 0:DH], tp[:, 0:DH])
                        nc.vector.tensor_copy(
                            vaug[:, j * HP + 1, 0:DH], tp[:, DH:2 * DH])

            with tc.tile_pool(name="s_ps", bufs=2, space="PSUM") as s_ps, \
                 tc.tile_pool(name="o_ps", bufs=1, space="PSUM") as o_ps:
                ot_sbs = [ot_pool.tile([DA, T], FP, tag=f"ot{h}", name=f"ot{h}")
                          for h in range(HP)]
                for k in range(NK):
                    q_lo = 1024 * k
                    q_hi = 1024 * (k + 1)
                    o_tiles = [o_ps.tile([DA, 1024], FP, tag=f"o{h}", name=f"o{h}")
                               for h in range(HP)]
                    for j in range(8 * (k + 1)):
                        s0 = j * P
                        a0 = max(s0, q_lo)
                        # 512-grid chunks of the valid q range in this stripe
                        chunks = []
                        m0 = a0 // 512
                        for m in range(m0, q_hi // 512):
                            a = max(a0, m * 512)
                            e = (m + 1) * 512
                            chunks.append((a, e))
                        stl = [s_ps.tile([P, 1024], FP, tag="s", name="s")
                               for _ in range(HP)]
                        # emit head pairs adjacently: rows 0-63 (head A) and
                        # 64-127 (head B) run concurrently in the PE array
                        for (a, e) in chunks:
                            for h in range(HP):
                                hs = slice(h * DH, (h + 1) * DH)
                                nc.tensor.matmul(
                                    stl[h][:, a - q_lo:e - q_lo],
                                    kt[hs, s0:s0 + P], qt[hs, a:e],
                                    start=True, stop=True,
                                    tile_position=(h * DH, 0))
                        if q_lo <= s0:
                            for h in range(HP):
                                # diagonal block: additive causal mask
                                nc.vector.tensor_add(
                                    stl[h][:, s0 - q_lo:s0 - q_lo + P],
                                    stl[h][:, s0 - q_lo:s0 - q_lo + P],
                                    negmask[:])
                        for h in range(HP):
                            ptk = pt_pool.tile([P, 1024], BF, tag="pt")
                            nc.scalar.activation(
                                ptk[:, a0 - q_lo:1024], stl[h][:, a0 - q_lo:1024],
                                AF.Exp, scale=scale)
                            va = vaug[:, j * HP + h, :]
                            for (a, e) in chunks:
                                last_j = e // P - 1
                                nc.tensor.matmul(
                                    o_tiles[h][:, a - q_lo:e - q_lo],
                                    va, ptk[:, a - q_lo:e - q_lo],
                                    start=(j == 0), stop=(j == last_j))
                    for h in range(HP):
                        nc.vector.tensor_copy(
                            ot_sbs[h][:, q_lo:q_hi], o_tiles[h][:])
                for h in range(HP):
                    nc.sync.dma_start(out=ot_d[b, h], in_=ot_sbs[h][:])
    nc.compile()
    return nc


# --------------------------------------------------------------------------
# phase 2: per-core Wo projection + residual + rmsnorm + FFN + rmsnorm
# --------------------------------------------------------------------------

def build_phase2(NTOK, C, DFF):
    NTB = NTOK // P             # 4 token tiles of 128
    NCH = C // P                # 8 channel chunks
    NDF = DFF // P              # 32 dff chunks
    NG = DFF // 512             # 8 dff groups of 512

    NKC = C // 256              # DoubleRow contraction pairs for Wo
    DRM = mybir.MatmulPerfMode.DoubleRow
    nc = bacc.Bacc("TRN2", debug=False)
    xc_d = nc.dram_tensor("xc", [NTOK, C], FP, kind="ExternalInput").ap()
    at_d = nc.dram_tensor("attnT8", [NKC, P, 2, NTOK], PF8,
                          kind="ExternalInput").ap()
    wo_d = nc.dram_tensor("wo8", [NKC, P, 2, C], PF8,
                          kind="ExternalInput").ap()
    # host-permuted W1: [g, c, 128, 512] so each (g, c) chunk is contiguous
    w1_d = nc.dram_tensor("w1p", [NG, NCH, P, 512], BF, kind="ExternalInput").ap()
    w2_d = nc.dram_tensor("w2", [DFF, C], BF, kind="ExternalInput").ap()
    g1_d = nc.dram_tensor("g1", [C], FP, kind="ExternalInput").ap()
    g2_d = nc.dram_tensor("g2", [C], FP, kind="ExternalInput").ap()
    b1_d = nc.dram_tensor("b1", [DFF], FP, kind="ExternalInput").ap()
    b2_d = nc.dram_tensor("b2", [C], FP, kind="ExternalInput").ap()
    out_d = nc.dram_tensor("out", [NTOK, C], FP, kind="ExternalOutput").ap()

    def bcast_rows(src_ap, cols):
        # DRAM vector [cols] -> [P, cols] (same row in every partition)
        return bass.AP(tensor=src_ap.tensor, offset=src_ap.offset,
                       ap=[[0, P], [1, cols]])

    with tile.TileContext(nc) as tc, ExitStack() as ctx:
        const = ctx.enter_context(tc.tile_pool(name="const", bufs=1))
        work = ctx.enter_context(tc.tile_pool(name="work", bufs=2))
        stats = ctx.enter_context(tc.tile_pool(name="stats", bufs=4))
        h_pool = ctx.enter_context(tc.tile_pool(name="hp", bufs=1))
        hb2p = ctx.enter_context(tc.tile_pool(name="hb2p", bufs=1))
        ht_pool = ctx.enter_context(tc.tile_pool(name="htp", bufs=1))
        at_pool = ctx.enter_context(tc.tile_pool(name="atp", bufs=1))
        w2sb = ctx.enter_context(tc.tile_pool(name="w2sb", bufs=10))

        ident = const.tile([P, P], BF)
        make_identity(nc, ident[:])
        eps_t = const.tile([P, 1], FP)
        nc.vector.memset(eps_t[:], EPS)
        sc16 = const.tile([P, 1], FP)
        nc.vector.memset(sc16[:], 1.0 / 16.0)   # undo the x4*x4 Wo fp8 scales
        # preload the Sqrt/Square table set before stage-0 norms need it
        warm = stats.tile([P, 1], FP, tag="warm")
        nc.scalar.activation(warm[:], eps_t[:], AF.Sqrt, scale=1.0)
        # g1 is needed first (stage-0 norms); the other broadcasts are
        # issued later, near their first consumer, to keep early HBM
        # bandwidth for the stage-0 operands
        g1b = const.tile([P, C], FP)
        nc.scalar.dma_start(out=g1b[:], in_=bcast_rows(g1_d, C))
        g2b = const.tile([P, C], FP)
        b2b = const.tile([P, C], FP)
        b1b = const.tile([P, DFF], FP)

        def rmsnorm_to(dst, src, gb):
            # dst = src * rsqrt(mean(src^2) + eps) * gb
            sq = work.tile([P, C], FP, tag="sq")
            ssum = stats.tile([P, 1], FP, tag="ssum")
            nc.scalar.activation(sq[:], src[:], AF.Square, accum_out=ssum[:])
            rstd = stats.tile([P, 1], FP, tag="rstd")
            nc.scalar.activation(rstd[:], ssum[:], AF.Sqrt,
                                 scale=1.0 / C, bias=eps_t[:])
            rinv = stats.tile([P, 1], FP, tag="rinv")
            nc.vector.reciprocal(rinv[:], rstd[:])
            nc.vector.scalar_tensor_tensor(
                dst[:], src[:], rinv[:], gb[:],
                op0=mybir.AluOpType.mult, op1=mybir.AluOpType.mult)

        # ---- stage 0: o = attnT^T @ Wo; r1 = x + o; h = rmsnorm(r1)*g1; hT
        hs = []
        hts_all = ht_pool.tile([P, NCH, NTOK], BF, tag="hts", name="hts_all")
        hts = [hts_all[:, c, :] for c in range(NCH)]
        with tc.tile_pool(name="o_ps", bufs=2, space="PSUM") as o_ps, \
             tc.tile_pool(name="t_ps", bufs=2, space="PSUM") as t_ps, \
             tc.tile_pool(name="wop", bufs=NCH) as wop, \
             tc.tile_pool(name="atsp", bufs=NCH) as atsp, \
             tc.tile_pool(name="xcp", bufs=1) as xcp, \
             tc.tile_pool(name="hbf", bufs=2) as hbf:
            atts, wots = [], []
            for c in range(NKC):
                att = atsp.tile([P, 2, NTOK], PF8, tag="at", name="at")
                nc.sync.dma_start(out=att[:], in_=at_d[c])
                wot = wop.tile([P, 2, C], PF8, tag="wo", name="wo")
                nc.sync.dma_start(out=wot[:], in_=wo_d[c])
                atts.append(att)
                wots.append(wot)
            xcs = []
            for tb in range(NTB):
                t = xcp.tile([P, C], FP, tag=f"xc{tb}")
                nc.scalar.dma_start(out=t[:], in_=xc_d[tb * P:(tb + 1) * P, :])
                xcs.append(t)

            def wo_mms(tb, o):
                for c in range(NKC):
                    for half in range(2):
                        nc.tensor.matmul(
                            o[:, half * 512:(half + 1) * 512],
                            atts[c][:, :, tb * P:(tb + 1) * P],
                            wots[c][:, :, half * 512:(half + 1) * 512],
                            start=(c == 0), stop=(c == NKC - 1),
                            perf_mode=DRM)

            def norm_h(tb, o):
                r1 = work.tile([P, C], FP, tag="r1")
                nc.vector.scalar_tensor_tensor(
                    r1[:], o[:], sc16[:], xcs[tb][:],
                    op0=mybir.AluOpType.mult, op1=mybir.AluOpType.add)
                h = h_pool.tile([P, C], FP, tag=f"h{tb}")
                rmsnorm_to(h, r1, g1b)
                hs.append(h)
                hb = hbf.tile([P, C], BF, tag="hbf")
                nc.vector.tensor_copy(hb[:], h[:])
                return hb

            def trans_h(tb, hb):
                # 8 PE transposes into one PSUM tile, ONE batched DVE copy
                tp = t_ps.tile([P, NCH, P], BF, tag="tp")
                for c in range(NCH):
                    nc.tensor.transpose(
                        tp[:, c, :], hb[:, c * P:(c + 1) * P], ident[:])
                nc.vector.tensor_copy(
                    hts_all[:, :, tb * P:(tb + 1) * P], tp[:])

            # pipeline: Wo(tb) | Wo(tb+1) + trans(tb) | ... so the PE never
            # waits on the norm chain
            o_tiles, hbs = [], []
            for tb in range(NTB):
                o = o_ps.tile([P, C], FP, tag="o")
                wo_mms(tb, o)
                o_tiles.append(o)
                hbs.append(norm_h(tb, o))
                if tb >= 1:
                    trans_h(tb - 1, hbs[tb - 1])
            trans_h(NTB - 1, hbs[NTB - 1])

        # ---- stage 2: aT = silu(W1^T @ h^T + b1)  [dff, tok] bf16
        ats = []
        with tc.tile_pool(name="a_ps", bufs=8, space="PSUM") as a_ps, \
             tc.tile_pool(name="w1p", bufs=24) as w1p, \
             tc.tile_pool(name="sgp", bufs=3) as sgp:
            for g in range(NG):
                w1ts = []
                for c in range(NCH):
                    w1t = w1p.tile([P, 512], BF, tag="w1")
                    nc.gpsimd.dma_start(out=w1t[:], in_=w1_d[g, c])
                    w1ts.append(w1t)
                if g == 0:
                    nc.gpsimd.dma_start(out=b1b[:], in_=bcast_rows(b1_d, DFF))
                if g == 2:
                    nc.gpsimd.dma_start(out=b2b[:], in_=bcast_rows(b2_d, C))
                aps = [a_ps.tile([P, NTOK], FP, tag="a", name="a")
                       for _ in range(4)]
                for c in range(NCH):
                    for u in range(4):
                        nc.tensor.matmul(
                            aps[u][:], w1ts[c][:, u * P:(u + 1) * P],
                            hts[c],
                            start=(c == 0), stop=(c == NCH - 1))
                for u in range(4):
                    d = 4 * g + u
                    sg = sgp.tile([P, NTOK], FP, tag="sg")
                    nc.scalar.activation(sg[:], aps[u][:], AF.Sigmoid,
                                         bias=b1b[:, d:d + 1], scale=1.0)
                    at_t = at_pool.tile([P, NTOK], BF, tag=f"at{d}")
                    # silu(z) for z = a + b1: (a + b1) * sigmoid(a + b1)
                    nc.vector.scalar_tensor_tensor(
                        at_t[:], aps[u][:], b1b[:, d:d + 1], sg[:],
                        op0=mybir.AluOpType.add, op1=mybir.AluOpType.mult)
                    ats.append(at_t)

        # hb[tb] = h + b2, precomputed on DVE while the PE runs W2
        hb2s = []
        for tb in range(NTB):
            hb2 = hb2p.tile([P, C], FP, tag=f"hb2_{tb}")
            nc.vector.tensor_add(hb2[:], hs[tb][:], b2b[:])
            hb2s.append(hb2)

        # preload the Sqrt table set during W2 so the tail norm doesn't
        # pay the ~2.7us table switch
        dummy = stats.tile([P, 1], FP, tag="dummy")
        nc.scalar.activation(dummy[:], eps_t[:], AF.Sqrt, scale=1.0)

        # ---- stage 3: f = aT^T @ W2; r2 = h + b2 + f; out = rmsnorm(r2)*g2
        with tc.tile_pool(name="f_ps", bufs=1, space="PSUM") as f_ps:
            fts = [f_ps.tile([P, C], FP, tag=f"f{tb}", name=f"f{tb}")
                   for tb in range(NTB)]
            for d in range(NDF):
                w2t = w2sb.tile([P, C], BF, tag="w2")
                nc.gpsimd.dma_start(out=w2t[:], in_=w2_d[d * P:(d + 1) * P, :])
                if d == 4:
                    nc.gpsimd.dma_start(out=g2b[:], in_=bcast_rows(g2_d, C))
                for tb in range(NTB):
                    for half in range(2):
                        nc.tensor.matmul(
                            fts[tb][:, half * 512:(half + 1) * 512],
                            ats[d][:, tb * P:(tb + 1) * P],
                            w2t[:, half * 512:(half + 1) * 512],
                            start=(d == 0), stop=(d == NDF - 1))
            for tb in range(NTB):
                r2 = work.tile([P, C], FP, tag="r2")
                nc.vector.tensor_add(r2[:], fts[tb][:], hb2s[tb][:])
                o = work.tile([P, C], FP, tag="outt")
                rmsnorm_to(o, r2, g2b)
                eng = nc.scalar if tb % 2 == 0 else nc.sync
                eng.dma_start(out=out_d[tb * P:(tb + 1) * P, :], in_=o[:])
    nc.compile()
    return nc


# --------------------------------------------------------------------------
# host orchestration
# --------------------------------------------------------------------------

_CACHE = {}


def _phase1(B, T, C, DH):
    key = ("p1", B, T, C, DH)
    if key not in _CACHE:
        _CACHE[key] = build_phase1(B, T, C, DH)
    return _CACHE[key]


def _phase2(NTOK, C, DFF):
    key = ("p2", NTOK, C, DFF)
    if key not in _CACHE:
        _CACHE[key] = build_phase2(NTOK, C, DFF)
    return _CACHE[key]


def _run(nc, in_maps, ldw_opt=False):
    import os
    import concourse.bass_utils as _bu
    _bu._ldw_opt_enable = ldw_opt    # DR ldweights are incompatible with it
    trace = bool(os.environ.get("KERNEL_TRACE"))
    kwargs = {}
    if trace:
        _install_ntff_hook_shim()
        tdir = os.environ.get("KERNEL_TRACE_DIR")
        if tdir:
            phase_dir = os.path.join(tdir, f"phase{len(LAST_EXEC_NS)}")
            os.makedirs(phase_dir, exist_ok=True)
            kwargs["tmpdir"] = phase_dir
    res = run_bass_kernel_spmd(nc, in_maps, core_ids=list(range(N_CORES)),
                               trace=trace, **kwargs)
    LAST_EXEC_NS.append(res.exec_time_ns)
    return res.results


def kernel(x, Wq, Wk, Wv, Wo, bo, W1, b1, W2, b2, g1, g2):
    f32 = lambda a: np.ascontiguousarray(np.asarray(a), dtype=np.float32)
    x = f32(x)
    Wq, Wk, Wv, Wo, bo = f32(Wq), f32(Wk), f32(Wv), f32(Wo), f32(bo)
    W1, b1, W2, b2, g1, g2 = f32(W1), f32(b1), f32(W2), f32(b2), f32(g1), f32(g2)

    B, T, C = x.shape
    H, _, DH = Wq.shape
    HP = H // N_CORES           # heads per core (2)
    DA = DH + 1
    LAST_EXEC_NS.clear()

    # ---- phase 1
    nc1 = _phase1(B, T, C, DH)
    NKK = C // 256
    # [B,T,C] -> [B, n, kk, 128, 2, 512] with channel c = (2*kk+s)*128 + p
    # and token t = n*512 + t'
    xT8 = np.ascontiguousarray(
        x.transpose(0, 2, 1).reshape(B, NKK, 2, 128, T // 512, 512)
        .transpose(0, 4, 1, 3, 2, 5)).astype(F8_NP)
    in1 = []
    for i in range(N_CORES):
        pq = Wq[HP * i:HP * (i + 1)].transpose(1, 0, 2).reshape(C, HP * DH)
        pk = Wk[HP * i:HP * (i + 1)].transpose(1, 0, 2).reshape(C, HP * DH)
        pv = Wv[HP * i:HP * (i + 1)].transpose(1, 0, 2).reshape(C, HP * DH)
        pair = lambda w: np.ascontiguousarray(
            (w * WS).reshape(NKK, 2, 128, HP * DH)
            .transpose(0, 2, 1, 3)).astype(F8_NP)
        in1.append({"xT8": xT8,
                    "wq8": pair(pq),
                    "wk8": pair(pk),
                    "wv8": np.ascontiguousarray(
                        (pv * WS).reshape(2 * NKK, 128, HP * DH)
                    ).astype(F8_NP)})
    res1 = _run(nc1, in1)

    attn = np.empty((B, T, C), np.float32)
    for i in range(N_CORES):
        ot = res1[i]["ot"]                    # [B, HP, DA, T]
        o = ot[:, :, :DH, :]
        den = ot[:, :, DH, :] * WS            # V carries a x32 scale
        on = o / den[:, :, None, :]
        for hh in range(HP):
            hcol = (HP * i + hh) * DH
            attn[:, :, hcol:hcol + DH] = on[:, hh].transpose(0, 2, 1)

    # ---- phase 2
    NTOK = B * T // N_CORES
    nc2 = _phase2(NTOK, C, W1.shape[1])
    xf = x.reshape(B * T, C) + bo             # fold bo into the residual
    af = attn.reshape(B * T, C)
    DFF = W1.shape[1]
    NKC = C // 256
    # Wo in fp8 DoubleRow pairs, x4 scale on both operands (undone by 1/16)
    wo8 = np.ascontiguousarray(
        (Wo * 4.0).reshape(NKC, 2, 128, C).transpose(0, 2, 1, 3)).astype(F8_NP)
    # [C, DFF] -> [g, c, 128, 512] so each (g, c) chunk is DMA-contiguous
    w1_bf = np.ascontiguousarray(
        W1.reshape(C // 128, 128, DFF // 512, 512)
        .transpose(2, 0, 1, 3)).astype(BF_NP)
    w2_bf = W2.astype(BF_NP)
    in2 = []
    for k in range(N_CORES):
        sl = slice(k * NTOK, (k + 1) * NTOK)
        at8 = np.ascontiguousarray(
            (af[sl].T * 4.0).reshape(NKC, 2, 128, NTOK)
            .transpose(0, 2, 1, 3)).astype(F8_NP)
        in2.append({
            "xc": np.ascontiguousarray(xf[sl]),
            "attnT8": at8,
            "wo8": wo8, "w1p": w1_bf, "w2": w2_bf,
            "g1": g1, "g2": g2, "b1": b1, "b2": b2,
        })
    res2 = _run(nc2, in2)
    out = np.concatenate([res2[k]["out"] for k in range(N_CORES)], axis=0)
    return out.reshape(B, T, C)



# revision 34
# speedup vs baseline: 1.4863x; 1.0256x over previous
"""Trainium2 Bass kernel for a dense transformer block, distributed over 8
NeuronCores.

Sharding:
  phase 1 (attention): tensor-parallel over heads — each core computes 2 of
    the 16 heads end-to-end (QKV projections + causal softmax(QK^T)V), and
    returns the unnormalized per-head output O^T together with the softmax
    denominators (obtained via a ones-column appended to V).
  phase 2 (Wo + norms + FFN): data-parallel over tokens — each core handles
    512 of the 4096 token rows with replicated weights.

The host glues the phases: transposes x, normalizes/concats heads, and
re-shards tokens.  All matmuls run as float32r (full-rate fp32 PE mode).
"""

import math
from contextlib import ExitStack

import ml_dtypes
import numpy as np

BF_NP = ml_dtypes.bfloat16

import concourse.bass as bass
import concourse.mybir as mybir
import concourse.tile as tile
from concourse import bacc
from concourse.bass_utils import run_bass_kernel_spmd
from concourse.masks import make_identity, make_upper_triangular

FP = mybir.dt.float32
FPR = mybir.dt.float32r
BF = mybir.dt.bfloat16
AF = mybir.ActivationFunctionType

N_CORES = 8
P = 128
EPS = 1e-6

# exec times (ns) of the most recent kernel() call, one entry per phase, when
# tracing was enabled via BASS_TRACE=1; None entries otherwise.
LAST_EXEC_NS = []


def _install_ntff_hook_shim():
    """Provide antenv.axon_hooks when the image lacks it, so trace=True can
    drive NTFF profiling through libaxon_pjrt's C ABI (same contract as
    trn_boot's step-6 hook). No-op if the real module exists or the .so is
    missing/old."""
    try:
        import antenv.axon_hooks  # noqa: F401
        return
    except ImportError:
        pass
    import contextlib
    import ctypes
    import sys
    import types

    try:
        lib = ctypes.CDLL("/opt/axon/libaxon_pjrt.so")
    except OSError:
        return
    if not hasattr(lib, "axon_start_nrt_profile"):
        return
    lib.axon_start_nrt_profile.argtypes = [
        ctypes.POINTER(ctypes.c_int64), ctypes.c_size_t]
    lib.axon_start_nrt_profile.restype = ctypes.c_int64
    lib.axon_stop_nrt_profile.argtypes = [ctypes.c_char_p]
    lib.axon_stop_nrt_profile.restype = ctypes.c_int64

    @contextlib.contextmanager
    def _hook(output_dir, device_ids):
        import jax
        jax.devices()
        if device_ids:
            ids = (ctypes.c_int64 * len(device_ids))(*device_ids)
            rc = lib.axon_start_nrt_profile(ids, len(device_ids))
        else:
            rc = lib.axon_start_nrt_profile(None, 0)
        if rc != 0:
            raise RuntimeError(f"axon_start_nrt_profile rc={rc}")
        try:
            yield
        finally:
            n = lib.axon_stop_nrt_profile(str(output_dir).encode())
            if n < 0:
                raise RuntimeError(f"axon_stop_nrt_profile rc={n}")

    mod = types.ModuleType("antenv.axon_hooks")
    mod.get_axon_ntff_profile_hook = lambda: _hook

    def set_axon_ntff_profile_hook(h):
        mod.get_axon_ntff_profile_hook = lambda: h

    mod.set_axon_ntff_profile_hook = set_axon_ntff_profile_hook
    import antenv
    antenv.axon_hooks = mod
    sys.modules["antenv.axon_hooks"] = mod


def _fpr(ap):
    return ap.bitcast(FPR)


def _enable_ldw_opt():
    """Flip walrus's --enable-ldw-opt to true so LDWEIGHTS overlaps with
    in-flight matmuls (background weight buffer). Verified by the rel-err
    check; idempotent."""
    import concourse.bass_utils as _bu
    if getattr(_bu, "_ldw_opt_patched", False):
        return
    orig = _bu.run_command

    def patched(cmd, *a, **kw):
        if isinstance(cmd, list) and getattr(_bu, "_ldw_opt_enable", False):
            cmd = ["--enable-ldw-opt=true" if c == "--enable-ldw-opt=false"
                   else c for c in cmd]
        return orig(cmd, *a, **kw)

    _bu.run_command = patched
    _bu._ldw_opt_patched = True


_enable_ldw_opt()


# --------------------------------------------------------------------------
# phase 1: per-core attention over a pair of heads (fp8 + DoubleRow)
# --------------------------------------------------------------------------

PF8 = mybir.dt.float8e4
F8_NP = ml_dtypes.float8_e4m3
WS = 32.0          # q/k/v weight upscale; folded out via exp scale + host div


class _Fillers:
    """FIFO of deferred PE-work emitters, drained one unit at a time into
    the exp-gated attention loop so the tensor engine never idles."""

    def __init__(self):
        self.q = []

    def add(self, fn):
        self.q.append(fn)

    def take(self, n):
        out, self.q = self.q[:n], self.q[n:]
        return out

    def drain(self):
        out, self.q = self.q, []
        return out


def build_phase1(B, T, C, DH):
    HP = 2                      # heads per core
    DA = DH + 1                 # head dim + ones row (softmax denominator)
    DAP = 80                    # DA padded so the DoubleRow pair stride is 16B-aligned
    NKK = C // 256              # DoubleRow contraction pairs
    NST = T // P                # key stripes of 128
    NM = T // 512               # query blocks of 512
    scale = float(C) ** -0.5 / (WS * WS)
    DR = mybir.MatmulPerfMode.DoubleRow

    nc = bacc.Bacc("TRN2", debug=False)
    # token-sliced so each (b, n, kk) chunk is contiguous and the first
    # projection can start after one 128KB DMA
    xT_d = nc.dram_tensor("xT8", [B, NM, NKK, P, 2, 512], PF8,
                          kind="ExternalInput").ap()
    wq_d = nc.dram_tensor("wq8", [NKK, P, 2, HP * DH], PF8,
                          kind="ExternalInput").ap()
    wk_d = nc.dram_tensor("wk8", [NKK, P, 2, HP * DH], PF8,
                          kind="ExternalInput").ap()
    wv_d = nc.dram_tensor("wv8", [2 * NKK, P, HP * DH], PF8,
                          kind="ExternalInput").ap()
    ot_d = nc.dram_tensor("ot", [B, HP, DA, T], FP, kind="ExternalOutput").ap()

    with tile.TileContext(nc) as tc, ExitStack() as ctx:
        const = ctx.enter_context(tc.tile_pool(name="const", bufs=1))
        xp = ctx.enter_context(tc.tile_pool(name="xp", bufs=1))
        wp = ctx.enter_context(tc.tile_pool(name="wp", bufs=1))
        qkp = ctx.enter_context(tc.tile_pool(name="qkp", bufs=1))
        vap = ctx.enter_context(tc.tile_pool(name="vap", bufs=1))
        ptkp = ctx.enter_context(tc.tile_pool(name="ptkp", bufs=6))
        otp = ctx.enter_context(tc.tile_pool(name="otp", bufs=1))
        pp = ctx.enter_context(tc.tile_pool(name="pp", bufs=2, space="PSUM"))
        sp = ctx.enter_context(tc.tile_pool(name="sp", bufs=2, space="PSUM"))
        op = ctx.enter_context(tc.tile_pool(name="op", bufs=1, space="PSUM"))

        # preload the Exp table set before the attention loop needs it
        warm = const.tile([P, 1], FP)
        nc.vector.memset(warm[:], 0.0)
        nc.scalar.activation(warm[:], warm[:], AF.Exp, scale=1.0)

        xts, qts, kts, vaugs, otsbs = {}, {}, {}, {}, {}
        wqs, wks, wvs = [], [], []

        def load_x(b, ns):
            # one DMA per (b, n) covering all kk chunks; rhs/lhsT slices of
            # the combined tile keep the DoubleRow pair layout
            for n in ns:
                t = xp.tile([P, NKK, 2, 512], PF8, tag=f"x{b}_{n}",
                            name=f"x{b}_{n}")
                src_ap = bass.AP(
                    tensor=xT_d.tensor,
                    offset=xT_d.offset + (b * NM + n) * NKK * P * 1024,
                    ap=[[1024, P], [P * 1024, NKK], [512, 2], [1, 512]])
                nc.sync.dma_start(out=t[:], in_=src_ap)
                for kk in range(NKK):
                    xts[(b, n, kk)] = t[:, kk, :, :]

        for b in range(B):
            qts[b] = qkp.tile([P, T], PF8, tag=f"qt{b}", name=f"qt{b}")
            kts[b] = qkp.tile([P, T], PF8, tag=f"kt{b}", name=f"kt{b}")
            va = vap.tile([P, NST // 2, 2, HP, DAP], PF8, tag=f"va{b}",
                          name=f"va{b}")
            nc.vector.memset(va[:], 1.0)    # ones column survives in col DH
            vaugs[b] = va
            for h in range(HP):
                otsbs[(b, h)] = otp.tile([DA, T], FP, tag=f"ot{b}{h}", name=f"ot{b}{h}")

        def emit_qk_chunk(b, proj, n):
            # one 512-token chunk of the Q or K projection (NKK DR matmuls)
            wt = wqs if proj == 0 else wks
            dst = qts[b] if proj == 0 else kts[b]
            ps = pp.tile([P, 512], FP, tag="pp")
            for kk in range(NKK):
                nc.tensor.matmul(
                    ps[:], wt[kk][:],
                    xts[(b, n, kk)],
                    start=(kk == 0), stop=(kk == NKK - 1), perf_mode=DR)
            nc.vector.tensor_copy(dst[:, n * 512:(n + 1) * 512], ps[:])

        def emit_v_stripe(b, s):
            # V^T stripe s directly in [token, head-dim] layout
            n, so = s // 4, (s % 4) * P
            ps = pp.tile([P, 512], FP, tag="pp")
            out = ps[:, 0:HP * DH]
            for c in range(2 * NKK):
                nc.tensor.matmul(
                    out, xts[(b, n, c // 2)][:, c % 2, so:so + P],
                    wvs[c][:],
                    start=(c == 0), stop=(c == 2 * NKK - 1))

            for h in range(HP):
                nc.vector.tensor_copy(
                    vaugs[b][:, s // 2, s % 2, h, 0:DH],
                    ps[:, h * DH:(h + 1) * DH])

        def emit_spv_block(b, m, fillers):
            o_tiles = [op.tile([DA, 512], FP, tag=f"o{h}", name=f"o{h}")
                       for h in range(HP)]
            npairs = 2 * (m + 1)
            for jj in range(npairs):
                ptk = ptkp.tile([P, 2, HP, 512], PF8, tag="ptk")
                for ss in range(2):
                    j = 2 * jj + ss
                    s0 = j * P
                    diag = s0 >= m * 512
                    off = s0 - m * 512 if diag else 0
                    st = sp.tile([P, HP, 512], FP, tag="st")
                    for h in range(HP):
                        hs = slice(h * DH, (h + 1) * DH)
                        nc.tensor.matmul(
                            st[:, h, off:512],
                            kts[b][hs, s0:s0 + P],
                            qts[b][hs, m * 512 + off:(m + 1) * 512],
                            start=True, stop=True,
                            tile_position=(h * DH, 0))
                    if diag and off > 0:
                        nc.gpsimd.memset(ptk[:, ss, :, 0:off], 0.0)
                    nc.scalar.activation(
                        ptk[:, ss, :, off:512], st[:, :, off:512],
                        AF.Exp, scale=scale)
                    if diag:
                        # zero the non-causal triangle of the diagonal 128
                        # cols after the exp, off the ACT critical path
                        for h in range(HP):
                            nc.gpsimd.affine_select(
                                out=ptk[:, ss, h, off:off + P],
                                in_=ptk[:, ss, h, off:off + P],
                                compare_op=mybir.AluOpType.is_ge, fill=0.0,
                                base=0, pattern=[[1, P]],
                                channel_multiplier=-1)
                    for f in fillers.take(1):
                        f()
                for h in range(HP):
                    nc.tensor.matmul(
                        o_tiles[h][:], vaugs[b][:, jj, :, h, 0:DA],
                        ptk[:, :, h, :],
                        start=(jj == 0), stop=(jj == npairs - 1),
                        perf_mode=DR)
            for h in range(HP):
                nc.vector.tensor_copy(
                    otsbs[(b, h)][:, m * 512:(m + 1) * 512], o_tiles[h][:])

        # ---- minimal prelude: only what block (0,0) needs up front,
        # DMA-ordered so the first matmul's operands land first
        load_x(0, [0])
        for kk in range(NKK):
            wq = wp.tile([P, 2, HP * DH], PF8, tag=f"wq{kk}", name=f"wq{kk}")
            nc.sync.dma_start(out=wq[:], in_=wq_d[kk])
            wk = wp.tile([P, 2, HP * DH], PF8, tag=f"wk{kk}", name=f"wk{kk}")
            nc.sync.dma_start(out=wk[:], in_=wk_d[kk])
            wqs.append(wq)
            wks.append(wk)
        for c in range(2 * NKK):
            wv = wp.tile([P, HP * DH], PF8, tag=f"wv{c}", name=f"wv{c}")
            nc.sync.dma_start(out=wv[:], in_=wv_d[c])
            wvs.append(wv)
        emit_qk_chunk(0, 0, 0)
        emit_qk_chunk(0, 1, 0)
        emit_v_stripe(0, 0)
        emit_v_stripe(0, 1)
        load_x(0, [1, 2, 3])
        load_x(1, range(NM))

        # b0/b1 blocks interleaved so the exp (ACT) stream never drains and
        # the PE always has S/PV + filler (V / Q/K projection) work. The
        # filler FIFO order matches each unit's first use in block order:
        # blocks b0m0(4 slots) b0m1(8) b0m2(12) b1m0(4) b0m3(16) b1m1(8)
        # b1m2(12) b1m3(16).
        fill = _Fillers()
        fill.add(lambda: emit_v_stripe(0, 2))
        fill.add(lambda: emit_v_stripe(0, 3))
        fill.add(lambda: emit_qk_chunk(0, 0, 1))
        fill.add(lambda: emit_qk_chunk(0, 1, 1))
        for s in range(4, 8):
            fill.add(lambda s=s: emit_v_stripe(0, s))
        for n in (2, 3):
            fill.add(lambda n=n: emit_qk_chunk(0, 0, n))
            fill.add(lambda n=n: emit_qk_chunk(0, 1, n))
        for s in range(8, 12):
            fill.add(lambda s=s: emit_v_stripe(0, s))
        fill.add(lambda: emit_qk_chunk(1, 0, 0))
        fill.add(lambda: emit_qk_chunk(1, 1, 0))
        for s in range(4):
            fill.add(lambda s=s: emit_v_stripe(1, s))
        fill.add(lambda: emit_qk_chunk(1, 0, 1))
        fill.add(lambda: emit_qk_chunk(1, 1, 1))
        for s in range(12, NST):
            fill.add(lambda s=s: emit_v_stripe(0, s))
        for n in (2, 3):
            fill.add(lambda n=n: emit_qk_chunk(1, 0, n))
            fill.add(lambda n=n: emit_qk_chunk(1, 1, n))
        for s in range(4, NST):
            fill.add(lambda s=s: emit_v_stripe(1, s))

        for b, m in ((0, 0), (0, 1), (0, 2), (1, 0),
                     (0, 3), (1, 1), (1, 2), (1, 3)):
            emit_spv_block(b, m, fill)
            for h in range(HP):
                nc.gpsimd.dma_start(
                    out=ot_d[b, h, :, m * 512:(m + 1) * 512],
                    in_=otsbs[(b, h)][:, m * 512:(m + 1) * 512])
        for f in fill.drain():
            f()
    nc.compile()
    return nc


def build_phase1_v1(B, T, C, DH):
    HP = 2                      # heads per core
    DA = DH + 1                 # head dim + ones row (softmax denominator)
    NCC = C // P                # contraction chunks
    NT = T // P                 # key/value blocks of 128
    NQ = T // 512               # query chunks of 512
    NK = T // 1024              # query tiles of 1024
    scale = float(C) ** -0.5    # NOTE: reference scales by C**-0.5, not DH

    nc = bacc.Bacc("TRN2", debug=False)
    xT_d = nc.dram_tensor("xT", [B, C, T], BF, kind="ExternalInput").ap()
    wq_d = nc.dram_tensor("wq", [C, HP * DH], BF, kind="ExternalInput").ap()
    wk_d = nc.dram_tensor("wk", [C, HP * DH], BF, kind="ExternalInput").ap()
    wv_d = nc.dram_tensor("wv", [C, HP * DH], BF, kind="ExternalInput").ap()
    ot_d = nc.dram_tensor("ot", [B, HP, DA, T], FP, kind="ExternalOutput").ap()

    with tile.TileContext(nc) as tc, ExitStack() as ctx:
        const = ctx.enter_context(tc.tile_pool(name="const", bufs=1))
        xpool = ctx.enter_context(tc.tile_pool(name="xp", bufs=1))
        wpool = ctx.enter_context(tc.tile_pool(name="wp", bufs=1))
        qk_pool = ctx.enter_context(tc.tile_pool(name="qk", bufs=2))
        vt_pool = ctx.enter_context(tc.tile_pool(name="vtp", bufs=2))
        vaug_pool = ctx.enter_context(tc.tile_pool(name="vaug", bufs=2))
        pt_pool = ctx.enter_context(tc.tile_pool(name="pt", bufs=4))
        ot_pool = ctx.enter_context(tc.tile_pool(name="otp", bufs=2))

        # additive mask for the diagonal 128x128 block of S^T [s', q']:
        # 0 where q' >= s' (causal-valid), -1e30 where q' < s'
        negmask = const.tile([P, P], FP)
        nc.gpsimd.memset(negmask[:], 0.0)
        nc.gpsimd.affine_select(
            out=negmask[:], in_=negmask[:],
            compare_op=mybir.AluOpType.is_ge, fill=-1e30,
            base=0, pattern=[[1, P]], channel_multiplier=-1)
        ident = const.tile([P, P], BF)
        make_identity(nc, ident[:])
        ones_col = const.tile([P, NT * HP, 1], FP)
        nc.vector.memset(ones_col[:], 1.0)

        # weight chunks, loaded once
        wts = {}
        for name, src in (("q", wq_d), ("k", wk_d), ("v", wv_d)):
            wts[name] = []
            for c in range(NCC):
                t = wpool.tile([P, HP * DH], BF, tag=f"w{name}{c}")
                nc.sync.dma_start(out=t[:], in_=src[c * P:(c + 1) * P, :])
                wts[name].append(t)

        for b in range(B):
            xts = []
            for c in range(NCC):
                xt = xpool.tile([P, T], BF, tag=f"x{c}")
                nc.sync.dma_start(out=xt[:], in_=xT_d[b, c * P:(c + 1) * P, :])
                xts.append(xt)

            qt = qk_pool.tile([P, T], BF, tag="qt")
            kt = qk_pool.tile([P, T], BF, tag="kt")
            vaug = vaug_pool.tile([P, NT * HP, DA], BF, tag="vaug")
            # ones column per head-block (softmax denominator row of O^T)
            nc.vector.tensor_copy(vaug[:, :, DA - 1:DA], ones_col[:])

            with tc.tile_pool(name="proj_ps", bufs=3, space="PSUM") as proj_ps, \
                 tc.tile_pool(name="vt_ps", bufs=2, space="PSUM") as vt_ps:
                for wt, dst in ((wts["q"], qt), (wts["k"], kt)):
                    for n in range(NQ):
                        ps = proj_ps.tile([P, 512], FP, tag="proj")
                        for c in range(NCC):
                            nc.tensor.matmul(
                                ps[:], wt[c][:], xts[c][:, n * 512:(n + 1) * 512],
                                start=(c == 0), stop=(c == NCC - 1))
                        nc.vector.tensor_copy(dst[:, n * 512:(n + 1) * 512], ps[:])
                # V, then transpose into [s, d] layout with ones columns
                for n in range(NQ):
                    ps = proj_ps.tile([P, 512], FP, tag="proj")
                    for c in range(NCC):
                        nc.tensor.matmul(
                            ps[:], wts["v"][c][:], xts[c][:, n * 512:(n + 1) * 512],
                            start=(c == 0), stop=(c == NCC - 1))
                    vt = vt_pool.tile([P, 512], BF, tag="vt")
                    nc.vector.tensor_copy(vt[:], ps[:])
                    for u in range(4):
                        j = 4 * n + u
                        tp = vt_ps.tile([P, P], BF, tag="vtp")
                        nc.tensor.transpose(tp[:], vt[:, u * P:(u + 1) * P], ident[:])
                        nc.vector.tensor_copy(
                            vaug[:, j * HP, 0:DH], tp[:, 0:DH])
                        nc.vector.tensor_copy(
                            vaug[:, j * HP + 1, 0:DH], tp[:, DH:2 * DH])

            with tc.tile_pool(name="s_ps", bufs=2, space="PSUM") as s_ps, \
                 tc.tile_pool(name="o_ps", bufs=1, space="PSUM") as o_ps:
                ot_sbs = [ot_pool.tile([DA, T], FP, tag=f"ot{h}", name=f"ot{h}")
                          for h in range(HP)]
                for k in range(NK):
                    q_lo = 1024 * k
                    q_hi = 1024 * (k + 1)
                    o_tiles = [o_ps.tile([DA, 1024], FP, tag=f"o{h}", name=f"o{h}")
                               for h in range(HP)]
                    for j in range(8 * (k + 1)):
                        s0 = j * P
                        a0 = max(s0, q_lo)
                        # 512-grid chunks of the valid q range in this stripe
                        chunks = []
                        m0 = a0 // 512
                        for m in range(m0, q_hi // 512):
                            a = max(a0, m * 512)
                            e = (m + 1) * 512
                            chunks.append((a, e))
                        stl = [s_ps.tile([P, 1024], FP, tag="s", name="s")
                               for _ in range(HP)]
                        # emit head pairs adjacently: rows 0-63 (head A) and
                        # 64-127 (head B) run concurrently in the PE array
                        for (a, e) in chunks:
                            for h in range(HP):
                                hs = slice(h * DH, (h + 1) * DH)
                                nc.tensor.matmul(
                                    stl[h][:, a - q_lo:e - q_lo],
                                    kt[hs, s0:s0 + P], qt[hs, a:e],
                                    start=True, stop=True,
                                    tile_position=(h * DH, 0))
                        if q_lo <= s0:
                            for h in range(HP):
                                # diagonal block: additive causal mask
                                nc.vector.tensor_add(
                                    stl[h][:, s0 - q_lo:s0 - q_lo + P],
                                    stl[h][:, s0 - q_lo:s0 - q_lo + P],
                                    negmask[:])
                        for h in range(HP):
                            ptk = pt_pool.tile([P, 1024], BF, tag="pt")
                            nc.scalar.activation(
                                ptk[:, a0 - q_lo:1024], stl[h][:, a0 - q_lo:1024],
                                AF.Exp, scale=scale)
                            va = vaug[:, j * HP + h, :]
                            for (a, e) in chunks:
                                last_j = e // P - 1
                                nc.tensor.matmul(
                                    o_tiles[h][:, a - q_lo:e - q_lo],
                                    va, ptk[:, a - q_lo:e - q_lo],
                                    start=(j == 0), stop=(j == last_j))
                    for h in range(HP):
                        nc.vector.tensor_copy(
                            ot_sbs[h][:, q_lo:q_hi], o_tiles[h][:])
                for h in range(HP):
                    nc.sync.dma_start(out=ot_d[b, h], in_=ot_sbs[h][:])
    nc.compile()
    return nc


# --------------------------------------------------------------------------
# phase 2: per-core Wo projection + residual + rmsnorm + FFN + rmsnorm
# --------------------------------------------------------------------------

def build_phase2(NTOK, C, DFF):
    NTB = NTOK // P             # 4 token tiles of 128
    NCH = C // P                # 8 channel chunks
    NDF = DFF // P              # 32 dff chunks
    NG = DFF // 512             # 8 dff groups of 512

    NKC = C // 256              # DoubleRow contraction pairs for Wo
    DRM = mybir.MatmulPerfMode.DoubleRow
    nc = bacc.Bacc("TRN2", debug=False)
    xc_d = nc.dram_tensor("xc", [NTOK, C], FP, kind="ExternalInput").ap()
    at_d = nc.dram_tensor("attnT8", [NKC, P, 2, NTOK], PF8,
                          kind="ExternalInput").ap()
    wo_d = nc.dram_tensor("wo8", [NKC, P, 2, C], PF8,
                          kind="ExternalInput").ap()
    # host-permuted W1: [g, c, 128, 512] so each (g, c) chunk is contiguous
    w1_d = nc.dram_tensor("w1p", [NG, NCH, P, 512], BF, kind="ExternalInput").ap()
    w2_d = nc.dram_tensor("w2", [DFF, C], BF, kind="ExternalInput").ap()
    g1_d = nc.dram_tensor("g1", [C], FP, kind="ExternalInput").ap()
    g2_d = nc.dram_tensor("g2", [C], FP, kind="ExternalInput").ap()
    b1_d = nc.dram_tensor("b1", [DFF], FP, kind="ExternalInput").ap()
    b2_d = nc.dram_tensor("b2", [C], FP, kind="ExternalInput").ap()
    out_d = nc.dram_tensor("out", [NTOK, C], FP, kind="ExternalOutput").ap()

    def bcast_rows(src_ap, cols):
        # DRAM vector [cols] -> [P, cols] (same row in every partition)
        return bass.AP(tensor=src_ap.tensor, offset=src_ap.offset,
                       ap=[[0, P], [1, cols]])

    with tile.TileContext(nc) as tc, ExitStack() as ctx:
        const = ctx.enter_context(tc.tile_pool(name="const", bufs=1))
        work = ctx.enter_context(tc.tile_pool(name="work", bufs=2))
        stats = ctx.enter_context(tc.tile_pool(name="stats", bufs=4))
        h_pool = ctx.enter_context(tc.tile_pool(name="hp", bufs=1))
        hb2p = ctx.enter_context(tc.tile_pool(name="hb2p", bufs=1))
        ht_pool = ctx.enter_context(tc.tile_pool(name="htp", bufs=1))
        at_pool = ctx.enter_context(tc.tile_pool(name="atp", bufs=1))
        w2sb = ctx.enter_context(tc.tile_pool(name="w2sb", bufs=10))

        ident = const.tile([P, P], BF)
        make_identity(nc, ident[:])
        eps_t = const.tile([P, 1], FP)
        nc.vector.memset(eps_t[:], EPS)
        sc16 = const.tile([P, 1], FP)
        nc.vector.memset(sc16[:], 1.0 / 16.0)   # undo the x4*x4 Wo fp8 scales
        # preload the Sqrt/Square table set before stage-0 norms need it
        warm = stats.tile([P, 1], FP, tag="warm")
        nc.scalar.activation(warm[:], eps_t[:], AF.Sqrt, scale=1.0)
        # g1 is needed first (stage-0 norms); the other broadcasts are
        # issued later, near their first consumer, to keep early HBM
        # bandwidth for the stage-0 operands
        g1b = const.tile([P, C], FP)
        nc.scalar.dma_start(out=g1b[:], in_=bcast_rows(g1_d, C))
        g2b = const.tile([P, C], FP)
        b2b = const.tile([P, C], FP)
        b1b = const.tile([P, DFF], FP)

        def rmsnorm_to(dst, src, gb):
            # dst = src * rsqrt(mean(src^2) + eps) * gb
            sq = work.tile([P, C], FP, tag="sq")
            ssum = stats.tile([P, 1], FP, tag="ssum")
            nc.scalar.activation(sq[:], src[:], AF.Square, accum_out=ssum[:])
            rstd = stats.tile([P, 1], FP, tag="rstd")
            nc.scalar.activation(rstd[:], ssum[:], AF.Sqrt,
                                 scale=1.0 / C, bias=eps_t[:])
            rinv = stats.tile([P, 1], FP, tag="rinv")
            nc.vector.reciprocal(rinv[:], rstd[:])
            nc.vector.scalar_tensor_tensor(
                dst[:], src[:], rinv[:], gb[:],
                op0=mybir.AluOpType.mult, op1=mybir.AluOpType.mult)

        # ---- stage 0: o = attnT^T @ Wo; r1 = x + o; h = rmsnorm(r1)*g1; hT
        hs = []
        hts_all = ht_pool.tile([P, NCH, NTOK], BF, tag="hts", name="hts_all")
        hts = [hts_all[:, c, :] for c in range(NCH)]
        with tc.tile_pool(name="o_ps", bufs=2, space="PSUM") as o_ps, \
             tc.tile_pool(name="t_ps", bufs=2, space="PSUM") as t_ps, \
             tc.tile_pool(name="wop", bufs=NCH) as wop, \
             tc.tile_pool(name="atsp", bufs=NCH) as atsp, \
             tc.tile_pool(name="xcp", bufs=1) as xcp, \
             tc.tile_pool(name="hbf", bufs=2) as hbf:
            atts, wots = [], []
            for c in range(NKC):
                att = atsp.tile([P, 2, NTOK], PF8, tag="at", name="at")
                nc.sync.dma_start(out=att[:], in_=at_d[c])
                wot = wop.tile([P, 2, C], PF8, tag="wo", name="wo")
                nc.sync.dma_start(out=wot[:], in_=wo_d[c])
                atts.append(att)
                wots.append(wot)
            xcs = []
            for tb in range(NTB):
                t = xcp.tile([P, C], FP, tag=f"xc{tb}")
                nc.sync.dma_start(out=t[:], in_=xc_d[tb * P:(tb + 1) * P, :])
                xcs.append(t)

            def wo_mms(tb, o):
                for c in range(NKC):
                    for half in range(2):
                        nc.tensor.matmul(
                            o[:, half * 512:(half + 1) * 512],
                            atts[c][:, :, tb * P:(tb + 1) * P],
                            wots[c][:, :, half * 512:(half + 1) * 512],
                            start=(c == 0), stop=(c == NKC - 1),
                            perf_mode=DRM)

            def norm_h(tb, o):
                r1 = work.tile([P, C], FP, tag="r1")
                nc.vector.scalar_tensor_tensor(
                    r1[:], o[:], sc16[:], xcs[tb][:],
                    op0=mybir.AluOpType.mult, op1=mybir.AluOpType.add)
                h = h_pool.tile([P, C], FP, tag=f"h{tb}")
                rmsnorm_to(h, r1, g1b)
                hs.append(h)
                hb = hbf.tile([P, C], BF, tag="hbf")
                nc.vector.tensor_copy(hb[:], h[:])
                return hb

            def trans_h(tb, hb):
                # 8 PE transposes into one PSUM tile, ONE batched DVE copy
                tp = t_ps.tile([P, NCH, P], BF, tag="tp")
                for c in range(NCH):
                    nc.tensor.transpose(
                        tp[:, c, :], hb[:, c * P:(c + 1) * P], ident[:])
                nc.vector.tensor_copy(
                    hts_all[:, :, tb * P:(tb + 1) * P], tp[:])

            # pipeline: Wo(tb) | Wo(tb+1) + trans(tb) | ... so the PE never
            # waits on the norm chain
            o_tiles, hbs = [], []
            for tb in range(NTB):
                o = o_ps.tile([P, C], FP, tag="o")
                wo_mms(tb, o)
                o_tiles.append(o)
                hbs.append(norm_h(tb, o))
                if tb >= 1:
                    trans_h(tb - 1, hbs[tb - 1])
            trans_h(NTB - 1, hbs[NTB - 1])

        # ---- stage 2: aT = silu(W1^T @ h^T + b1)  [dff, tok] bf16
        ats = []
        with tc.tile_pool(name="a_ps", bufs=8, space="PSUM") as a_ps, \
             tc.tile_pool(name="w1p", bufs=24) as w1p, \
             tc.tile_pool(name="sgp", bufs=3) as sgp:
            for g in range(NG):
                w1ts = []
                for c in range(NCH):
                    w1t = w1p.tile([P, 512], BF, tag="w1")
                    nc.gpsimd.dma_start(out=w1t[:], in_=w1_d[g, c])
                    w1ts.append(w1t)
                if g == 0:
                    nc.gpsimd.dma_start(out=b1b[:], in_=bcast_rows(b1_d, DFF))
                if g == 2:
                    nc.gpsimd.dma_start(out=b2b[:], in_=bcast_rows(b2_d, C))
                aps = [a_ps.tile([P, NTOK], FP, tag="a", name="a")
                       for _ in range(4)]
                for c in range(NCH):
                    for u in range(4):
                        nc.tensor.matmul(
                            aps[u][:], w1ts[c][:, u * P:(u + 1) * P],
                            hts[c],
                            start=(c == 0), stop=(c == NCH - 1))
                for u in range(4):
                    d = 4 * g + u
                    sg = sgp.tile([P, NTOK], FP, tag="sg")
                    nc.scalar.activation(sg[:], aps[u][:], AF.Sigmoid,
                                         bias=b1b[:, d:d + 1], scale=1.0)
                    at_t = at_pool.tile([P, NTOK], BF, tag=f"at{d}")
                    # silu(z) for z = a + b1: (a + b1) * sigmoid(a + b1)
                    nc.vector.scalar_tensor_tensor(
                        at_t[:], aps[u][:], b1b[:, d:d + 1], sg[:],
                        op0=mybir.AluOpType.add, op1=mybir.AluOpType.mult)
                    ats.append(at_t)

        # hb[tb] = h + b2, precomputed on DVE while the PE runs W2
        hb2s = []
        for tb in range(NTB):
            hb2 = hb2p.tile([P, C], FP, tag=f"hb2_{tb}")
            nc.vector.tensor_add(hb2[:], hs[tb][:], b2b[:])
            hb2s.append(hb2)

        # preload the Sqrt table set during W2 so the tail norm doesn't
        # pay the ~2.7us table switch
        dummy = stats.tile([P, 1], FP, tag="dummy")
        nc.scalar.activation(dummy[:], eps_t[:], AF.Sqrt, scale=1.0)

        # ---- stage 3: f = aT^T @ W2; r2 = h + b2 + f; out = rmsnorm(r2)*g2
        with tc.tile_pool(name="f_ps", bufs=1, space="PSUM") as f_ps:
            fts = [f_ps.tile([P, C], FP, tag=f"f{tb}", name=f"f{tb}")
                   for tb in range(NTB)]
            for d in range(NDF):
                w2t = w2sb.tile([P, C], BF, tag="w2")
                nc.gpsimd.dma_start(out=w2t[:], in_=w2_d[d * P:(d + 1) * P, :])
                if d == 4:
                    nc.gpsimd.dma_start(out=g2b[:], in_=bcast_rows(g2_d, C))
                for tb in range(NTB):
                    for half in range(2):
                        nc.tensor.matmul(
                            fts[tb][:, half * 512:(half + 1) * 512],
                            ats[d][:, tb * P:(tb + 1) * P],
                            w2t[:, half * 512:(half + 1) * 512],
                            start=(d == 0), stop=(d == NDF - 1))
            for tb in range(NTB):
                r2 = work.tile([P, C], FP, tag="r2")
                nc.vector.tensor_add(r2[:], fts[tb][:], hb2s[tb][:])
                o = work.tile([P, C], FP, tag="outt")
                rmsnorm_to(o, r2, g2b)
                eng = nc.scalar if tb % 2 == 0 else nc.sync
                eng.dma_start(out=out_d[tb * P:(tb + 1) * P, :], in_=o[:])
    nc.compile()
    return nc


# --------------------------------------------------------------------------
# host orchestration
# --------------------------------------------------------------------------

_CACHE = {}


def _phase1(B, T, C, DH):
    key = ("p1", B, T, C, DH)
    if key not in _CACHE:
        _CACHE[key] = build_phase1(B, T, C, DH)
    return _CACHE[key]


def _phase2(NTOK, C, DFF):
    key = ("p2", NTOK, C, DFF)
    if key not in _CACHE:
        _CACHE[key] = build_phase2(NTOK, C, DFF)
    return _CACHE[key]


def _run(nc, in_maps, ldw_opt=False):
    import os
    import concourse.bass_utils as _bu
    _bu._ldw_opt_enable = ldw_opt    # DR ldweights are incompatible with it
    trace = bool(os.environ.get("KERNEL_TRACE"))
    kwargs = {}
    if trace:
        _install_ntff_hook_shim()
        tdir = os.environ.get("KERNEL_TRACE_DIR")
        if tdir:
            phase_dir = os.path.join(tdir, f"phase{len(LAST_EXEC_NS)}")
            os.makedirs(phase_dir, exist_ok=True)
            kwargs["tmpdir"] = phase_dir
    res = run_bass_kernel_spmd(nc, in_maps, core_ids=list(range(N_CORES)),
                               trace=trace, **kwargs)
    LAST_EXEC_NS.append(res.exec_time_ns)
    return res.results


def kernel(x, Wq, Wk, Wv, Wo, bo, W1, b1, W2, b2, g1, g2):
    f32 = lambda a: np.ascontiguousarray(np.asarray(a), dtype=np.float32)
    x = f32(x)
    Wq, Wk, Wv, Wo, bo = f32(Wq), f32(Wk), f32(Wv), f32(Wo), f32(bo)
    W1, b1, W2, b2, g1, g2 = f32(W1), f32(b1), f32(W2), f32(b2), f32(g1), f32(g2)

    B, T, C = x.shape
    H, _, DH = Wq.shape
    HP = H // N_CORES           # heads per core (2)
    DA = DH + 1
    LAST_EXEC_NS.clear()

    # ---- phase 1
    nc1 = _phase1(B, T, C, DH)
    NKK = C // 256
    # [B,T,C] -> [B, n, kk, 128, 2, 512] with channel c = (2*kk+s)*128 + p
    # and token t = n*512 + t'
    xT8 = np.ascontiguousarray(
        x.transpose(0, 2, 1).reshape(B, NKK, 2, 128, T // 512, 512)
        .transpose(0, 4, 1, 3, 2, 5)).astype(F8_NP)
    in1 = []
    for i in range(N_CORES):
        pq = Wq[HP * i:HP * (i + 1)].transpose(1, 0, 2).reshape(C, HP * DH)
        pk = Wk[HP * i:HP * (i + 1)].transpose(1, 0, 2).reshape(C, HP * DH)
        pv = Wv[HP * i:HP * (i + 1)].transpose(1, 0, 2).reshape(C, HP * DH)
        pair = lambda w: np.ascontiguousarray(
            (w * WS).reshape(NKK, 2, 128, HP * DH)
            .transpose(0, 2, 1, 3)).astype(F8_NP)
        in1.append({"xT8": xT8,
                    "wq8": pair(pq),
                    "wk8": pair(pk),
                    "wv8": np.ascontiguousarray(
                        (pv * WS).reshape(2 * NKK, 128, HP * DH)
                    ).astype(F8_NP)})
    res1 = _run(nc1, in1)

    attn = np.empty((B, T, C), np.float32)
    for i in range(N_CORES):
        ot = res1[i]["ot"]                    # [B, HP, DA, T]
        o = ot[:, :, :DH, :]
        den = ot[:, :, DH, :] * WS            # V carries a x32 scale
        on = o / den[:, :, None, :]
        for hh in range(HP):
            hcol = (HP * i + hh) * DH
            attn[:, :, hcol:hcol + DH] = on[:, hh].transpose(0, 2, 1)

    # ---- phase 2
    NTOK = B * T // N_CORES
    nc2 = _phase2(NTOK, C, W1.shape[1])
    xf = x.reshape(B * T, C) + bo             # fold bo into the residual
    af = attn.reshape(B * T, C)
    DFF = W1.shape[1]
    NKC = C // 256
    # Wo in fp8 DoubleRow pairs, x4 scale on both operands (undone by 1/16)
    wo8 = np.ascontiguousarray(
        (Wo * 4.0).reshape(NKC, 2, 128, C).transpose(0, 2, 1, 3)).astype(F8_NP)
    # [C, DFF] -> [g, c, 128, 512] so each (g, c) chunk is DMA-contiguous
    w1_bf = np.ascontiguousarray(
        W1.reshape(C // 128, 128, DFF // 512, 512)
        .transpose(2, 0, 1, 3)).astype(BF_NP)
    w2_bf = W2.astype(BF_NP)
    in2 = []
    for k in range(N_CORES):
        sl = slice(k * NTOK, (k + 1) * NTOK)
        at8 = np.ascontiguousarray(
            (af[sl].T * 4.0).reshape(NKC, 2, 128, NTOK)
            .transpose(0, 2, 1, 3)).astype(F8_NP)
        in2.append({
            "xc": np.ascontiguousarray(xf[sl]),
            "attnT8": at8,
            "wo8": wo8, "w1p": w1_bf, "w2": w2_bf,
            "g1": g1, "g2": g2, "b1": b1, "b2": b2,
        })
    res2 = _run(nc2, in2)
    out = np.concatenate([res2[k]["out"] for k in range(N_CORES)], axis=0)
    return out.reshape(B, T, C)



# revision 36
# speedup vs baseline: 1.5146x; 1.0191x over previous
"""Trainium2 Bass kernel for a dense transformer block, distributed over 8
NeuronCores.

Sharding:
  phase 1 (attention): tensor-parallel over heads — each core computes 2 of
    the 16 heads end-to-end (QKV projections + causal softmax(QK^T)V), and
    returns the unnormalized per-head output O^T together with the softmax
    denominators (obtained via a ones-column appended to V).
  phase 2 (Wo + norms + FFN): data-parallel over tokens — each core handles
    512 of the 4096 token rows with replicated weights.

The host glues the phases: transposes x, normalizes/concats heads, and
re-shards tokens.  All matmuls run as float32r (full-rate fp32 PE mode).
"""

import math
from contextlib import ExitStack

import ml_dtypes
import numpy as np

BF_NP = ml_dtypes.bfloat16

import concourse.bass as bass
import concourse.mybir as mybir
import concourse.tile as tile
from concourse import bacc
from concourse.bass_utils import run_bass_kernel_spmd
from concourse.masks import make_identity, make_upper_triangular

FP = mybir.dt.float32
FPR = mybir.dt.float32r
BF = mybir.dt.bfloat16
AF = mybir.ActivationFunctionType

N_CORES = 8
P = 128
EPS = 1e-6

# exec times (ns) of the most recent kernel() call, one entry per phase, when
# tracing was enabled via BASS_TRACE=1; None entries otherwise.
LAST_EXEC_NS = []


def _install_ntff_hook_shim():
    """Provide antenv.axon_hooks when the image lacks it, so trace=True can
    drive NTFF profiling through libaxon_pjrt's C ABI (same contract as
    trn_boot's step-6 hook). No-op if the real module exists or the .so is
    missing/old."""
    try:
        import antenv.axon_hooks  # noqa: F401
        return
    except ImportError:
        pass
    import contextlib
    import ctypes
    import sys
    import types

    try:
        lib = ctypes.CDLL("/opt/axon/libaxon_pjrt.so")
    except OSError:
        return
    if not hasattr(lib, "axon_start_nrt_profile"):
        return
    lib.axon_start_nrt_profile.argtypes = [
        ctypes.POINTER(ctypes.c_int64), ctypes.c_size_t]
    lib.axon_start_nrt_profile.restype = ctypes.c_int64
    lib.axon_stop_nrt_profile.argtypes = [ctypes.c_char_p]
    lib.axon_stop_nrt_profile.restype = ctypes.c_int64

    @contextlib.contextmanager
    def _hook(output_dir, device_ids):
        import jax
        jax.devices()
        if device_ids:
            ids = (ctypes.c_int64 * len(device_ids))(*device_ids)
            rc = lib.axon_start_nrt_profile(ids, len(device_ids))
        else:
            rc = lib.axon_start_nrt_profile(None, 0)
        if rc != 0:
            raise RuntimeError(f"axon_start_nrt_profile rc={rc}")
        try:
            yield
        finally:
            n = lib.axon_stop_nrt_profile(str(output_dir).encode())
            if n < 0:
                raise RuntimeError(f"axon_stop_nrt_profile rc={n}")

    mod = types.ModuleType("antenv.axon_hooks")
    mod.get_axon_ntff_profile_hook = lambda: _hook

    def set_axon_ntff_profile_hook(h):
        mod.get_axon_ntff_profile_hook = lambda: h

    mod.set_axon_ntff_profile_hook = set_axon_ntff_profile_hook
    import antenv
    antenv.axon_hooks = mod
    sys.modules["antenv.axon_hooks"] = mod


def _fpr(ap):
    return ap.bitcast(FPR)


def _enable_ldw_opt():
    """Flip walrus's --enable-ldw-opt to true so LDWEIGHTS overlaps with
    in-flight matmuls (background weight buffer). Verified by the rel-err
    check; idempotent."""
    import concourse.bass_utils as _bu
    if getattr(_bu, "_ldw_opt_patched", False):
        return
    orig = _bu.run_command

    def patched(cmd, *a, **kw):
        if isinstance(cmd, list) and getattr(_bu, "_ldw_opt_enable", False):
            cmd = ["--enable-ldw-opt=true" if c == "--enable-ldw-opt=false"
                   else c for c in cmd]
        return orig(cmd, *a, **kw)

    _bu.run_command = patched
    _bu._ldw_opt_patched = True


_enable_ldw_opt()


# --------------------------------------------------------------------------
# phase 1: per-core attention over a pair of heads (fp8 + DoubleRow)
# --------------------------------------------------------------------------

PF8 = mybir.dt.float8e4
F8_NP = ml_dtypes.float8_e4m3
WS = 32.0          # q/k/v weight upscale; folded out via exp scale + host div


class _Fillers:
    """FIFO of deferred PE-work emitters, drained one unit at a time into
    the exp-gated attention loop so the tensor engine never idles."""

    def __init__(self):
        self.q = []

    def add(self, fn):
        self.q.append(fn)

    def take(self, n):
        out, self.q = self.q[:n], self.q[n:]
        return out

    def drain(self):
        out, self.q = self.q, []
        return out


def build_phase1(B, T, C, DH):
    HP = 2                      # heads per core
    DA = DH + 1                 # head dim + ones row (softmax denominator)
    DAP = 80                    # DA padded so the DoubleRow pair stride is 16B-aligned
    NKK = C // 256              # DoubleRow contraction pairs
    NST = T // P                # key stripes of 128
    NM = T // 512               # query blocks of 512
    scale = float(C) ** -0.5 / (WS * WS)
    DR = mybir.MatmulPerfMode.DoubleRow

    nc = bacc.Bacc("TRN2", debug=False)
    # token-sliced so each (b, n, kk) chunk is contiguous and the first
    # projection can start after one 128KB DMA
    xT_d = nc.dram_tensor("xT8", [B, NM, NKK, P, 2, 512], PF8,
                          kind="ExternalInput").ap()
    wq_d = nc.dram_tensor("wq8", [NKK, P, 2, HP * DH], PF8,
                          kind="ExternalInput").ap()
    wk_d = nc.dram_tensor("wk8", [NKK, P, 2, HP * DH], PF8,
                          kind="ExternalInput").ap()
    wv_d = nc.dram_tensor("wv8", [2 * NKK, P, HP * DH], PF8,
                          kind="ExternalInput").ap()
    ot_d = nc.dram_tensor("ot", [B, HP, DA, T], FP, kind="ExternalOutput").ap()

    with tile.TileContext(nc) as tc, ExitStack() as ctx:
        const = ctx.enter_context(tc.tile_pool(name="const", bufs=1))
        xp = ctx.enter_context(tc.tile_pool(name="xp", bufs=1))
        wp = ctx.enter_context(tc.tile_pool(name="wp", bufs=1))
        qkp = ctx.enter_context(tc.tile_pool(name="qkp", bufs=1))
        vap = ctx.enter_context(tc.tile_pool(name="vap", bufs=1))
        ptkp = ctx.enter_context(tc.tile_pool(name="ptkp", bufs=6))
        otp = ctx.enter_context(tc.tile_pool(name="otp", bufs=1))
        pp = ctx.enter_context(tc.tile_pool(name="pp", bufs=2, space="PSUM"))
        sp = ctx.enter_context(tc.tile_pool(name="sp", bufs=2, space="PSUM"))
        op = ctx.enter_context(tc.tile_pool(name="op", bufs=1, space="PSUM"))

        # preload the Exp table set before the attention loop needs it
        warm = const.tile([P, 1], FP)
        nc.vector.memset(warm[:], 0.0)
        nc.scalar.activation(warm[:], warm[:], AF.Exp, scale=1.0)

        xts, qts, kts, vaugs, otsbs = {}, {}, {}, {}, {}
        wqs, wks, wvs = [], [], []

        def load_x(b, ns):
            # one DMA per (b, n) covering all kk chunks; rhs/lhsT slices of
            # the combined tile keep the DoubleRow pair layout
            for n in ns:
                t = xp.tile([P, NKK, 2, 512], PF8, tag=f"x{b}_{n}",
                            name=f"x{b}_{n}")
                src_ap = bass.AP(
                    tensor=xT_d.tensor,
                    offset=xT_d.offset + (b * NM + n) * NKK * P * 1024,
                    ap=[[1024, P], [P * 1024, NKK], [512, 2], [1, 512]])
                nc.sync.dma_start(out=t[:], in_=src_ap)
                for kk in range(NKK):
                    xts[(b, n, kk)] = t[:, kk, :, :]

        for b in range(B):
            qts[b] = qkp.tile([P, T], PF8, tag=f"qt{b}", name=f"qt{b}")
            kts[b] = qkp.tile([P, T], PF8, tag=f"kt{b}", name=f"kt{b}")
            va = vap.tile([P, NST // 2, 2, HP, DAP], PF8, tag=f"va{b}",
                          name=f"va{b}")
            nc.vector.memset(va[:], 1.0)    # ones column survives in col DH
            vaugs[b] = va
            for h in range(HP):
                otsbs[(b, h)] = otp.tile([DA, T], FP, tag=f"ot{b}{h}", name=f"ot{b}{h}")

        def emit_qk_chunk(b, proj, n):
            # one 512-token chunk of the Q or K projection (NKK DR matmuls)
            wt = wqs if proj == 0 else wks
            dst = qts[b] if proj == 0 else kts[b]
            ps = pp.tile([P, 512], FP, tag="pp")
            for kk in range(NKK):
                nc.tensor.matmul(
                    ps[:], wt[kk][:],
                    xts[(b, n, kk)],
                    start=(kk == 0), stop=(kk == NKK - 1), perf_mode=DR)
            nc.vector.tensor_copy(dst[:, n * 512:(n + 1) * 512], ps[:])

        def emit_v_stripe(b, s):
            # V^T stripe s directly in [token, head-dim] layout
            n, so = s // 4, (s % 4) * P
            ps = pp.tile([P, 512], FP, tag="pp")
            out = ps[:, 0:HP * DH]
            for c in range(2 * NKK):
                nc.tensor.matmul(
                    out, xts[(b, n, c // 2)][:, c % 2, so:so + P],
                    wvs[c][:],
                    start=(c == 0), stop=(c == 2 * NKK - 1))

            for h in range(HP):
                nc.vector.tensor_copy(
                    vaugs[b][:, s // 2, s % 2, h, 0:DH],
                    ps[:, h * DH:(h + 1) * DH])

        def emit_spv_block(b, m, fillers):
            o_tiles = [op.tile([DA, 512], FP, tag=f"o{h}", name=f"o{h}")
                       for h in range(HP)]
            npairs = 2 * (m + 1)
            for jj in range(npairs):
                ptk = ptkp.tile([P, 2, HP, 512], PF8, tag="ptk")
                for ss in range(2):
                    j = 2 * jj + ss
                    s0 = j * P
                    diag = s0 >= m * 512
                    off = s0 - m * 512 if diag else 0
                    st = sp.tile([P, HP, 512], FP, tag="st")
                    for h in range(HP):
                        hs = slice(h * DH, (h + 1) * DH)
                        nc.tensor.matmul(
                            st[:, h, off:512],
                            kts[b][hs, s0:s0 + P],
                            qts[b][hs, m * 512 + off:(m + 1) * 512],
                            start=True, stop=True,
                            tile_position=(h * DH, 0))
                    if diag and off > 0:
                        nc.gpsimd.memset(ptk[:, ss, :, 0:off], 0.0)
                    nc.scalar.activation(
                        ptk[:, ss, :, off:512], st[:, :, off:512],
                        AF.Exp, scale=scale)
                    if diag:
                        # zero the non-causal triangle of the diagonal 128
                        # cols after the exp, off the ACT critical path
                        for h in range(HP):
                            nc.gpsimd.affine_select(
                                out=ptk[:, ss, h, off:off + P],
                                in_=ptk[:, ss, h, off:off + P],
                                compare_op=mybir.AluOpType.is_ge, fill=0.0,
                                base=0, pattern=[[1, P]],
                                channel_multiplier=-1)
                    for f in fillers.take(1):
                        f()
                for h in range(HP):
                    nc.tensor.matmul(
                        o_tiles[h][:], vaugs[b][:, jj, :, h, 0:DA],
                        ptk[:, :, h, :],
                        start=(jj == 0), stop=(jj == npairs - 1),
                        perf_mode=DR)
            for h in range(HP):
                nc.vector.tensor_copy(
                    otsbs[(b, h)][:, m * 512:(m + 1) * 512], o_tiles[h][:])

        # ---- minimal prelude: only what block (0,0) needs up front,
        # DMA-ordered so the first matmul's operands land first
        load_x(0, [0])
        for kk in range(NKK):
            wq = wp.tile([P, 2, HP * DH], PF8, tag=f"wq{kk}", name=f"wq{kk}")
            nc.sync.dma_start(out=wq[:], in_=wq_d[kk])
            wk = wp.tile([P, 2, HP * DH], PF8, tag=f"wk{kk}", name=f"wk{kk}")
            nc.sync.dma_start(out=wk[:], in_=wk_d[kk])
            wqs.append(wq)
            wks.append(wk)
        for c in range(2 * NKK):
            wv = wp.tile([P, HP * DH], PF8, tag=f"wv{c}", name=f"wv{c}")
            nc.sync.dma_start(out=wv[:], in_=wv_d[c])
            wvs.append(wv)
        emit_qk_chunk(0, 0, 0)
        emit_qk_chunk(0, 1, 0)
        emit_v_stripe(0, 0)
        emit_v_stripe(0, 1)
        load_x(0, [1, 2, 3])
        load_x(1, range(NM))

        # b0/b1 blocks interleaved so the exp (ACT) stream never drains and
        # the PE always has S/PV + filler (V / Q/K projection) work. The
        # filler FIFO order matches each unit's first use in block order:
        # blocks b0m0(4 slots) b0m1(8) b0m2(12) b1m0(4) b0m3(16) b1m1(8)
        # b1m2(12) b1m3(16).
        fill = _Fillers()
        fill.add(lambda: emit_v_stripe(0, 2))
        fill.add(lambda: emit_v_stripe(0, 3))
        fill.add(lambda: emit_qk_chunk(0, 0, 1))
        fill.add(lambda: emit_qk_chunk(0, 1, 1))
        for s in range(4, 8):
            fill.add(lambda s=s: emit_v_stripe(0, s))
        for n in (2, 3):
            fill.add(lambda n=n: emit_qk_chunk(0, 0, n))
            fill.add(lambda n=n: emit_qk_chunk(0, 1, n))
        for s in range(8, 12):
            fill.add(lambda s=s: emit_v_stripe(0, s))
        fill.add(lambda: emit_qk_chunk(1, 0, 0))
        fill.add(lambda: emit_qk_chunk(1, 1, 0))
        for s in range(4):
            fill.add(lambda s=s: emit_v_stripe(1, s))
        fill.add(lambda: emit_qk_chunk(1, 0, 1))
        fill.add(lambda: emit_qk_chunk(1, 1, 1))
        for s in range(12, NST):
            fill.add(lambda s=s: emit_v_stripe(0, s))
        for n in (2, 3):
            fill.add(lambda n=n: emit_qk_chunk(1, 0, n))
            fill.add(lambda n=n: emit_qk_chunk(1, 1, n))
        for s in range(4, NST):
            fill.add(lambda s=s: emit_v_stripe(1, s))

        for b, m in ((0, 0), (0, 1), (0, 2), (1, 0),
                     (0, 3), (1, 1), (1, 2), (1, 3)):
            emit_spv_block(b, m, fill)
            for h in range(HP):
                nc.gpsimd.dma_start(
                    out=ot_d[b, h, :, m * 512:(m + 1) * 512],
                    in_=otsbs[(b, h)][:, m * 512:(m + 1) * 512])
        for f in fill.drain():
            f()
    nc.compile()
    return nc


def build_phase1_v1(B, T, C, DH):
    HP = 2                      # heads per core
    DA = DH + 1                 # head dim + ones row (softmax denominator)
    NCC = C // P                # contraction chunks
    NT = T // P                 # key/value blocks of 128
    NQ = T // 512               # query chunks of 512
    NK = T // 1024              # query tiles of 1024
    scale = float(C) ** -0.5    # NOTE: reference scales by C**-0.5, not DH

    nc = bacc.Bacc("TRN2", debug=False)
    xT_d = nc.dram_tensor("xT", [B, C, T], BF, kind="ExternalInput").ap()
    wq_d = nc.dram_tensor("wq", [C, HP * DH], BF, kind="ExternalInput").ap()
    wk_d = nc.dram_tensor("wk", [C, HP * DH], BF, kind="ExternalInput").ap()
    wv_d = nc.dram_tensor("wv", [C, HP * DH], BF, kind="ExternalInput").ap()
    ot_d = nc.dram_tensor("ot", [B, HP, DA, T], FP, kind="ExternalOutput").ap()

    with tile.TileContext(nc) as tc, ExitStack() as ctx:
        const = ctx.enter_context(tc.tile_pool(name="const", bufs=1))
        xpool = ctx.enter_context(tc.tile_pool(name="xp", bufs=1))
        wpool = ctx.enter_context(tc.tile_pool(name="wp", bufs=1))
        qk_pool = ctx.enter_context(tc.tile_pool(name="qk", bufs=2))
        vt_pool = ctx.enter_context(tc.tile_pool(name="vtp", bufs=2))
        vaug_pool = ctx.enter_context(tc.tile_pool(name="vaug", bufs=2))
        pt_pool = ctx.enter_context(tc.tile_pool(name="pt", bufs=4))
        ot_pool = ctx.enter_context(tc.tile_pool(name="otp", bufs=2))

        # additive mask for the diagonal 128x128 block of S^T [s', q']:
        # 0 where q' >= s' (causal-valid), -1e30 where q' < s'
        negmask = const.tile([P, P], FP)
        nc.gpsimd.memset(negmask[:], 0.0)
        nc.gpsimd.affine_select(
            out=negmask[:], in_=negmask[:],
            compare_op=mybir.AluOpType.is_ge, fill=-1e30,
            base=0, pattern=[[1, P]], channel_multiplier=-1)
        ident = const.tile([P, P], BF)
        make_identity(nc, ident[:])
        identf = const.tile([P, P], FP)
        make_identity(nc, identf[:])
        ones_col = const.tile([P, NT * HP, 1], FP)
        nc.vector.memset(ones_col[:], 1.0)

        # weight chunks, loaded once
        wts = {}
        for name, src in (("q", wq_d), ("k", wk_d), ("v", wv_d)):
            wts[name] = []
            for c in range(NCC):
                t = wpool.tile([P, HP * DH], BF, tag=f"w{name}{c}")
                nc.sync.dma_start(out=t[:], in_=src[c * P:(c + 1) * P, :])
                wts[name].append(t)

        for b in range(B):
            xts = []
            for c in range(NCC):
                xt = xpool.tile([P, T], BF, tag=f"x{c}")
                nc.sync.dma_start(out=xt[:], in_=xT_d[b, c * P:(c + 1) * P, :])
                xts.append(xt)

            qt = qk_pool.tile([P, T], BF, tag="qt")
            kt = qk_pool.tile([P, T], BF, tag="kt")
            vaug = vaug_pool.tile([P, NT * HP, DA], BF, tag="vaug")
            # ones column per head-block (softmax denominator row of O^T)
            nc.vector.tensor_copy(vaug[:, :, DA - 1:DA], ones_col[:])

            with tc.tile_pool(name="proj_ps", bufs=3, space="PSUM") as proj_ps, \
                 tc.tile_pool(name="vt_ps", bufs=2, space="PSUM") as vt_ps:
                for wt, dst in ((wts["q"], qt), (wts["k"], kt)):
                    for n in range(NQ):
                        ps = proj_ps.tile([P, 512], FP, tag="proj")
                        for c in range(NCC):
                            nc.tensor.matmul(
                                ps[:], wt[c][:], xts[c][:, n * 512:(n + 1) * 512],
                                start=(c == 0), stop=(c == NCC - 1))
                        nc.vector.tensor_copy(dst[:, n * 512:(n + 1) * 512], ps[:])
                # V, then transpose into [s, d] layout with ones columns
                for n in range(NQ):
                    ps = proj_ps.tile([P, 512], FP, tag="proj")
                    for c in range(NCC):
                        nc.tensor.matmul(
                            ps[:], wts["v"][c][:], xts[c][:, n * 512:(n + 1) * 512],
                            start=(c == 0), stop=(c == NCC - 1))
                    vt = vt_pool.tile([P, 512], BF, tag="vt")
                    nc.vector.tensor_copy(vt[:], ps[:])
                    for u in range(4):
                        j = 4 * n + u
                        tp = vt_ps.tile([P, P], BF, tag="vtp")
                        nc.tensor.transpose(tp[:], vt[:, u * P:(u + 1) * P], ident[:])
                        nc.vector.tensor_copy(
                            vaug[:, j * HP, 0:DH], tp[:, 0:DH])
                        nc.vector.tensor_copy(
                            vaug[:, j * HP + 1, 0:DH], tp[:, DH:2 * DH])

            with tc.tile_pool(name="s_ps", bufs=2, space="PSUM") as s_ps, \
                 tc.tile_pool(name="o_ps", bufs=1, space="PSUM") as o_ps:
                ot_sbs = [ot_pool.tile([DA, T], FP, tag=f"ot{h}", name=f"ot{h}")
                          for h in range(HP)]
                for k in range(NK):
                    q_lo = 1024 * k
                    q_hi = 1024 * (k + 1)
                    o_tiles = [o_ps.tile([DA, 1024], FP, tag=f"o{h}", name=f"o{h}")
                               for h in range(HP)]
                    for j in range(8 * (k + 1)):
                        s0 = j * P
                        a0 = max(s0, q_lo)
                        # 512-grid chunks of the valid q range in this stripe
                        chunks = []
                        m0 = a0 // 512
                        for m in range(m0, q_hi // 512):
                            a = max(a0, m * 512)
                            e = (m + 1) * 512
                            chunks.append((a, e))
                        stl = [s_ps.tile([P, 1024], FP, tag="s", name="s")
                               for _ in range(HP)]
                        # emit head pairs adjacently: rows 0-63 (head A) and
                        # 64-127 (head B) run concurrently in the PE array
                        for (a, e) in chunks:
                            for h in range(HP):
                                hs = slice(h * DH, (h + 1) * DH)
                                nc.tensor.matmul(
                                    stl[h][:, a - q_lo:e - q_lo],
                                    kt[hs, s0:s0 + P], qt[hs, a:e],
                                    start=True, stop=True,
                                    tile_position=(h * DH, 0))
                        if q_lo <= s0:
                            for h in range(HP):
                                # diagonal block: additive causal mask
                                nc.vector.tensor_add(
                                    stl[h][:, s0 - q_lo:s0 - q_lo + P],
                                    stl[h][:, s0 - q_lo:s0 - q_lo + P],
                                    negmask[:])
                        for h in range(HP):
                            ptk = pt_pool.tile([P, 1024], BF, tag="pt")
                            nc.scalar.activation(
                                ptk[:, a0 - q_lo:1024], stl[h][:, a0 - q_lo:1024],
                                AF.Exp, scale=scale)
                            va = vaug[:, j * HP + h, :]
                            for (a, e) in chunks:
                                last_j = e // P - 1
                                nc.tensor.matmul(
                                    o_tiles[h][:, a - q_lo:e - q_lo],
                                    va, ptk[:, a - q_lo:e - q_lo],
                                    start=(j == 0), stop=(j == last_j))
                    for h in range(HP):
                        nc.vector.tensor_copy(
                            ot_sbs[h][:, q_lo:q_hi], o_tiles[h][:])
                for h in range(HP):
                    nc.sync.dma_start(out=ot_d[b, h], in_=ot_sbs[h][:])
    nc.compile()
    return nc


# --------------------------------------------------------------------------
# phase 2: per-core Wo projection + residual + rmsnorm + FFN + rmsnorm
# --------------------------------------------------------------------------

def build_phase2(NTOK, C, DFF):
    NTB = NTOK // P             # 4 token tiles of 128
    NCH = C // P                # 8 channel chunks
    NDF = DFF // P              # 32 dff chunks
    NG = DFF // 512             # 8 dff groups of 512

    NKC = C // 256              # DoubleRow contraction pairs for Wo
    DRM = mybir.MatmulPerfMode.DoubleRow
    nc = bacc.Bacc("TRN2", debug=False)
    xc_d = nc.dram_tensor("xc", [NTOK, C], FP, kind="ExternalInput").ap()
    at_d = nc.dram_tensor("attnT8", [NKC, P, 2, NTOK], PF8,
                          kind="ExternalInput").ap()
    wo_d = nc.dram_tensor("wo8", [NKC, P, 2, C], PF8,
                          kind="ExternalInput").ap()
    # host-permuted W1: [g, c, 128, 512] so each (g, c) chunk is contiguous
    w1_d = nc.dram_tensor("w1p", [NG, NCH, P, 512], BF, kind="ExternalInput").ap()
    w2_d = nc.dram_tensor("w2", [DFF, C], BF, kind="ExternalInput").ap()
    g1_d = nc.dram_tensor("g1", [C], FP, kind="ExternalInput").ap()
    g2_d = nc.dram_tensor("g2", [C], FP, kind="ExternalInput").ap()
    b1_d = nc.dram_tensor("b1", [DFF], FP, kind="ExternalInput").ap()
    b2_d = nc.dram_tensor("b2", [C], FP, kind="ExternalInput").ap()
    out_d = nc.dram_tensor("out", [NTOK, C], FP, kind="ExternalOutput").ap()

    def bcast_rows(src_ap, cols):
        # DRAM vector [cols] -> [P, cols] (same row in every partition)
        return bass.AP(tensor=src_ap.tensor, offset=src_ap.offset,
                       ap=[[0, P], [1, cols]])

    with tile.TileContext(nc) as tc, ExitStack() as ctx:
        const = ctx.enter_context(tc.tile_pool(name="const", bufs=1))
        work = ctx.enter_context(tc.tile_pool(name="work", bufs=2))
        stats = ctx.enter_context(tc.tile_pool(name="stats", bufs=4))
        h_pool = ctx.enter_context(tc.tile_pool(name="hp", bufs=1))
        hb2p = ctx.enter_context(tc.tile_pool(name="hb2p", bufs=1))
        ht_pool = ctx.enter_context(tc.tile_pool(name="htp", bufs=1))
        at_pool = ctx.enter_context(tc.tile_pool(name="atp", bufs=1))
        w2sb = ctx.enter_context(tc.tile_pool(name="w2sb", bufs=10))

        ident = const.tile([P, P], BF)
        make_identity(nc, ident[:])
        identf = const.tile([P, P], FP)
        make_identity(nc, identf[:])
        eps_t = const.tile([P, 1], FP)
        nc.vector.memset(eps_t[:], EPS)
        sc16 = const.tile([P, 1], FP)
        nc.vector.memset(sc16[:], 1.0 / 16.0)   # undo the x4*x4 Wo fp8 scales
        # preload the Sqrt/Square table set before stage-0 norms need it
        warm = stats.tile([P, 1], FP, tag="warm")
        nc.scalar.activation(warm[:], eps_t[:], AF.Sqrt, scale=1.0)
        # g1 is needed first (stage-0 norms); the other broadcasts are
        # issued later, near their first consumer, to keep early HBM
        # bandwidth for the stage-0 operands
        g1b = const.tile([P, C], FP)
        nc.scalar.dma_start(out=g1b[:], in_=bcast_rows(g1_d, C))
        g2b = const.tile([P, C], FP)
        b2b = const.tile([P, C], FP)
        # b1 reshaped to [128, NDF] with b1r[p, d] = b1[d*128+p]: load as
        # [NDF, 128] (contiguous rows) and transpose once on the PE
        b1sq = const.tile([NDF, P], FP)
        b1r = const.tile([P, NDF], FP)

        def rmsnorm_to(dst, src, gb):
            # dst = src * rsqrt(mean(src^2) + eps) * gb
            sq = work.tile([P, C], FP, tag="sq")
            ssum = stats.tile([P, 1], FP, tag="ssum")
            nc.scalar.activation(sq[:], src[:], AF.Square, accum_out=ssum[:])
            rstd = stats.tile([P, 1], FP, tag="rstd")
            nc.scalar.activation(rstd[:], ssum[:], AF.Sqrt,
                                 scale=1.0 / C, bias=eps_t[:])
            rinv = stats.tile([P, 1], FP, tag="rinv")
            nc.vector.reciprocal(rinv[:], rstd[:])
            nc.vector.scalar_tensor_tensor(
                dst[:], src[:], rinv[:], gb[:],
                op0=mybir.AluOpType.mult, op1=mybir.AluOpType.mult)

        # ---- stage 0: o = attnT^T @ Wo; r1 = x + o; h = rmsnorm(r1)*g1; hT
        hs = []
        hts_all = ht_pool.tile([P, NCH, NTOK], BF, tag="hts", name="hts_all")
        hts = [hts_all[:, c, :] for c in range(NCH)]
        with tc.tile_pool(name="o_ps", bufs=2, space="PSUM") as o_ps, \
             tc.tile_pool(name="t_ps", bufs=2, space="PSUM") as t_ps, \
             tc.tile_pool(name="wop", bufs=NCH) as wop, \
             tc.tile_pool(name="atsp", bufs=NCH) as atsp, \
             tc.tile_pool(name="xcp", bufs=1) as xcp, \
             tc.tile_pool(name="hbf", bufs=2) as hbf:
            atts, wots = [], []
            for c in range(NKC):
                att = atsp.tile([P, 2, NTOK], PF8, tag="at", name="at")
                nc.sync.dma_start(out=att[:], in_=at_d[c])
                wot = wop.tile([P, 2, C], PF8, tag="wo", name="wo")
                nc.sync.dma_start(out=wot[:], in_=wo_d[c])
                atts.append(att)
                wots.append(wot)
            xcs = []
            for tb in range(NTB):
                t = xcp.tile([P, C], FP, tag=f"xc{tb}")
                nc.sync.dma_start(out=t[:], in_=xc_d[tb * P:(tb + 1) * P, :])
                xcs.append(t)

            def wo_mms(tb, o):
                for c in range(NKC):
                    for half in range(2):
                        nc.tensor.matmul(
                            o[:, half * 512:(half + 1) * 512],
                            atts[c][:, :, tb * P:(tb + 1) * P],
                            wots[c][:, :, half * 512:(half + 1) * 512],
                            start=(c == 0), stop=(c == NKC - 1),
                            perf_mode=DRM)

            def norm_h(tb, o):
                r1 = work.tile([P, C], FP, tag="r1")
                nc.vector.scalar_tensor_tensor(
                    r1[:], o[:], sc16[:], xcs[tb][:],
                    op0=mybir.AluOpType.mult, op1=mybir.AluOpType.add)
                h = h_pool.tile([P, C], FP, tag=f"h{tb}")
                rmsnorm_to(h, r1, g1b)
                hs.append(h)
                hb = hbf.tile([P, C], BF, tag="hbf")
                nc.vector.tensor_copy(hb[:], h[:])
                return hb

            def trans_h(tb, hb):
                # 8 PE transposes into one PSUM tile, ONE batched DVE copy
                tp = t_ps.tile([P, NCH, P], BF, tag="tp")
                for c in range(NCH):
                    nc.tensor.transpose(
                        tp[:, c, :], hb[:, c * P:(c + 1) * P], ident[:])
                nc.vector.tensor_copy(
                    hts_all[:, :, tb * P:(tb + 1) * P], tp[:])

            nc.gpsimd.dma_start(
                out=b1sq[:],
                in_=bass.AP(tensor=b1_d.tensor, offset=b1_d.offset,
                            ap=[[P, NDF], [1, P]]))
            tb1 = t_ps.tile([P, NDF], FP, tag="tb1", name="tb1")
            nc.tensor.transpose(tb1[:], b1sq[:], identf[0:NDF, 0:NDF])
            nc.vector.tensor_copy(b1r[:], tb1[:])

            # pipeline: Wo(tb) | Wo(tb+1) + trans(tb) | ... so the PE never
            # waits on the norm chain
            o_tiles, hbs = [], []
            for tb in range(NTB):
                o = o_ps.tile([P, C], FP, tag="o")
                wo_mms(tb, o)
                o_tiles.append(o)
                hbs.append(norm_h(tb, o))
                if tb >= 1:
                    trans_h(tb - 1, hbs[tb - 1])
            trans_h(NTB - 1, hbs[NTB - 1])

        # ---- stage 2: aT = silu(W1^T @ h^T + b1)  [dff, tok] bf16
        ats = []
        with tc.tile_pool(name="a_ps", bufs=8, space="PSUM") as a_ps, \
             tc.tile_pool(name="w1p", bufs=24) as w1p, \
             tc.tile_pool(name="sgp", bufs=3) as sgp:
            for g in range(NG):
                w1ts = []
                for c in range(NCH):
                    w1t = w1p.tile([P, 512], BF, tag="w1")
                    nc.gpsimd.dma_start(out=w1t[:], in_=w1_d[g, c])
                    w1ts.append(w1t)
                if g == 2:
                    nc.gpsimd.dma_start(out=b2b[:], in_=bcast_rows(b2_d, C))
                aps = [a_ps.tile([P, NTOK], FP, tag="a", name="a")
                       for _ in range(4)]
                for c in range(NCH):
                    for u in range(4):
                        nc.tensor.matmul(
                            aps[u][:], w1ts[c][:, u * P:(u + 1) * P],
                            hts[c],
                            start=(c == 0), stop=(c == NCH - 1))
                for u in range(4):
                    d = 4 * g + u
                    sg = sgp.tile([P, NTOK], FP, tag="sg")
                    nc.scalar.activation(sg[:], aps[u][:], AF.Sigmoid,
                                         bias=b1r[:, d:d + 1], scale=1.0)
                    at_t = at_pool.tile([P, NTOK], BF, tag=f"at{d}")
                    # silu(z) for z = a + b1: (a + b1) * sigmoid(a + b1)
                    nc.vector.scalar_tensor_tensor(
                        at_t[:], aps[u][:], b1r[:, d:d + 1], sg[:],
                        op0=mybir.AluOpType.add, op1=mybir.AluOpType.mult)
                    ats.append(at_t)

        # hb[tb] = h + b2, precomputed on DVE while the PE runs W2
        hb2s = []
        for tb in range(NTB):
            hb2 = hb2p.tile([P, C], FP, tag=f"hb2_{tb}")
            nc.vector.tensor_add(hb2[:], hs[tb][:], b2b[:])
            hb2s.append(hb2)

        # preload the Sqrt table set during W2 so the tail norm doesn't
        # pay the ~2.7us table switch
        dummy = stats.tile([P, 1], FP, tag="dummy")
        nc.scalar.activation(dummy[:], eps_t[:], AF.Sqrt, scale=1.0)

        # ---- stage 3: f = aT^T @ W2; r2 = h + b2 + f; out = rmsnorm(r2)*g2
        with tc.tile_pool(name="f_ps", bufs=1, space="PSUM") as f_ps:
            fts = [f_ps.tile([P, C], FP, tag=f"f{tb}", name=f"f{tb}")
                   for tb in range(NTB)]
            for d in range(NDF):
                w2t = w2sb.tile([P, C], BF, tag="w2")
                nc.gpsimd.dma_start(out=w2t[:], in_=w2_d[d * P:(d + 1) * P, :])
                if d == 4:
                    nc.gpsimd.dma_start(out=g2b[:], in_=bcast_rows(g2_d, C))
                for tb in range(NTB):
                    for half in range(2):
                        nc.tensor.matmul(
                            fts[tb][:, half * 512:(half + 1) * 512],
                            ats[d][:, tb * P:(tb + 1) * P],
                            w2t[:, half * 512:(half + 1) * 512],
                            start=(d == 0), stop=(d == NDF - 1))
            for tb in range(NTB):
                r2 = work.tile([P, C], FP, tag="r2")
                nc.vector.tensor_add(r2[:], fts[tb][:], hb2s[tb][:])
                o = work.tile([P, C], FP, tag="outt")
                rmsnorm_to(o, r2, g2b)
                eng = nc.scalar if tb % 2 == 0 else nc.sync
                eng.dma_start(out=out_d[tb * P:(tb + 1) * P, :], in_=o[:])
    nc.compile()
    return nc


# --------------------------------------------------------------------------
# host orchestration
# --------------------------------------------------------------------------

_CACHE = {}


def _phase1(B, T, C, DH):
    key = ("p1", B, T, C, DH)
    if key not in _CACHE:
        _CACHE[key] = build_phase1(B, T, C, DH)
    return _CACHE[key]


def _phase2(NTOK, C, DFF):
    key = ("p2", NTOK, C, DFF)
    if key not in _CACHE:
        _CACHE[key] = build_phase2(NTOK, C, DFF)
    return _CACHE[key]


def _run(nc, in_maps, ldw_opt=False):
    import os
    import concourse.bass_utils as _bu
    _bu._ldw_opt_enable = ldw_opt    # DR ldweights are incompatible with it
    trace = bool(os.environ.get("KERNEL_TRACE"))
    kwargs = {}
    if trace:
        _install_ntff_hook_shim()
        tdir = os.environ.get("KERNEL_TRACE_DIR")
        if tdir:
            phase_dir = os.path.join(tdir, f"phase{len(LAST_EXEC_NS)}")
            os.makedirs(phase_dir, exist_ok=True)
            kwargs["tmpdir"] = phase_dir
    res = run_bass_kernel_spmd(nc, in_maps, core_ids=list(range(N_CORES)),
                               trace=trace, **kwargs)
    LAST_EXEC_NS.append(res.exec_time_ns)
    return res.results


def kernel(x, Wq, Wk, Wv, Wo, bo, W1, b1, W2, b2, g1, g2):
    f32 = lambda a: np.ascontiguousarray(np.asarray(a), dtype=np.float32)
    x = f32(x)
    Wq, Wk, Wv, Wo, bo = f32(Wq), f32(Wk), f32(Wv), f32(Wo), f32(bo)
    W1, b1, W2, b2, g1, g2 = f32(W1), f32(b1), f32(W2), f32(b2), f32(g1), f32(g2)

    B, T, C = x.shape
    H, _, DH = Wq.shape
    HP = H // N_CORES           # heads per core (2)
    DA = DH + 1
    LAST_EXEC_NS.clear()

    # ---- phase 1
    nc1 = _phase1(B, T, C, DH)
    NKK = C // 256
    # [B,T,C] -> [B, n, kk, 128, 2, 512] with channel c = (2*kk+s)*128 + p
    # and token t = n*512 + t'
    xT8 = np.ascontiguousarray(
        x.transpose(0, 2, 1).reshape(B, NKK, 2, 128, T // 512, 512)
        .transpose(0, 4, 1, 3, 2, 5)).astype(F8_NP)
    in1 = []
    for i in range(N_CORES):
        pq = Wq[HP * i:HP * (i + 1)].transpose(1, 0, 2).reshape(C, HP * DH)
        pk = Wk[HP * i:HP * (i + 1)].transpose(1, 0, 2).reshape(C, HP * DH)
        pv = Wv[HP * i:HP * (i + 1)].transpose(1, 0, 2).reshape(C, HP * DH)
        pair = lambda w: np.ascontiguousarray(
            (w * WS).reshape(NKK, 2, 128, HP * DH)
            .transpose(0, 2, 1, 3)).astype(F8_NP)
        in1.append({"xT8": xT8,
                    "wq8": pair(pq),
                    "wk8": pair(pk),
                    "wv8": np.ascontiguousarray(
                        (pv * WS).reshape(2 * NKK, 128, HP * DH)
                    ).astype(F8_NP)})
    res1 = _run(nc1, in1)

    attn = np.empty((B, T, C), np.float32)
    for i in range(N_CORES):
        ot = res1[i]["ot"]                    # [B, HP, DA, T]
        o = ot[:, :, :DH, :]
        den = ot[:, :, DH, :] * WS            # V carries a x32 scale
        on = o / den[:, :, None, :]
        for hh in range(HP):
            hcol = (HP * i + hh) * DH
            attn[:, :, hcol:hcol + DH] = on[:, hh].transpose(0, 2, 1)

    # ---- phase 2
    NTOK = B * T // N_CORES
    nc2 = _phase2(NTOK, C, W1.shape[1])
    xf = x.reshape(B * T, C) + bo             # fold bo into the residual
    af = attn.reshape(B * T, C)
    DFF = W1.shape[1]
    NKC = C // 256
    # Wo in fp8 DoubleRow pairs, x4 scale on both operands (undone by 1/16)
    wo8 = np.ascontiguousarray(
        (Wo * 4.0).reshape(NKC, 2, 128, C).transpose(0, 2, 1, 3)).astype(F8_NP)
    # [C, DFF] -> [g, c, 128, 512] so each (g, c) chunk is DMA-contiguous
    w1_bf = np.ascontiguousarray(
        W1.reshape(C // 128, 128, DFF // 512, 512)
        .transpose(2, 0, 1, 3)).astype(BF_NP)
    w2_bf = W2.astype(BF_NP)
    in2 = []
    for k in range(N_CORES):
        sl = slice(k * NTOK, (k + 1) * NTOK)
        at8 = np.ascontiguousarray(
            (af[sl].T * 4.0).reshape(NKC, 2, 128, NTOK)
            .transpose(0, 2, 1, 3)).astype(F8_NP)
        in2.append({
            "xc": np.ascontiguousarray(xf[sl]),
            "attnT8": at8,
            "wo8": wo8, "w1p": w1_bf, "w2": w2_bf,
            "g1": g1, "g2": g2, "b1": b1, "b2": b2,
        })
    res2 = _run(nc2, in2)
    out = np.concatenate([res2[k]["out"] for k in range(N_CORES)], axis=0)
    return out.reshape(B, T, C)

